# revision 1
# baseline (speedup 1.0000x reference)
"""Distributed 2-layer GCN + graph pooling + MLP head on 8 TRN2 NeuronCores.

Collective-free data-parallel strategy (per the sharding hint):
  - Graphs (and their nodes, contiguously -- node2graph is sorted) are
    partitioned into 8 shards with ~equal node counts. Weights replicated.
  - Each core owns the edges whose dst node it owns. Layer 2 needs
    h1[src] for those edges; instead of an AllGather, each core computes
    layer 1 *locally* for exactly the source nodes its edges reference
    (~40% of all nodes). No collectives -> no cross-core barrier: each
    core's NEFF window is its own compute, immune to start-time skew.
  - Layer 1 aggregates raw *features* (segment_sum commutes with the
    linear map), so its edge gather moves 128-dim rows. Sources are
    gathered from the full feature table (raw node order, padded to
    100096 rows) via batched GPSIMD dma_gather with int16 indices;
    the row space is split into 4 windows of <=32768 rows.
  - h1 (fp16) for the needed ~40k sources is written to a core-local
    DRAM table; layer 2 gathers 512-dim rows from it (2 windows).
  - Segment-sum on chip: a 0/1 indicator built by DVE (is_equal vs an
    iota row) turns each 128-edge chunk into PE matmuls:
    aggT[f, n] += G_chunk[:, f].T @ I_chunk[:, n].
  - Graph mean-pooling is another indicator matmul with 1/count weights
    (host-precomputed fp16), fused after layer 2 per node block.
  - The MLP head runs feature-major so biases are per-partition.

Device compute fp16 (PSUM fp32); biases fp32; output fp32.
"""

import sys

sys.path.insert(0, "/opt/trn_rl_repo")

import numpy as np

import concourse.bass as bass
import concourse.mybir as mybir
import concourse.tile as tile
from concourse import bacc
from concourse.bass_utils import run_bass_kernel_spmd

P = 128
NCORES = 8
IN_DIM = 128
HID = 512
N_DESC = 200
N_CLASSES = 2
U1 = 512  # padded head hidden 1 (500 -> 512)
U2 = 128  # padded head hidden 2 (100 -> 128)
NPAD = 100096  # feature table rows (100000 padded to 128)
WSZ1 = 32768  # feature-table gather window
NWIN1 = 4  # ceil(100096 / 32768)
WSZ2 = 32768  # h1-table gather window
BUDGET1 = 40  # chunk slots per layer-1 gather group
BUDGET2 = 20  # chunk slots per layer-2 gather group

F16 = mybir.dt.float16
F32 = mybir.dt.float32
I16 = mybir.dt.int16


class Prep:
    pass


def _mk_schedule(cnt_rbw, nblocks, nwin, budget):
    """Uniform (across cores) chunk schedule per (block, window).

    Returns (group_sched, slot_of_bw, TOTC, Cgmax, chunks_bw)."""
    chunks_bw = (cnt_rbw.max(axis=0) + P - 1) // P  # [nblocks, nwin]
    empty = chunks_bw.sum(axis=1) == 0
    chunks_bw[empty, 0] = 1  # >=1 chunk per block (zeroes psum)

    per_block = chunks_bw.sum(axis=1)
    ggroups = []
    b = 0
    while b < nblocks:
        b2 = b
        csum = 0
        while b2 < nblocks and (b2 == b or csum + per_block[b2] <= budget):
            csum += per_block[b2]
            b2 += 1
        ggroups.append((b, b2))
        b = b2

    slot_of_bw = np.full((nblocks, nwin), -1, dtype=np.int64)
    group_sched = []
    slot = 0
    for b0, b1 in ggroups:
        base = slot
        runs = []
        blocks = {bb: [] for bb in range(b0, b1)}
        for w in range(nwin):
            c0 = slot
            for bb in range(b0, b1):
                nbw = int(chunks_bw[bb, w])
                if nbw:
                    slot_of_bw[bb, w] = slot
                    for k in range(nbw):
                        blocks[bb].append(slot + k)
                    slot += nbw
            if slot > c0:
                runs.append((w, c0, slot - c0))
        group_sched.append(dict(b0=b0, b1=b1, base=base, runs=runs, blocks=blocks))
    TOTC = slot
    Cgmax = max(s["runs"][-1][1] + s["runs"][-1][2] - s["base"] for s in group_sched)
    return group_sched, slot_of_bw, TOTC, Cgmax, chunks_bw


def _place_edges(slot_of_bw, blk, win, dpl_mod, idx, TOTC, nblocks, nwin):
    """Edge placement for one core -> (idx_slot [TOTC,P], dstc [P,TOTC])."""
    idx_slot = np.zeros((TOTC, P), dtype=np.int16)
    dstc = np.full((P, TOTC), -1.0, dtype=np.float16)
    keys = blk * nwin + win
    order = np.lexsort((keys,))
    blk, win, dpl_mod, idx, keys = (
        blk[order], win[order], dpl_mod[order], idx[order], keys[order])
    start_of = np.concatenate(
        [[0], np.cumsum(np.bincount(keys, minlength=nblocks * nwin))])
    pos_in_bw = np.arange(len(blk)) - start_of[keys]
    slots = slot_of_bw[blk, win] + pos_in_bw // P
    part = pos_in_bw % P
    idx_slot[slots, part] = idx.astype(np.int16)
    dstc[part, slots] = dpl_mod.astype(np.float32)
    return idx_slot, dstc


def _pack_idx16(idx_slot, group_sched, TOTC):
    """Wrap slot-major int16 indices into the dma_gather layout [128, TOTC*8]."""
    idx16 = np.zeros((16, TOTC * 8), dtype=np.int16)
    for gs in group_sched:
        for w, c0, n in gs["runs"]:
            flat = idx_slot[c0 : c0 + n, :].reshape(-1)
            wrapped = flat.reshape(-1, 16).T  # [16, n*8]
            idx16[:, c0 * 8 : (c0 + n) * 8] = wrapped
    return np.tile(idx16, (8, 1))  # replicate to 128 partitions


# --------------------------------------------------------------------------
# Host-side preprocessing: partition, pad, schedule
# --------------------------------------------------------------------------
def _preprocess(features, descriptors, src, dst, node2graph):
    pr = Prep()
    N = features.shape[0]
    B = descriptors.shape[0]

    n2g = np.asarray(node2graph).astype(np.int64)
    src = np.asarray(src).astype(np.int64)
    dst = np.asarray(dst).astype(np.int64)

    gstart = np.searchsorted(n2g, np.arange(B + 1))  # node range per graph

    # partition graphs into NCORES shards with ~equal node counts
    cuts = np.searchsorted(gstart, (np.arange(1, NCORES) * N) // NCORES)
    gcuts = np.concatenate([[0], cuts, [B]])

    # per-core pool groups of <=128 graphs; group nodes padded to 128-blocks
    core_groups = []
    for r in range(NCORES):
        g0, g1 = gcuts[r], gcuts[r + 1]
        groups = []
        g = g0
        while g < g1:
            ge = min(g + P, g1)
            groups.append((g, ge))
            g = ge
        if not groups:
            groups = [(g0, g0)]
        core_groups.append(groups)
    NG = max(len(gr) for gr in core_groups)

    blocks_per_group_core = np.zeros((NCORES, NG), dtype=np.int64)
    for r in range(NCORES):
        for gi, (g0, g1) in enumerate(core_groups[r]):
            nn = gstart[g1] - gstart[g0]
            blocks_per_group_core[r, gi] = max((nn + P - 1) // P, 1)
    bpg = blocks_per_group_core.max(axis=0)
    NB = int(bpg.sum())
    block_group = np.repeat(np.arange(NG), bpg)

    # padded-local index + owner of each node (layer-2 / pooling space)
    plocal = np.zeros(N, dtype=np.int64)
    owner = np.zeros(N, dtype=np.int64)
    group_base = np.concatenate([[0], np.cumsum(bpg) * P])
    for r in range(NCORES):
        for gi, (g0, g1) in enumerate(core_groups[r]):
            ns, ne = gstart[g0], gstart[g1]
            if ne > ns:
                plocal[ns:ne] = group_base[gi] + np.arange(ne - ns)
                owner[ns:ne] = r

    # per-core needed-source sets (sorted unique srcs of locally-owned edges)
    e_owner = owner[dst]
    uniq_r, l1_edges, l2_edges = [], [], []
    for r in range(NCORES):
        es = np.nonzero(e_owner == r)[0]
        uq = np.unique(src[es])
        uniq_r.append(uq)
        l2_edges.append(es)
    SBLK = max((len(u) + P - 1) // P for u in uniq_r)
    SROWS = SBLK * P
    NWIN2 = (SROWS + WSZ2 - 1) // WSZ2
    assert SROWS <= 2 * 32768, f"h1 table too large for int16 windows: {SROWS}"

    # layer-1 edges per core: all graph edges whose dst is a needed source.
    # Needed sources are ordered by in-degree (descending) so the per-block
    # edge counts are similar across cores -> less chunk padding in the
    # uniform (max-over-cores) schedule.
    cnt1 = np.zeros((NCORES, SBLK, NWIN1), dtype=np.int64)
    l1_dat = []
    rank_maps = []
    for r in range(NCORES):
        uq = uniq_r[r]
        pos = np.searchsorted(uq, dst)
        pos_cl = np.minimum(pos, len(uq) - 1)
        m = uq[pos_cl] == dst  # edge's dst is in the needed set
        e1 = np.nonzero(m)[0]
        orank = pos[e1]
        indeg = np.bincount(orank, minlength=len(uq))
        order = np.argsort(-indeg, kind="stable")
        newrank = np.empty(len(uq), dtype=np.int64)
        newrank[order] = np.arange(len(uq))
        rank_maps.append(newrank)
        rank = newrank[orank]  # aggregation target (local row in h1 table)
        sidx = src[e1]  # gather source (raw feature row)
        blk = rank // P
        win = sidx // WSZ1
        l1_dat.append((blk, win, rank % P, sidx - win * WSZ1))
        np.add.at(cnt1[r], (blk, win), 1)

    sched1, slot1, T1, Cg1, _ = _mk_schedule(cnt1, SBLK, NWIN1, BUDGET1)

    # layer-2 edges per core: local edges; src -> rank in needed set
    cnt2 = np.zeros((NCORES, NB, NWIN2), dtype=np.int64)
    l2_dat = []
    for r in range(NCORES):
        es = l2_edges[r]
        dpl = plocal[dst[es]]
        rank = rank_maps[r][np.searchsorted(uniq_r[r], src[es])]
        blk = dpl // P
        win = rank // WSZ2
        l2_dat.append((blk, win, dpl % P, rank - win * WSZ2))
        np.add.at(cnt2[r], (blk, win), 1)

    sched2, slot2, T2, Cg2, _ = _mk_schedule(cnt2, NB, NWIN2, BUDGET2)

    BMAX = max(n for sched in (sched1, sched2) for gs in sched
               for (_, _, n) in gs["runs"])

    idx16_1 = np.zeros((NCORES, P, T1 * 8), dtype=np.int16)
    dstc1 = np.zeros((NCORES, P, T1), dtype=np.float16)
    idx16_2 = np.zeros((NCORES, P, T2 * 8), dtype=np.int16)
    dstc2 = np.zeros((NCORES, P, T2), dtype=np.float16)
    for r in range(NCORES):
        blk, win, dm, idx = l1_dat[r]
        isl, dc = _place_edges(slot1, blk, win, dm, idx, T1, SBLK, NWIN1)
        idx16_1[r], dstc1[r] = _pack_idx16(isl, sched1, T1), dc
        blk, win, dm, idx = l2_dat[r]
        isl, dc = _place_edges(slot2, blk, win, dm, idx, T2, NB, NWIN2)
        idx16_2[r], dstc2[r] = _pack_idx16(isl, sched2, T2), dc

    # feature table, raw node order, padded
    featsg = np.zeros((NPAD, IN_DIM), dtype=np.float16)
    featsg[:N] = np.asarray(features, np.float32).astype(np.float16)

    # pooling weights [P(node-in-block), NB, P(graph-in-group)] = 1/count
    gcount = np.diff(gstart)
    inv_cnt = (1.0 / np.maximum(gcount, 1)).astype(np.float32)
    poolw = np.zeros((NCORES, P, NB, P), dtype=np.float16)
    for r in range(NCORES):
        for gi, (g0, g1) in enumerate(core_groups[r]):
            ns, ne = gstart[g0], gstart[g1]
            if ne <= ns:
                continue
            nodes = np.arange(ns, ne)
            pl = plocal[nodes]
            poolw[r, pl % P, pl // P, n2g[nodes] - g0] = inv_cnt[n2g[nodes]]

    # descriptors, feature-major, padded [P, 2, NG*P]
    GPAD = NG * P
    desc_pad = np.zeros((B, 256), dtype=np.float32)
    desc_pad[:, :N_DESC] = np.asarray(descriptors, dtype=np.float32)
    desct = np.zeros((NCORES, P, 2, GPAD), dtype=np.float16)
    for r in range(NCORES):
        for gi, (g0, g1) in enumerate(core_groups[r]):
            ncols = g1 - g0
            if ncols <= 0:
                continue
            blockd = desc_pad[g0:g1].T.reshape(2, P, ncols).transpose(1, 0, 2)
            desct[r, :, :, gi * P : gi * P + ncols] = blockd.astype(np.float16)

    pr.N, pr.B = N, B
    pr.NG, pr.NB, pr.GPAD = NG, NB, GPAD
    pr.SBLK, pr.SROWS, pr.NWIN2 = SBLK, SROWS, NWIN2
    pr.BMAX = BMAX
    pr.T1, pr.Cg1, pr.sched1 = T1, Cg1, sched1
    pr.T2, pr.Cg2, pr.sched2 = T2, Cg2, sched2
    pr.block_group = block_group
    pr.core_groups = core_groups
    pr.idx16_1, pr.dstc1 = idx16_1, dstc1
    pr.idx16_2, pr.dstc2 = idx16_2, dstc2
    pr.featsg, pr.poolw, pr.desct = featsg, poolw, desct
    return pr


# --------------------------------------------------------------------------
# Bass program builder (single SPMD program; per-core data via in_maps)
# --------------------------------------------------------------------------
def _build(pr):
    nc = bacc.Bacc("TRN2", target_bir_lowering=False, num_devices=NCORES)

    NB, NG, GPAD = pr.NB, pr.NG, pr.GPAD
    SBLK, SROWS, NWIN2 = pr.SBLK, pr.SROWS, pr.NWIN2
    T1, Cg1, sched1 = pr.T1, pr.Cg1, pr.sched1
    T2, Cg2, sched2 = pr.T2, pr.Cg2, pr.sched2
    block_group = pr.block_group

    BMAX = pr.BMAX

    featsg_d = nc.dram_tensor("featsg", [NPAD, IN_DIM], F16, kind="ExternalInput")
    idx1_d = nc.dram_tensor("idx16_1", [P, T1 * 8], I16, kind="ExternalInput")
    dstc1_d = nc.dram_tensor("dstc1", [P, T1, 1], F16, kind="ExternalInput")
    idx2_d = nc.dram_tensor("idx16_2", [P, T2 * 8], I16, kind="ExternalInput")
    dstc2_d = nc.dram_tensor("dstc2", [P, T2, 1], F16, kind="ExternalInput")
    iotaw_d = nc.dram_tensor("iotaw", [P, BMAX, P], F16, kind="ExternalInput")
    ones1_d = nc.dram_tensor("ones1", [1, P], F16, kind="ExternalInput")
    ident_d = nc.dram_tensor("ident", [P, P], F16, kind="ExternalInput")
    poolw_d = nc.dram_tensor("poolw", [P, NB, P], F16, kind="ExternalInput")
    desct_d = nc.dram_tensor("desct", [P, 2, GPAD], F16, kind="ExternalInput")
    w1_d = nc.dram_tensor("w1", [P, HID], F16, kind="ExternalInput")
    w2t_d = nc.dram_tensor("w2t", [P, 4, HID], F16, kind="ExternalInput")
    b1_d = nc.dram_tensor("b1v", [1, HID], F16, kind="ExternalInput")
    b2_d = nc.dram_tensor("b2v", [1, HID], F16, kind="ExternalInput")
    lw1t_d = nc.dram_tensor("lw1t", [P, 6, U1], F16, kind="ExternalInput")
    lb1t_d = nc.dram_tensor("lb1t", [P, 4], F32, kind="ExternalInput")
    lw2t_d = nc.dram_tensor("lw2t", [P, 4, U2], F16, kind="ExternalInput")
    lb2t_d = nc.dram_tensor("lb2t", [P, 1], F32, kind="ExternalInput")
    cwt_d = nc.dram_tensor("cwt", [P, N_CLASSES], F16, kind="ExternalInput")
    cbt_d = nc.dram_tensor("cbt", [N_CLASSES, 1], F32, kind="ExternalInput")
    out_d = nc.dram_tensor("out", [N_CLASSES, GPAD], F32, kind="ExternalOutput")

    is_eq = mybir.AluOpType.is_equal
    add = mybir.AluOpType.add
    Copy = mybir.ActivationFunctionType.Copy
    Relu = mybir.ActivationFunctionType.Relu

    with tile.TileContext(nc) as tc:
        with (
            tc.tile_pool(name="const", bufs=1) as cp,
            tc.tile_pool(name="gath1", bufs=2) as gp1,
            tc.tile_pool(name="gath2", bufs=2) as gp2,
            tc.tile_pool(name="ind", bufs=2) as ip,
            tc.tile_pool(name="work", bufs=3) as wp,
            tc.tile_pool(name="psA", bufs=4, space="PSUM") as psA,
            tc.tile_pool(name="psB", bufs=3, space="PSUM") as psB,
            tc.tile_pool(name="psP", bufs=1, space="PSUM") as psP,
            tc.tile_pool(name="dram", bufs=1, space="DRAM") as dp,
        ):
            h1_d = dp.tile([SROWS, HID], F16)

            idx1_sb = cp.tile([P, T1 * 8], I16)
            nc.sync.dma_start(idx1_sb[:], idx1_d[:])
            dstc1_sb = cp.tile([P, T1, 1], F16)
            nc.sync.dma_start(dstc1_sb[:], dstc1_d[:])
            idx2_sb = cp.tile([P, T2 * 8], I16)
            nc.sync.dma_start(idx2_sb[:], idx2_d[:])
            dstc2_sb = cp.tile([P, T2, 1], F16)
            nc.sync.dma_start(dstc2_sb[:], dstc2_d[:])
            iotaw_sb = cp.tile([P, BMAX, P], F16)
            nc.sync.dma_start(iotaw_sb[:], iotaw_d[:])
            ones1_sb = cp.tile([1, P], F16)
            nc.sync.dma_start(ones1_sb[:], ones1_d[:])
            ident_sb = cp.tile([P, P], F16)
            nc.sync.dma_start(ident_sb[:], ident_d[:])
            poolw_sb = cp.tile([P, NB, P], F16)
            nc.sync.dma_start(poolw_sb[:], poolw_d[:])
            desct_sb = cp.tile([P, 2, GPAD], F16)
            nc.sync.dma_start(desct_sb[:], desct_d[:])
            w1_sb = cp.tile([P, HID], F16)
            nc.sync.dma_start(w1_sb[:], w1_d[:])
            w2t_sb = cp.tile([P, 4, HID], F16)
            nc.sync.dma_start(w2t_sb[:], w2t_d[:])
            b1_sb = cp.tile([1, HID], F16)
            nc.sync.dma_start(b1_sb[:], b1_d[:])
            b2_sb = cp.tile([1, HID], F16)
            nc.sync.dma_start(b2_sb[:], b2_d[:])
            lw1t_sb = cp.tile([P, 6, U1], F16)
            nc.sync.dma_start(lw1t_sb[:], lw1t_d[:])
            lb1t_sb = cp.tile([P, 4], F32)
            nc.sync.dma_start(lb1t_sb[:], lb1t_d[:])
            lw2t_sb = cp.tile([P, 4, U2], F16)
            nc.sync.dma_start(lw2t_sb[:], lw2t_d[:])
            lb2t_sb = cp.tile([P, 1], F32)
            nc.sync.dma_start(lb2t_sb[:], lb2t_d[:])
            cwt_sb = cp.tile([P, N_CLASSES], F16)
            nc.sync.dma_start(cwt_sb[:], cwt_d[:])
            cbt_sb = cp.tile([N_CLASSES, 1], F32)
            nc.sync.dma_start(cbt_sb[:], cbt_d[:])

            def gather_group(gs, gpool, table, nrows, wsz, idx_sb, elem, name):
                gt = gpool.tile([P, Cg1 if gpool is gp1 else Cg2, elem], F16,
                                tag=f"g{elem}", name=f"{name}_{gs['base']}")
                for w, c0, n in gs["runs"]:
                    lo, hi = w * wsz, min((w + 1) * wsz, nrows)
                    nc.gpsimd.dma_gather(
                        out_ap=gt[:, c0 - gs["base"] : c0 - gs["base"] + n, :],
                        in_ap=table[lo:hi, :],
                        idxs_ap=idx_sb[:, c0 * 8 : (c0 + n) * 8],
                        num_idxs=n * P,
                        num_idxs_reg=n * P,
                        elem_size=elem,
                        single_packet=False,
                    )
                return gt

            CGMAX = max(Cg1, Cg2)

            def indicators(gs, dstc_sb, name):
                """One is_equal per gather run: ind[p, s, j] = (j == dstc[p, s])."""
                ind = ip.tile([P, CGMAX, P], F16, tag="ind", name=name)
                for w, c0, n in gs["runs"]:
                    o = c0 - gs["base"]
                    nc.vector.tensor_tensor(
                        out=ind[:, o : o + n, :],
                        in0=iotaw_sb[:, :n, :],
                        in1=dstc_sb[:, c0 : c0 + n, :].to_broadcast((P, n, P)),
                        op=is_eq,
                    )
                return ind

            # ================= Layer 1 (needed sources) =================
            for gs in sched1:
                g1 = gather_group(gs, gp1, featsg_d, NPAD, WSZ1, idx1_sb,
                                  IN_DIM, "g1")
                ind1 = indicators(gs, dstc1_sb, f"i1_{gs['base']}")
                for b in range(gs["b0"], gs["b1"]):
                    slots = gs["blocks"][b]
                    aggT = psA.tile([P, P], F32, tag="psA", name=f"agg1_{b}")
                    for i, s in enumerate(slots):
                        nc.tensor.matmul(
                            out=aggT[:],
                            lhsT=g1[:, s - gs["base"], :],
                            rhs=ind1[:, s - gs["base"], :],
                            start=(i == 0),
                            stop=(i == len(slots) - 1),
                        )
                    aggT_sb = wp.tile([P, IN_DIM], F16, tag="agg1sb",
                                      name=f"agg1sb{b}")
                    nc.scalar.activation(aggT_sb[:], aggT[:], Copy)
                    h1ps = psB.tile([P, HID], F32, tag="psB", name=f"h1ps{b}")
                    nc.tensor.matmul(out=h1ps[:], lhsT=aggT_sb[:], rhs=w1_sb[:],
                                     start=True, stop=False)
                    nc.tensor.matmul(out=h1ps[:], lhsT=ones1_sb[:], rhs=b1_sb[:],
                                     start=False, stop=True)
                    h1 = wp.tile([P, HID], F16, tag="h1", name=f"h1_{b}")
                    nc.scalar.activation(h1[:], h1ps[:], Relu)
                    nc.sync.dma_start(h1_d[b * P : (b + 1) * P, :], h1[:])

            # ================= Layer 2 + pooling + head =================
            pool_ps = None
            for gs in sched2:
                g2 = gather_group(gs, gp2, h1_d, SROWS, WSZ2, idx2_sb, HID, "g2")
                ind2 = indicators(gs, dstc2_sb, f"i2_{gs['base']}")
                for b in range(gs["b0"], gs["b1"]):
                    slots = gs["blocks"][b]
                    gi = int(block_group[b])
                    first_in_grp = b == 0 or block_group[b - 1] != gi
                    last_in_grp = b == NB - 1 or block_group[b + 1] != gi

                    aggs = [
                        psA.tile([P, P], F32, tag="psA", name=f"agg2_{b}_{fc}")
                        for fc in range(4)
                    ]
                    for i, s in enumerate(slots):
                        for fc in range(4):
                            nc.tensor.matmul(
                                out=aggs[fc][:],
                                lhsT=g2[:, s - gs["base"], fc * P : (fc + 1) * P],
                                rhs=ind2[:, s - gs["base"], :],
                                start=(i == 0),
                                stop=(i == len(slots) - 1),
                            )
                    aggT_sb = wp.tile([P, 4, P], F16, tag="agg2sb",
                                      name=f"agg2sb{b}")
                    for fc in range(4):
                        if fc % 2 == 0:
                            nc.scalar.activation(aggT_sb[:, fc, :], aggs[fc][:],
                                                 Copy)
                        else:
                            nc.vector.tensor_copy(out=aggT_sb[:, fc, :],
                                                  in_=aggs[fc][:])
                    h2ps = psB.tile([P, HID], F32, tag="psB", name=f"h2ps{b}")
                    for fc in range(4):
                        nc.tensor.matmul(
                            out=h2ps[:],
                            lhsT=aggT_sb[:, fc, :],
                            rhs=w2t_sb[:, fc, :],
                            start=(fc == 0),
                            stop=False,
                        )
                    nc.tensor.matmul(out=h2ps[:], lhsT=ones1_sb[:], rhs=b2_sb[:],
                                     start=False, stop=True)
                    h2 = wp.tile([P, HID], F16, tag="h2", name=f"h2_{b}")
                    nc.scalar.activation(h2[:], h2ps[:], Relu)

                    if first_in_grp:
                        pool_ps = psP.tile([P, HID], F32, tag="psP",
                                           name=f"pool{gi}")
                    nc.tensor.matmul(
                        out=pool_ps[:],
                        lhsT=poolw_sb[:, b, :],
                        rhs=h2[:],
                        start=first_in_grp,
                        stop=last_in_grp,
                    )

                    if last_in_grp:
                        hg = wp.tile([P, HID], F16, tag="hg", name=f"hg{gi}")
                        nc.scalar.activation(hg[:], pool_ps[:], Copy)
                        hgT = wp.tile([P, 4, P], F16, tag="hgT", name=f"hgT{gi}")
                        for fc in range(4):
                            tps = psB.tile([P, P], F16, tag="psB",
                                           name=f"tps{gi}_{fc}")
                            nc.tensor.transpose(
                                out=tps[:],
                                in_=hg[:, fc * P : (fc + 1) * P],
                                identity=ident_sb[:],
                            )
                            nc.scalar.activation(hgT[:, fc, :], tps[:], Copy)
                        x1 = wp.tile([P, 4, P], F16, tag="x1", name=f"x1_{gi}")
                        for uc in range(4):
                            x1ps = psB.tile([P, P], F32, tag="psB",
                                            name=f"x1ps{gi}_{uc}")
                            for kc in range(6):
                                rhs = (
                                    hgT[:, kc, :]
                                    if kc < 4
                                    else desct_sb[:, kc - 4, gi * P : (gi + 1) * P]
                                )
                                nc.tensor.matmul(
                                    out=x1ps[:],
                                    lhsT=lw1t_sb[:, kc, uc * P : (uc + 1) * P],
                                    rhs=rhs,
                                    start=(kc == 0),
                                    stop=(kc == 5),
                                )
                            nc.scalar.activation(
                                x1[:, uc, :], x1ps[:], Relu,
                                bias=lb1t_sb[:, uc : uc + 1],
                            )
                        x2ps = psB.tile([P, P], F32, tag="psB", name=f"x2ps{gi}")
                        for kc in range(4):
                            nc.tensor.matmul(
                                out=x2ps[:],
                                lhsT=lw2t_sb[:, kc, :],
                                rhs=x1[:, kc, :],
                                start=(kc == 0),
                                stop=(kc == 3),
                            )
                        x2 = wp.tile([P, P], F16, tag="x2", name=f"x2_{gi}")
                        nc.scalar.activation(x2[:], x2ps[:], Relu,
                                             bias=lb2t_sb[:, :1])
                        lgps = psB.tile([P, P], F32, tag="psB", name=f"lg{gi}")
                        nc.tensor.matmul(
                            out=lgps[:N_CLASSES, :],
                            lhsT=cwt_sb[:],
                            rhs=x2[:],
                            start=True,
                            stop=True,
                        )
                        lg = wp.tile([N_CLASSES, P], F32, tag="lg",
                                     name=f"lgsb{gi}")
                        nc.vector.tensor_tensor(
                            out=lg[:],
                            in0=lgps[:N_CLASSES, :],
                            in1=cbt_sb[:, :1].to_broadcast((N_CLASSES, P)),
                            op=add,
                        )
                        nc.sync.dma_start(out_d[:, gi * P : (gi + 1) * P], lg[:])

    nc.compile()
    return nc


# --------------------------------------------------------------------------
# Entry point
# --------------------------------------------------------------------------
def prepare(features, descriptors, src, dst, node2graph,
            W1, b1, W2, b2, lw1, lb1, lw2, lb2, cw, cb):
    """Preprocess + build; returns (pr, nc, in_maps)."""
    pr = _preprocess(features, descriptors, src, dst, node2graph)
    nc = _build(pr)

    f16 = np.float16
    iotaw = np.broadcast_to(np.arange(P, dtype=f16), (P, pr.BMAX, P)).copy()

    w1 = np.asarray(W1, np.float32).astype(f16)
    w2t = np.asarray(W2, np.float32).reshape(4, P, HID).transpose(1, 0, 2).astype(f16)
    w2t = np.ascontiguousarray(w2t)
    b1v = np.asarray(b1, np.float32).reshape(1, HID).astype(f16)
    b2v = np.asarray(b2, np.float32).reshape(1, HID).astype(f16)

    KD = 768
    lw1_pad = np.zeros((KD, U1), np.float32)
    lw1_pad[: HID + N_DESC, :500] = np.asarray(lw1, np.float32)
    lw1t = np.ascontiguousarray(
        lw1_pad.reshape(6, P, U1).transpose(1, 0, 2)).astype(f16)
    lb1_pad = np.zeros((U1,), np.float32)
    lb1_pad[:500] = np.asarray(lb1, np.float32)
    lb1t = np.ascontiguousarray(lb1_pad.reshape(4, P).T)
    lw2_pad = np.zeros((U1, U2), np.float32)
    lw2_pad[:500, :100] = np.asarray(lw2, np.float32)
    lw2t = np.ascontiguousarray(
        lw2_pad.reshape(4, P, U2).transpose(1, 0, 2)).astype(f16)
    lb2_pad = np.zeros((U2, 1), np.float32)
    lb2_pad[:100, 0] = np.asarray(lb2, np.float32)
    cw_pad = np.zeros((P, N_CLASSES), np.float32)
    cw_pad[:100] = np.asarray(cw, np.float32)
    cbt = np.asarray(cb, np.float32).reshape(N_CLASSES, 1)

    in_maps = []
    for r in range(NCORES):
        in_maps.append({
            "featsg": pr.featsg,
            "idx16_1": pr.idx16_1[r],
            "dstc1": pr.dstc1[r][:, :, None],
            "idx16_2": pr.idx16_2[r],
            "dstc2": pr.dstc2[r][:, :, None],
            "iotaw": iotaw,
            "ones1": np.ones((1, P), dtype=f16),
            "ident": np.eye(P, dtype=f16),
            "poolw": pr.poolw[r],
            "desct": pr.desct[r],
            "w1": w1,
            "w2t": w2t,
            "b1v": b1v,
            "b2v": b2v,
            "lw1t": lw1t,
            "lb1t": lb1t,
            "lw2t": lw2t,
            "lb2t": lb2_pad,
            "cwt": cw_pad.astype(f16),
            "cbt": cbt,
        })

    return pr, nc, in_maps


def kernel(features, descriptors, src, dst, node2graph,
           W1, b1, W2, b2, lw1, lb1, lw2, lb2, cw, cb, _run_opts=None):
    opts0 = dict(_run_opts or {})
    opts0.pop("_last_result", None)
    pr, nc, in_maps = prepare(features, descriptors, src, dst, node2graph,
                              W1, b1, W2, b2, lw1, lb1, lw2, lb2, cw, cb)
    res = run_bass_kernel_spmd(nc, in_maps, core_ids=list(range(NCORES)), **opts0)

    out = np.zeros((pr.B, N_CLASSES), dtype=np.float32)
    for r in range(NCORES):
        o = np.asarray(res.results[r]["out"])
        for gi, (g0, g1) in enumerate(pr.core_groups[r]):
            ncols = g1 - g0
            if ncols > 0:
                out[g0:g1] = o[:, gi * P : gi * P + ncols].T
    if _run_opts is not None:
        _run_opts["_last_result"] = res
    return out



# revision 29
# speedup vs baseline: 1.5689x; 1.5689x over previous
"""Distributed 2-layer GCN + graph pooling + MLP head on 8 TRN2 NeuronCores.

Collective-free data-parallel strategy (per the sharding hint):
  - Graphs (and their nodes, contiguously -- node2graph is sorted) are
    partitioned into 8 shards with ~equal node counts. Weights replicated.
  - Each core owns the edges whose dst node it owns. Layer 2 needs
    h1[src] for those edges; instead of an AllGather, each core computes
    layer 1 *locally* for exactly the source nodes its edges reference
    (~40% of all nodes). No collectives -> no cross-core barrier.
  - Layer 1 aggregates raw *features* (segment_sum commutes with the
    linear map); its edge gather moves 128-dim rows from a per-core
    DEDUPED feature table (unique sources only -> 3 int16 windows).
  - h1 (fp16) for the needed sources is written to a core-local DRAM
    table; layer 2 gathers 512-dim rows from it (2 windows).
  - Edge gathers use GPSIMD dma_gather, whose descriptor-generation
    time (the kernel's serial bottleneck) is proportional to the index
    count. Each (group, window) run is packed contiguously per core
    (block boundaries fall mid-slot, so no per-(block,window) chunk
    quantization) and only the run tail is padded (with table row 0,
    dst column -1): total gathered slots track the real edge count to
    within ~5%, while the slot layout stays uniform across cores.
  - Segment-sum on chip: per aggregation block, a 0/1 indicator built by
    DVE (is_equal of a [0,128) iota row vs per-edge dst values) over the
    block's slot range turns edge chunks into PE matmuls:
    aggT[f, n] += G_slot[:, f].T @ I_col[:, n].  Slots shared between
    blocks are matmul'd into both blocks' PSUMs; the per-block dst
    columns carry -1 for foreign edges, zeroing their indicator.
  - Graph mean-pooling is another indicator matmul with 1/count weights
    (host-precomputed fp16), fused after layer 2 per node block.
  - The MLP head runs feature-major so biases are per-partition.

Device compute fp16 (PSUM fp32); biases fp32; output fp32.
"""

import sys

sys.path.insert(0, "/opt/trn_rl_repo")

import numpy as np

import concourse.bass as bass
import concourse.mybir as mybir
import concourse.tile as tile
from concourse import bacc
from concourse.bass_utils import run_bass_kernel_spmd

P = 128
NCORES = 8
IN_DIM = 128
HID = 512
N_DESC = 200
N_CLASSES = 2
U1 = 512  # padded head hidden 1 (500 -> 512)
U2 = 128  # padded head hidden 2 (100 -> 128)
WSZ = 32768  # int16 gather window (table rows per window)
SB1 = 64  # layer-1 gather-group slot budget (chunks of 128 edges)
SB2 = 20  # layer-2 gather-group slot budget

F16 = mybir.dt.float16
F32 = mybir.dt.float32
I16 = mybir.dt.int16


class Prep:
    pass


class Sched:
    pass


def _mk_sched(edata, nblocks, nwin, budget):
    """Contiguous-packing gather schedule, uniform across cores.

    edata[r] = (blk, win, dloc, gidx) int64 arrays per core: aggregation
    block, gather window, dst row-in-block [0,128), window-local gather
    row. Groups are consecutive block ranges sized so each group's total
    slot count stays <= budget. Returns a Sched with the group structure
    plus per-core packed int16 index streams (-1 tail padding) and
    per-block dst columns.
    """
    R = len(edata)

    cnt = np.zeros((R, nblocks, nwin), dtype=np.int64)
    for r, (blk, win, dloc, gidx) in enumerate(edata):
        np.add.at(cnt[r], (blk, win), 1)

    def group_slots(b0, b1):
        c = cnt[:, b0:b1, :].sum(axis=1)  # [R, nwin]
        return int(((c.max(axis=0) + P - 1) // P).sum())

    # greedy slot-budget grouping over consecutive blocks
    bounds = []
    b = 0
    while b < nblocks:
        b2 = b + 1
        while b2 < nblocks and group_slots(b, b2 + 1) <= budget:
            b2 += 1
        bounds.append((b, b2))
        b = b2
    ngroups = len(bounds)
    blk2grp = np.zeros(nblocks, dtype=np.int64)
    for g, (b0, b1) in enumerate(bounds):
        blk2grp[b0:b1] = g

    # pass 1: runs, per-(block,window) union slot ranges, indicator cols
    groups = []
    slot = 0
    col = 0
    rid = 0
    run_cnts = []  # per run: [R] real edge counts
    run_c0 = np.full((ngroups, nwin), -1, dtype=np.int64)
    s0_bw = np.full((nblocks, nwin), -1, dtype=np.int64)
    colbase_bw = np.full((nblocks, nwin), -1, dtype=np.int64)
    for g, (b0, b1) in enumerate(bounds):
        base = slot
        runs = []
        ranges = {b: [] for b in range(b0, b1)}  # (w, s0, s1) abs slots
        for w in range(nwin):
            c_r = cnt[:, b0:b1, w]  # [R, nb]
            tot = c_r.sum(axis=1)
            n = int((tot.max() + P - 1) // P)
            if n == 0:
                continue
            c0 = slot
            run_c0[g, w] = c0
            runs.append((w, c0, n, rid))
            run_cnts.append(tot.copy())
            rid += 1
            pfx = np.concatenate(
                [np.zeros((R, 1), dtype=np.int64), np.cumsum(c_r, axis=1)], axis=1
            )
            for bi in range(b1 - b0):
                m = c_r[:, bi] > 0
                if not m.any():
                    continue
                s0 = int((pfx[m, bi] // P).min()) + c0
                s1 = int(((pfx[m, bi + 1] - 1) // P).max()) + 1 + c0
                ranges[b0 + bi].append((w, s0, s1))
            slot += n
        if not runs:
            runs.append((0, slot, 1, rid))  # dummy run so the group tile exists
            run_cnts.append(np.zeros(R, dtype=np.int64))
            rid += 1
            run_c0[g, 0] = slot
            slot += 1
        # indicator columns per block (contiguous across its windows)
        bcols = {}
        bslots = {}
        for b in range(b0, b1):
            bcols[b] = col
            slots = []
            for w, s0, s1 in ranges[b]:
                s0_bw[b, w] = s0
                colbase_bw[b, w] = col + len(slots)
                slots.extend(range(s0 - base, s1 - base))
            if not slots:
                slots = [runs[0][1] - base]  # dummy col; dstc stays -1
            bslots[b] = slots
            col += len(slots)
        groups.append(
            dict(b0=b0, b1=b1, base=base, runs=runs, bcols=bcols, bslots=bslots,
                 cg=slot - base)
        )

    T, D = slot, col
    NRUNS = rid
    CG = max(gs["cg"] for gs in groups)
    RMAX = max(len(s) for gs in groups for s in gs["bslots"].values())

    # pass 2: per-core packed index streams and dst columns. Padding lanes
    # gather table row 0 (always valid); their dst columns stay -1 so the
    # indicator zeroes them. Every lane of every slot is written -> no
    # stale/NaN lanes, and the schedule stays a plain full-slot gather.
    idx_slot = np.zeros((R, T, P), dtype=np.int16)
    dstcp = np.full((R, P, D), -1.0, dtype=np.float16)
    nreal = np.zeros(R, dtype=np.int64)
    for r, (blk, win, dloc, gidx) in enumerate(edata):
        nreal[r] = len(blk)
        if len(blk) == 0:
            continue
        grp = blk2grp[blk]
        order = np.lexsort((blk, win, grp))
        blk_o, win_o = blk[order], win[order]
        dloc_o, gidx_o = dloc[order], gidx[order]
        grp_o = grp[order]
        key = grp_o * nwin + win_o
        starts = np.concatenate(
            [[0], np.cumsum(np.bincount(key, minlength=ngroups * nwin))]
        )
        pos = np.arange(len(key)) - starts[key]
        sabs = run_c0[grp_o, win_o] + pos // P
        lane = pos % P
        idx_slot[r][sabs, lane] = gidx_o.astype(np.int16)
        colv = colbase_bw[blk_o, win_o] + (sabs - s0_bw[blk_o, win_o])
        dstcp[r][lane, colv] = dloc_o.astype(np.float32)

    sc = Sched()
    sc.groups, sc.T, sc.D, sc.CG, sc.RMAX = groups, T, D, CG, RMAX
    sc.NRUNS = NRUNS
    sc.idx_slot, sc.dstcp, sc.nreal = idx_slot, dstcp, nreal
    return sc


def _pack_idx16(idx_slot):
    """[T, P] int16 slot-major stream -> dma_gather layout [128, T*8]."""
    wrapped = idx_slot.reshape(-1).reshape(-1, 16).T  # [16, T*8]
    return np.tile(wrapped, (8, 1))


# --------------------------------------------------------------------------
# Host-side preprocessing: partition, dedup tables, schedule
# --------------------------------------------------------------------------
def _preprocess(features, descriptors, src, dst, node2graph):
    pr = Prep()
    N = features.shape[0]
    B = descriptors.shape[0]

    n2g = np.asarray(node2graph).astype(np.int64)
    src = np.asarray(src).astype(np.int64)
    dst = np.asarray(dst).astype(np.int64)

    gstart = np.searchsorted(n2g, np.arange(B + 1))  # node range per graph

    # partition graphs into NCORES shards with ~equal node counts
    cuts = np.searchsorted(gstart, (np.arange(1, NCORES) * N) // NCORES)
    gcuts = np.concatenate([[0], cuts, [B]])

    # per-core pool groups of <=128 graphs; group nodes padded to 128-blocks
    core_groups = []
    for r in range(NCORES):
        g0, g1 = gcuts[r], gcuts[r + 1]
        groups = []
        g = g0
        while g < g1:
            ge = min(g + P, g1)
            groups.append((g, ge))
            g = ge
        if not groups:
            groups = [(g0, g0)]
        core_groups.append(groups)
    NG = max(len(gr) for gr in core_groups)

    blocks_per_group_core = np.zeros((NCORES, NG), dtype=np.int64)
    for r in range(NCORES):
        for gi, (g0, g1) in enumerate(core_groups[r]):
            nn = gstart[g1] - gstart[g0]
            blocks_per_group_core[r, gi] = max((nn + P - 1) // P, 1)
    bpg = blocks_per_group_core.max(axis=0)
    NB = int(bpg.sum())
    block_group = np.repeat(np.arange(NG), bpg)

    # padded-local index + owner of each node (layer-2 / pooling space)
    plocal = np.zeros(N, dtype=np.int64)
    owner = np.zeros(N, dtype=np.int64)
    group_base = np.concatenate([[0], np.cumsum(bpg) * P])
    for r in range(NCORES):
        for gi, (g0, g1) in enumerate(core_groups[r]):
            ns, ne = gstart[g0], gstart[g1]
            if ne > ns:
                plocal[ns:ne] = group_base[gi] + np.arange(ne - ns)
                owner[ns:ne] = r

    # per-core needed-source sets (sorted unique srcs of locally-owned edges)
    e_owner = owner[dst]
    uniq_r, l2_edges = [], []
    for r in range(NCORES):
        es = np.nonzero(e_owner == r)[0]
        uq = np.unique(src[es])
        uniq_r.append(uq)
        l2_edges.append(es)
    SBLK = max((len(u) + P - 1) // P for u in uniq_r)
    SROWS = SBLK * P
    NWIN2 = (SROWS + WSZ - 1) // WSZ
    assert SROWS <= 2 * WSZ, f"h1 table too large for int16 windows: {SROWS}"

    # layer-1 edges per core: all graph edges whose dst is a needed source.
    # Needed sources are ranked by in-degree (descending) so per-block edge
    # counts are similar across cores. The gather table is the per-core
    # deduped set of source features (unique srcs of layer-1 edges).
    l1_dat = []
    rank_maps = []
    usrcs = []
    for r in range(NCORES):
        uq = uniq_r[r]
        pos = np.searchsorted(uq, dst)
        pos_cl = np.minimum(pos, len(uq) - 1)
        m = uq[pos_cl] == dst  # edge's dst is in the needed set
        e1 = np.nonzero(m)[0]
        orank = pos[e1]
        indeg = np.bincount(orank, minlength=len(uq))
        order = np.argsort(-indeg, kind="stable")
        newrank = np.empty(len(uq), dtype=np.int64)
        newrank[order] = np.arange(len(uq))
        rank_maps.append(newrank)
        rank = newrank[orank]  # aggregation target (local row in h1 table)
        usrc = np.unique(src[e1])
        usrcs.append(usrc)
        gidx = np.searchsorted(usrc, src[e1])
        l1_dat.append((rank // P, gidx // WSZ, rank % P, gidx % WSZ))
    U = max(len(u) for u in usrcs)
    UPAD = ((U + P - 1) // P) * P
    NWIN1 = (UPAD + WSZ - 1) // WSZ

    sc1 = _mk_sched(l1_dat, SBLK, NWIN1, SB1)

    # layer-2 edges per core: local edges; src -> rank in needed set
    l2_dat = []
    for r in range(NCORES):
        es = l2_edges[r]
        dpl = plocal[dst[es]]
        rank = rank_maps[r][np.searchsorted(uniq_r[r], src[es])]
        l2_dat.append((dpl // P, rank // WSZ, dpl % P, rank % WSZ))

    sc2 = _mk_sched(l2_dat, NB, NWIN2, SB2)

    RMAX = max(sc1.RMAX, sc2.RMAX)

    idx16_1 = np.stack([_pack_idx16(sc1.idx_slot[r]) for r in range(NCORES)])
    idx16_2 = np.stack([_pack_idx16(sc2.idx_slot[r]) for r in range(NCORES)])

    # per-core deduped feature tables (fp16, padded)
    featsg = np.zeros((NCORES, UPAD, IN_DIM), dtype=np.float16)
    f16feat = np.asarray(features, np.float32).astype(np.float16)
    for r in range(NCORES):
        featsg[r, : len(usrcs[r])] = f16feat[usrcs[r]]

    # pooling weights [P(node-in-block), NB, P(graph-in-group)] = 1/count
    gcount = np.diff(gstart)
    inv_cnt = (1.0 / np.maximum(gcount, 1)).astype(np.float32)
    poolw = np.zeros((NCORES, P, NB, P), dtype=np.float16)
    for r in range(NCORES):
        for gi, (g0, g1) in enumerate(core_groups[r]):
            ns, ne = gstart[g0], gstart[g1]
            if ne <= ns:
                continue
            nodes = np.arange(ns, ne)
            pl = plocal[nodes]
            poolw[r, pl % P, pl // P, n2g[nodes] - g0] = inv_cnt[n2g[nodes]]

    # descriptors, feature-major, padded [P, 2, NG*P]
    GPAD = NG * P
    desc_pad = np.zeros((B, 256), dtype=np.float32)
    desc_pad[:, :N_DESC] = np.asarray(descriptors, dtype=np.float32)
    desct = np.zeros((NCORES, P, 2, GPAD), dtype=np.float16)
    for r in range(NCORES):
        for gi, (g0, g1) in enumerate(core_groups[r]):
            ncols = g1 - g0
            if ncols <= 0:
                continue
            blockd = desc_pad[g0:g1].T.reshape(2, P, ncols).transpose(1, 0, 2)
            desct[r, :, :, gi * P : gi * P + ncols] = blockd.astype(np.float16)

    pr.N, pr.B = N, B
    pr.NG, pr.NB, pr.GPAD = NG, NB, GPAD
    pr.SBLK, pr.SROWS, pr.NWIN2 = SBLK, SROWS, NWIN2
    pr.UPAD, pr.NWIN1 = UPAD, NWIN1
    pr.RMAX = RMAX
    pr.sc1, pr.sc2 = sc1, sc2
    pr.block_group = block_group
    pr.core_groups = core_groups
    pr.idx16_1, pr.idx16_2 = idx16_1, idx16_2
    pr.featsg, pr.poolw, pr.desct = featsg, poolw, desct
    return pr


# --------------------------------------------------------------------------
# Bass program builder (single SPMD program; per-core data via in_maps)
# --------------------------------------------------------------------------
def _build(pr):
    nc = bacc.Bacc("TRN2", target_bir_lowering=False, num_devices=NCORES)

    NB, NG, GPAD = pr.NB, pr.NG, pr.GPAD
    SROWS, NWIN2 = pr.SROWS, pr.NWIN2
    UPAD = pr.UPAD
    sc1, sc2 = pr.sc1, pr.sc2
    RMAX = pr.RMAX
    block_group = pr.block_group

    featsg_d = nc.dram_tensor("featsg", [UPAD, IN_DIM], F16, kind="ExternalInput")
    idx1_d = nc.dram_tensor("idx16_1", [P, sc1.T * 8], I16, kind="ExternalInput")
    dstc1_d = nc.dram_tensor("dstc1", [P, sc1.D, 1], F16, kind="ExternalInput")
    idx2_d = nc.dram_tensor("idx16_2", [P, sc2.T * 8], I16, kind="ExternalInput")
    dstc2_d = nc.dram_tensor("dstc2", [P, sc2.D, 1], F16, kind="ExternalInput")
    iotaw_d = nc.dram_tensor("iotaw", [P, RMAX, P], F16, kind="ExternalInput")
    ones1_d = nc.dram_tensor("ones1", [1, P], F16, kind="ExternalInput")
    ident_d = nc.dram_tensor("ident", [P, P], F16, kind="ExternalInput")
    poolw_d = nc.dram_tensor("poolw", [P, NB, P], F16, kind="ExternalInput")
    desct_d = nc.dram_tensor("desct", [P, 2, GPAD], F16, kind="ExternalInput")
    w1_d = nc.dram_tensor("w1", [P, HID], F16, kind="ExternalInput")
    w2t_d = nc.dram_tensor("w2t", [P, 4, HID], F16, kind="ExternalInput")
    b1_d = nc.dram_tensor("b1v", [1, HID], F16, kind="ExternalInput")
    b2_d = nc.dram_tensor("b2v", [1, HID], F16, kind="ExternalInput")
    lw1t_d = nc.dram_tensor("lw1t", [P, 6, U1], F16, kind="ExternalInput")
    lb1t_d = nc.dram_tensor("lb1t", [P, 4], F32, kind="ExternalInput")
    lw2t_d = nc.dram_tensor("lw2t", [P, 4, U2], F16, kind="ExternalInput")
    lb2t_d = nc.dram_tensor("lb2t", [P, 1], F32, kind="ExternalInput")
    cwt_d = nc.dram_tensor("cwt", [P, N_CLASSES], F16, kind="ExternalInput")
    cbt_d = nc.dram_tensor("cbt", [N_CLASSES, 1], F32, kind="ExternalInput")
    out_d = nc.dram_tensor("out", [N_CLASSES, GPAD], F32, kind="ExternalOutput")

    is_eq = mybir.AluOpType.is_equal
    add = mybir.AluOpType.add
    Copy = mybir.ActivationFunctionType.Copy
    Relu = mybir.ActivationFunctionType.Relu

    with tile.TileContext(nc) as tc:
        with (
            tc.tile_pool(name="const", bufs=1) as cp,
            tc.tile_pool(name="gath1", bufs=2) as gp1,
            tc.tile_pool(name="gath2", bufs=2) as gp2,
            tc.tile_pool(name="ind", bufs=3) as ip,
            tc.tile_pool(name="work", bufs=3) as wp,
            tc.tile_pool(name="psA", bufs=4, space="PSUM") as psA,
            tc.tile_pool(name="psB", bufs=3, space="PSUM") as psB,
            tc.tile_pool(name="psP", bufs=1, space="PSUM") as psP,
            tc.tile_pool(name="dram", bufs=1, space="DRAM") as dp,
        ):
            h1_d = dp.tile([SROWS, HID], F16)

            idx1_sb = cp.tile([P, sc1.T * 8], I16)
            nc.sync.dma_start(idx1_sb[:], idx1_d[:])
            dstc1_sb = cp.tile([P, sc1.D, 1], F16)
            nc.sync.dma_start(dstc1_sb[:], dstc1_d[:])
            idx2_sb = cp.tile([P, sc2.T * 8], I16)
            nc.sync.dma_start(idx2_sb[:], idx2_d[:])
            dstc2_sb = cp.tile([P, sc2.D, 1], F16)
            nc.sync.dma_start(dstc2_sb[:], dstc2_d[:])
            iotaw_sb = cp.tile([P, RMAX, P], F16)
            nc.sync.dma_start(iotaw_sb[:], iotaw_d[:])
            ones1_sb = cp.tile([1, P], F16)
            nc.sync.dma_start(ones1_sb[:], ones1_d[:])
            ident_sb = cp.tile([P, P], F16)
            nc.sync.dma_start(ident_sb[:], ident_d[:])
            poolw_sb = cp.tile([P, NB, P], F16)
            nc.sync.dma_start(poolw_sb[:], poolw_d[:])
            desct_sb = cp.tile([P, 2, GPAD], F16)
            nc.sync.dma_start(desct_sb[:], desct_d[:])
            w1_sb = cp.tile([P, HID], F16)
            nc.sync.dma_start(w1_sb[:], w1_d[:])
            w2t_sb = cp.tile([P, 4, HID], F16)
            nc.sync.dma_start(w2t_sb[:], w2t_d[:])
            b1_sb = cp.tile([1, HID], F16)
            nc.sync.dma_start(b1_sb[:], b1_d[:])
            b2_sb = cp.tile([1, HID], F16)
            nc.sync.dma_start(b2_sb[:], b2_d[:])
            lw1t_sb = cp.tile([P, 6, U1], F16)
            nc.sync.dma_start(lw1t_sb[:], lw1t_d[:])
            lb1t_sb = cp.tile([P, 4], F32)
            nc.sync.dma_start(lb1t_sb[:], lb1t_d[:])
            lw2t_sb = cp.tile([P, 4, U2], F16)
            nc.sync.dma_start(lw2t_sb[:], lw2t_d[:])
            lb2t_sb = cp.tile([P, 1], F32)
            nc.sync.dma_start(lb2t_sb[:], lb2t_d[:])
            cwt_sb = cp.tile([P, N_CLASSES], F16)
            nc.sync.dma_start(cwt_sb[:], cwt_d[:])
            cbt_sb = cp.tile([N_CLASSES, 1], F32)
            nc.sync.dma_start(cbt_sb[:], cbt_d[:])

            def gather_group(gi, gs, gpool, cg, table, nrows, idx_sb, elem,
                             name):
                gt = gpool.tile([P, cg, elem], F16, tag=f"g{elem}",
                                name=f"{name}_{gs['base']}")
                for w, c0, n, rid in gs["runs"]:
                    lo, hi = w * WSZ, min((w + 1) * WSZ, nrows)
                    nc.gpsimd.dma_gather(
                        out_ap=gt[:, c0 - gs["base"] : c0 - gs["base"] + n, :],
                        in_ap=table[lo:hi, :],
                        idxs_ap=idx_sb[:, c0 * 8 : (c0 + n) * 8],
                        num_idxs=n * P,
                        num_idxs_reg=n * P,
                        elem_size=elem,
                        single_packet=False,
                    )
                return gt

            def indicator(gs, b, dstc_sb, name):
                slots = gs["bslots"][b]
                K = len(slots)
                c0 = gs["bcols"][b]
                ind = ip.tile([P, RMAX, P], F16, tag="ind", name=name)
                nc.vector.tensor_tensor(
                    out=ind[:, :K, :],
                    in0=iotaw_sb[:, :K, :],
                    in1=dstc_sb[:, c0 : c0 + K, :].to_broadcast((P, K, P)),
                    op=is_eq,
                )
                return ind, slots

            # ================= Layer 1 (needed sources) =================
            for gi, gs in enumerate(sc1.groups):
                g1 = gather_group(gi, gs, gp1, sc1.CG, featsg_d, UPAD, idx1_sb,
                                  IN_DIM, "g1")
                for b in range(gs["b0"], gs["b1"]):
                    ind1, slots = indicator(gs, b, dstc1_sb, f"i1_{b}")
                    aggT = psA.tile([P, P], F32, tag="psA", name=f"agg1_{b}")
                    for i, s in enumerate(slots):
                        nc.tensor.matmul(
                            out=aggT[:],
                            lhsT=g1[:, s, :],
                            rhs=ind1[:, i, :],
                            start=(i == 0),
                            stop=(i == len(slots) - 1),
                        )
                    aggT_sb = wp.tile([P, IN_DIM], F16, tag="agg1sb",
                                      name=f"agg1sb{b}")
                    nc.scalar.activation(aggT_sb[:], aggT[:], Copy)
                    h1ps = psB.tile([P, HID], F32, tag="psB", name=f"h1ps{b}")
                    nc.tensor.matmul(out=h1ps[:], lhsT=aggT_sb[:], rhs=w1_sb[:],
                                     start=True, stop=False)
                    nc.tensor.matmul(out=h1ps[:], lhsT=ones1_sb[:], rhs=b1_sb[:],
                                     start=False, stop=True)
                    h1 = wp.tile([P, HID], F16, tag="h1", name=f"h1_{b}")
                    nc.scalar.activation(h1[:], h1ps[:], Relu)
                    nc.sync.dma_start(h1_d[b * P : (b + 1) * P, :], h1[:])

            # ================= Layer 2 + pooling + head =================
            pool_ps = None
            for gi, gs in enumerate(sc2.groups):
                g2 = gather_group(gi, gs, gp2, sc2.CG, h1_d, SROWS, idx2_sb,
                                  HID, "g2")
                for b in range(gs["b0"], gs["b1"]):
                    grp = int(block_group[b])
                    first_in_grp = b == 0 or block_group[b - 1] != grp
                    last_in_grp = b == NB - 1 or block_group[b + 1] != grp

                    ind2, slots = indicator(gs, b, dstc2_sb, f"i2_{b}")
                    aggs = [
                        psA.tile([P, P], F32, tag="psA", name=f"agg2_{b}_{fc}")
                        for fc in range(4)
                    ]
                    for i, s in enumerate(slots):
                        for fc in range(4):
                            nc.tensor.matmul(
                                out=aggs[fc][:],
                                lhsT=g2[:, s, fc * P : (fc + 1) * P],
                                rhs=ind2[:, i, :],
                                start=(i == 0),
                                stop=(i == len(slots) - 1),
                            )
                    aggT_sb = wp.tile([P, 4, P], F16, tag="agg2sb",
                                      name=f"agg2sb{b}")
                    for fc in range(4):
                        if fc % 2 == 0:
                            nc.scalar.activation(aggT_sb[:, fc, :], aggs[fc][:],
                                                 Copy)
                        else:
                            nc.vector.tensor_copy(out=aggT_sb[:, fc, :],
                                                  in_=aggs[fc][:])
                    h2ps = psB.tile([P, HID], F32, tag="psB", name=f"h2ps{b}")
                    for fc in range(4):
                        nc.tensor.matmul(
                            out=h2ps[:],
                            lhsT=aggT_sb[:, fc, :],
                            rhs=w2t_sb[:, fc, :],
                            start=(fc == 0),
                            stop=False,
                        )
                    nc.tensor.matmul(out=h2ps[:], lhsT=ones1_sb[:], rhs=b2_sb[:],
                                     start=False, stop=True)
                    h2 = wp.tile([P, HID], F16, tag="h2", name=f"h2_{b}")
                    nc.scalar.activation(h2[:], h2ps[:], Relu)

                    if first_in_grp:
                        pool_ps = psP.tile([P, HID], F32, tag="psP",
                                           name=f"pool{grp}")
                    nc.tensor.matmul(
                        out=pool_ps[:],
                        lhsT=poolw_sb[:, b, :],
                        rhs=h2[:],
                        start=first_in_grp,
                        stop=last_in_grp,
                    )

                    if last_in_grp:
                        hg = wp.tile([P, HID], F16, tag="hg", name=f"hg{grp}")
                        nc.scalar.activation(hg[:], pool_ps[:], Copy)
                        hgT = wp.tile([P, 4, P], F16, tag="hgT", name=f"hgT{grp}")
                        for fc in range(4):
                            tps = psB.tile([P, P], F16, tag="psB",
                                           name=f"tps{grp}_{fc}")
                            nc.tensor.transpose(
                                out=tps[:],
                                in_=hg[:, fc * P : (fc + 1) * P],
                                identity=ident_sb[:],
                            )
                            nc.scalar.activation(hgT[:, fc, :], tps[:], Copy)
                        x1 = wp.tile([P, 4, P], F16, tag="x1", name=f"x1_{grp}")
                        for uc in range(4):
                            x1ps = psB.tile([P, P], F32, tag="psB",
                                            name=f"x1ps{grp}_{uc}")
                            for kc in range(6):
                                rhs = (
                                    hgT[:, kc, :]
                                    if kc < 4
                                    else desct_sb[:, kc - 4, grp * P : (grp + 1) * P]
                                )
                                nc.tensor.matmul(
                                    out=x1ps[:],
                                    lhsT=lw1t_sb[:, kc, uc * P : (uc + 1) * P],
                                    rhs=rhs,
                                    start=(kc == 0),
                                    stop=(kc == 5),
                                )
                            nc.scalar.activation(
                                x1[:, uc, :], x1ps[:], Relu,
                                bias=lb1t_sb[:, uc : uc + 1],
                            )
                        x2ps = psB.tile([P, P], F32, tag="psB", name=f"x2ps{grp}")
                        for kc in range(4):
                            nc.tensor.matmul(
                                out=x2ps[:],
                                lhsT=lw2t_sb[:, kc, :],
                                rhs=x1[:, kc, :],
                                start=(kc == 0),
                                stop=(kc == 3),
                            )
                        x2 = wp.tile([P, P], F16, tag="x2", name=f"x2_{grp}")
                        nc.scalar.activation(x2[:], x2ps[:], Relu,
                                             bias=lb2t_sb[:, :1])
                        lgps = psB.tile([P, P], F32, tag="psB", name=f"lg{grp}")
                        nc.tensor.matmul(
                            out=lgps[:N_CLASSES, :],
                            lhsT=cwt_sb[:],
                            rhs=x2[:],
                            start=True,
                            stop=True,
                        )
                        lg = wp.tile([N_CLASSES, P], F32, tag="lg",
                                     name=f"lgsb{grp}")
                        nc.vector.tensor_tensor(
                            out=lg[:],
                            in0=lgps[:N_CLASSES, :],
                            in1=cbt_sb[:, :1].to_broadcast((N_CLASSES, P)),
                            op=add,
                        )
                        nc.sync.dma_start(out_d[:, grp * P : (grp + 1) * P], lg[:])

    nc.compile()
    return nc


# --------------------------------------------------------------------------
# Entry point
# --------------------------------------------------------------------------
def prepare(features, descriptors, src, dst, node2graph,
            W1, b1, W2, b2, lw1, lb1, lw2, lb2, cw, cb):
    """Preprocess + build; returns (pr, nc, in_maps)."""
    pr = _preprocess(features, descriptors, src, dst, node2graph)
    nc = _build(pr)

    f16 = np.float16
    iotaw = np.broadcast_to(np.arange(P, dtype=f16), (P, pr.RMAX, P)).copy()

    w1 = np.asarray(W1, np.float32).astype(f16)
    w2t = np.asarray(W2, np.float32).reshape(4, P, HID).transpose(1, 0, 2).astype(f16)
    w2t = np.ascontiguousarray(w2t)
    b1v = np.asarray(b1, np.float32).reshape(1, HID).astype(f16)
    b2v = np.asarray(b2, np.float32).reshape(1, HID).astype(f16)

    KD = 768
    lw1_pad = np.zeros((KD, U1), np.float32)
    lw1_pad[: HID + N_DESC, :500] = np.asarray(lw1, np.float32)
    lw1t = np.ascontiguousarray(
        lw1_pad.reshape(6, P, U1).transpose(1, 0, 2)).astype(f16)
    lb1_pad = np.zeros((U1,), np.float32)
    lb1_pad[:500] = np.asarray(lb1, np.float32)
    lb1t = np.ascontiguousarray(lb1_pad.reshape(4, P).T)
    lw2_pad = np.zeros((U1, U2), np.float32)
    lw2_pad[:500, :100] = np.asarray(lw2, np.float32)
    lw2t = np.ascontiguousarray(
        lw2_pad.reshape(4, P, U2).transpose(1, 0, 2)).astype(f16)
    lb2_pad = np.zeros((U2, 1), np.float32)
    lb2_pad[:100, 0] = np.asarray(lb2, np.float32)
    cw_pad = np.zeros((P, N_CLASSES), np.float32)
    cw_pad[:100] = np.asarray(cw, np.float32)
    cbt = np.asarray(cb, np.float32).reshape(N_CLASSES, 1)

    in_maps = []
    for r in range(NCORES):
        in_maps.append({
            "featsg": pr.featsg[r],
            "idx16_1": pr.idx16_1[r],
            "dstc1": pr.sc1.dstcp[r][:, :, None],
            "idx16_2": pr.idx16_2[r],
            "dstc2": pr.sc2.dstcp[r][:, :, None],
            "iotaw": iotaw,
            "ones1": np.ones((1, P), dtype=f16),
            "ident": np.eye(P, dtype=f16),
            "poolw": pr.poolw[r],
            "desct": pr.desct[r],
            "w1": w1,
            "w2t": w2t,
            "b1v": b1v,
            "b2v": b2v,
            "lw1t": lw1t,
            "lb1t": lb1t,
            "lw2t": lw2t,
            "lb2t": lb2_pad,
            "cwt": cw_pad.astype(f16),
            "cbt": cbt,
        })

    return pr, nc, in_maps


def kernel(features, descriptors, src, dst, node2graph,
           W1, b1, W2, b2, lw1, lb1, lw2, lb2, cw, cb, _run_opts=None):
    opts0 = dict(_run_opts or {})
    opts0.pop("_last_result", None)
    pr, nc, in_maps = prepare(features, descriptors, src, dst, node2graph,
                              W1, b1, W2, b2, lw1, lb1, lw2, lb2, cw, cb)
    res = run_bass_kernel_spmd(nc, in_maps, core_ids=list(range(NCORES)), **opts0)

    out = np.zeros((pr.B, N_CLASSES), dtype=np.float32)
    for r in range(NCORES):
        o = np.asarray(res.results[r]["out"])
        for gi, (g0, g1) in enumerate(pr.core_groups[r]):
            ncols = g1 - g0
            if ncols > 0:
                out[g0:g1] = o[:, gi * P : gi * P + ncols].T
    if _run_opts is not None:
        _run_opts["_last_result"] = res
    return out


# revision 31
# speedup vs baseline: 2.0386x; 1.2994x over previous
"""Distributed 2-layer GCN + graph pooling + MLP head on 8 TRN2 NeuronCores.

Collective-free data-parallel strategy (per the sharding hint):
  - Graphs (and their nodes, contiguously -- node2graph is sorted) are
    partitioned into 8 shards with ~equal node counts. Weights replicated.
  - Each core owns the edges whose dst node it owns. Layer 2 needs
    h1[src] for those edges; instead of an AllGather, each core computes
    layer 1 *locally* for exactly the source nodes its edges reference
    (~40% of all nodes). No collectives -> no cross-core barrier.
  - Layer 1 aggregates raw *features* (segment_sum commutes with the
    linear map); its edge gather moves 128-dim rows from a per-core
    DEDUPED feature table (unique sources only -> 3 int16 windows).
  - h1 (fp16) for the needed sources is written to a core-local DRAM
    table; layer 2 gathers 512-dim rows from it (2 windows).
  - Edge gathers use GPSIMD dma_gather, whose descriptor-generation
    time (the kernel's serial bottleneck) is proportional to the index
    count. Each (group, window) run is packed contiguously per core
    (block boundaries fall mid-slot, so no per-(block,window) chunk
    quantization) and only the run tail is padded (with table row 0,
    dst column -1): total gathered slots track the real edge count to
    within ~5%, while the slot layout stays uniform across cores.
  - Segment-sum on chip: per aggregation block, a 0/1 indicator built by
    DVE (is_equal of a [0,128) iota row vs per-edge dst values) over the
    block's slot range turns edge chunks into PE matmuls:
    aggT[f, n] += G_slot[:, f].T @ I_col[:, n].  Slots shared between
    blocks are matmul'd into both blocks' PSUMs; the per-block dst
    columns carry -1 for foreign edges, zeroing their indicator.
  - Graph mean-pooling is another indicator matmul with 1/count weights
    (host-precomputed fp16), fused after layer 2 per node block.
  - The MLP head runs feature-major so biases are per-partition.

Device compute fp16 (PSUM fp32); biases fp32; output fp32.
"""

import sys

sys.path.insert(0, "/opt/trn_rl_repo")

import numpy as np

import concourse.bass as bass
import concourse.mybir as mybir
import concourse.tile as tile
from concourse import bacc
from concourse.bass_utils import run_bass_kernel_spmd

P = 128
NCORES = 8
IN_DIM = 128
HID = 512
N_DESC = 200
N_CLASSES = 2
U1 = 512  # padded head hidden 1 (500 -> 512)
U2 = 128  # padded head hidden 2 (100 -> 128)
WSZ = 32768  # int16 gather window (table rows per window)
SB1 = 64  # layer-1 gather-group slot budget (chunks of 128 edges)
SB2 = 20  # layer-2 gather-group slot budget

F16 = mybir.dt.float16
F32 = mybir.dt.float32
I16 = mybir.dt.int16


class Prep:
    pass


class Sched:
    pass


def _mk_sched(edata, nblocks, nwin, budget):
    """Contiguous-packing gather schedule, uniform across cores.

    edata[r] = (blk, win, dloc, gidx) int64 arrays per core: aggregation
    block, gather window, dst row-in-block [0,128), window-local gather
    row. Groups are consecutive block ranges sized so each group's total
    slot count stays <= budget. Returns a Sched with the group structure
    plus per-core packed int16 index streams (-1 tail padding) and
    per-block dst columns.
    """
    R = len(edata)

    cnt = np.zeros((R, nblocks, nwin), dtype=np.int64)
    for r, (blk, win, dloc, gidx) in enumerate(edata):
        np.add.at(cnt[r], (blk, win), 1)

    def group_slots(b0, b1):
        c = cnt[:, b0:b1, :].sum(axis=1)  # [R, nwin]
        return int(((c.max(axis=0) + P - 1) // P).sum())

    # greedy slot-budget grouping over consecutive blocks
    bounds = []
    b = 0
    while b < nblocks:
        b2 = b + 1
        while b2 < nblocks and group_slots(b, b2 + 1) <= budget:
            b2 += 1
        bounds.append((b, b2))
        b = b2
    ngroups = len(bounds)
    blk2grp = np.zeros(nblocks, dtype=np.int64)
    for g, (b0, b1) in enumerate(bounds):
        blk2grp[b0:b1] = g

    # pass 1: runs, per-(block,window) union slot ranges, indicator cols
    groups = []
    slot = 0
    col = 0
    rid = 0
    run_cnts = []  # per run: [R] real edge counts
    run_c0 = np.full((ngroups, nwin), -1, dtype=np.int64)
    s0_bw = np.full((nblocks, nwin), -1, dtype=np.int64)
    colbase_bw = np.full((nblocks, nwin), -1, dtype=np.int64)
    for g, (b0, b1) in enumerate(bounds):
        base = slot
        runs = []
        ranges = {b: [] for b in range(b0, b1)}  # (w, s0, s1) abs slots
        for w in range(nwin):
            c_r = cnt[:, b0:b1, w]  # [R, nb]
            tot = c_r.sum(axis=1)
            n = int((tot.max() + P - 1) // P)
            if n == 0:
                continue
            c0 = slot
            run_c0[g, w] = c0
            runs.append((w, c0, n, rid))
            run_cnts.append(tot.copy())
            rid += 1
            pfx = np.concatenate(
                [np.zeros((R, 1), dtype=np.int64), np.cumsum(c_r, axis=1)], axis=1
            )
            for bi in range(b1 - b0):
                m = c_r[:, bi] > 0
                if not m.any():
                    continue
                s0 = int((pfx[m, bi] // P).min()) + c0
                s1 = int(((pfx[m, bi + 1] - 1) // P).max()) + 1 + c0
                ranges[b0 + bi].append((w, s0, s1))
            slot += n
        if not runs:
            runs.append((0, slot, 1, rid))  # dummy run so the group tile exists
            run_cnts.append(np.zeros(R, dtype=np.int64))
            rid += 1
            run_c0[g, 0] = slot
            slot += 1
        # indicator columns per block (contiguous across its windows)
        bcols = {}
        bslots = {}
        for b in range(b0, b1):
            bcols[b] = col
            slots = []
            for w, s0, s1 in ranges[b]:
                s0_bw[b, w] = s0
                colbase_bw[b, w] = col + len(slots)
                slots.extend(range(s0 - base, s1 - base))
            if not slots:
                slots = [runs[0][1] - base]  # dummy col; dstc stays -1
            bslots[b] = slots
            col += len(slots)
        groups.append(
            dict(b0=b0, b1=b1, base=base, runs=runs, bcols=bcols, bslots=bslots,
                 cg=slot - base)
        )

    T, D = slot, col
    NRUNS = rid
    CG = max(gs["cg"] for gs in groups)
    RMAX = max(len(s) for gs in groups for s in gs["bslots"].values())

    # pass 2: per-core packed index streams and dst columns. Padding lanes
    # gather table row 0 (always valid); their dst columns stay -1 so the
    # indicator zeroes them. Every lane of every slot is written -> no
    # stale/NaN lanes, and the schedule stays a plain full-slot gather.
    idx_slot = np.zeros((R, T, P), dtype=np.int16)
    dstcp = np.full((R, P, D), -1.0, dtype=np.float16)
    nreal = np.zeros(R, dtype=np.int64)
    for r, (blk, win, dloc, gidx) in enumerate(edata):
        nreal[r] = len(blk)
        if len(blk) == 0:
            continue
        grp = blk2grp[blk]
        order = np.lexsort((blk, win, grp))
        blk_o, win_o = blk[order], win[order]
        dloc_o, gidx_o = dloc[order], gidx[order]
        grp_o = grp[order]
        key = grp_o * nwin + win_o
        starts = np.concatenate(
            [[0], np.cumsum(np.bincount(key, minlength=ngroups * nwin))]
        )
        pos = np.arange(len(key)) - starts[key]
        sabs = run_c0[grp_o, win_o] + pos // P
        lane = pos % P
        idx_slot[r][sabs, lane] = gidx_o.astype(np.int16)
        colv = colbase_bw[blk_o, win_o] + (sabs - s0_bw[blk_o, win_o])
        dstcp[r][lane, colv] = dloc_o.astype(np.float32)

    sc = Sched()
    sc.groups, sc.T, sc.D, sc.CG, sc.RMAX = groups, T, D, CG, RMAX
    sc.NRUNS = NRUNS
    sc.idx_slot, sc.dstcp, sc.nreal = idx_slot, dstcp, nreal
    return sc


def _pack_idx16(idx_slot):
    """[T, P] int16 slot-major stream -> dma_gather layout [128, T*8]."""
    wrapped = idx_slot.reshape(-1).reshape(-1, 16).T  # [16, T*8]
    return np.tile(wrapped, (8, 1))


# --------------------------------------------------------------------------
# Host-side preprocessing: partition, dedup tables, schedule
# --------------------------------------------------------------------------
def _preprocess(features, descriptors, src, dst, node2graph):
    pr = Prep()
    N = features.shape[0]
    B = descriptors.shape[0]

    n2g = np.asarray(node2graph).astype(np.int64)
    src = np.asarray(src).astype(np.int64)
    dst = np.asarray(dst).astype(np.int64)

    gstart = np.searchsorted(n2g, np.arange(B + 1))  # node range per graph

    # partition graphs into NCORES shards with ~equal node counts
    cuts = np.searchsorted(gstart, (np.arange(1, NCORES) * N) // NCORES)
    gcuts = np.concatenate([[0], cuts, [B]])

    # per-core pool groups of <=128 graphs; group nodes padded to 128-blocks
    core_groups = []
    for r in range(NCORES):
        g0, g1 = gcuts[r], gcuts[r + 1]
        groups = []
        g = g0
        while g < g1:
            ge = min(g + P, g1)
            groups.append((g, ge))
            g = ge
        if not groups:
            groups = [(g0, g0)]
        core_groups.append(groups)
    NG = max(len(gr) for gr in core_groups)

    blocks_per_group_core = np.zeros((NCORES, NG), dtype=np.int64)
    for r in range(NCORES):
        for gi, (g0, g1) in enumerate(core_groups[r]):
            nn = gstart[g1] - gstart[g0]
            blocks_per_group_core[r, gi] = max((nn + P - 1) // P, 1)
    bpg = blocks_per_group_core.max(axis=0)
    NB = int(bpg.sum())
    block_group = np.repeat(np.arange(NG), bpg)

    # padded-local index + owner of each node (layer-2 / pooling space)
    plocal = np.zeros(N, dtype=np.int64)
    owner = np.zeros(N, dtype=np.int64)
    group_base = np.concatenate([[0], np.cumsum(bpg) * P])
    for r in range(NCORES):
        for gi, (g0, g1) in enumerate(core_groups[r]):
            ns, ne = gstart[g0], gstart[g1]
            if ne > ns:
                plocal[ns:ne] = group_base[gi] + np.arange(ne - ns)
                owner[ns:ne] = r

    # per-core needed-source sets (sorted unique srcs of locally-owned edges)
    e_owner = owner[dst]
    uniq_r, l2_edges = [], []
    for r in range(NCORES):
        es = np.nonzero(e_owner == r)[0]
        uq = np.unique(src[es])
        uniq_r.append(uq)
        l2_edges.append(es)
    SBLK = max((len(u) + P - 1) // P for u in uniq_r)
    SROWS = SBLK * P
    NWIN2 = (SROWS + WSZ - 1) // WSZ
    assert SROWS <= 2 * WSZ, f"h1 table too large for int16 windows: {SROWS}"

    # layer-1 edges per core: all graph edges whose dst is a needed source.
    # Needed sources are ranked by in-degree (descending) so per-block edge
    # counts are similar across cores. The gather table is the per-core
    # deduped set of source features (unique srcs of layer-1 edges).
    l1_dat = []
    rank_maps = []
    usrcs = []
    for r in range(NCORES):
        uq = uniq_r[r]
        pos = np.searchsorted(uq, dst)
        pos_cl = np.minimum(pos, len(uq) - 1)
        m = uq[pos_cl] == dst  # edge's dst is in the needed set
        e1 = np.nonzero(m)[0]
        orank = pos[e1]
        indeg = np.bincount(orank, minlength=len(uq))
        order = np.argsort(-indeg, kind="stable")
        newrank = np.empty(len(uq), dtype=np.int64)
        newrank[order] = np.arange(len(uq))
        rank_maps.append(newrank)
        rank = newrank[orank]  # aggregation target (local row in h1 table)
        usrc = np.unique(src[e1])
        usrcs.append(usrc)
        gidx = np.searchsorted(usrc, src[e1])
        l1_dat.append((rank // P, gidx // WSZ, rank % P, gidx % WSZ))
    U = max(len(u) for u in usrcs)
    UPAD = ((U + P - 1) // P) * P
    NWIN1 = (UPAD + WSZ - 1) // WSZ

    sc1 = _mk_sched(l1_dat, SBLK, NWIN1, SB1)

    # layer-2 edges per core: local edges; src -> rank in needed set
    l2_dat = []
    for r in range(NCORES):
        es = l2_edges[r]
        dpl = plocal[dst[es]]
        rank = rank_maps[r][np.searchsorted(uniq_r[r], src[es])]
        l2_dat.append((dpl // P, rank // WSZ, dpl % P, rank % WSZ))

    sc2 = _mk_sched(l2_dat, NB, NWIN2, SB2)

    RMAX = max(sc1.RMAX, sc2.RMAX)

    idx16_1 = np.stack([_pack_idx16(sc1.idx_slot[r]) for r in range(NCORES)])
    idx16_2 = np.stack([_pack_idx16(sc2.idx_slot[r]) for r in range(NCORES)])

    # per-core deduped feature tables (fp16, padded)
    featsg = np.zeros((NCORES, UPAD, IN_DIM), dtype=np.float16)
    f16feat = np.asarray(features, np.float32).astype(np.float16)
    for r in range(NCORES):
        featsg[r, : len(usrcs[r])] = f16feat[usrcs[r]]

    # pooling weights [P(node-in-block), NB, P(graph-in-group)] = 1/count
    gcount = np.diff(gstart)
    inv_cnt = (1.0 / np.maximum(gcount, 1)).astype(np.float32)
    poolw = np.zeros((NCORES, P, NB, P), dtype=np.float16)
    for r in range(NCORES):
        for gi, (g0, g1) in enumerate(core_groups[r]):
            ns, ne = gstart[g0], gstart[g1]
            if ne <= ns:
                continue
            nodes = np.arange(ns, ne)
            pl = plocal[nodes]
            poolw[r, pl % P, pl // P, n2g[nodes] - g0] = inv_cnt[n2g[nodes]]

    # descriptors, feature-major, padded [P, 2, NG*P]
    GPAD = NG * P
    desc_pad = np.zeros((B, 256), dtype=np.float32)
    desc_pad[:, :N_DESC] = np.asarray(descriptors, dtype=np.float32)
    desct = np.zeros((NCORES, P, 2, GPAD), dtype=np.float16)
    for r in range(NCORES):
        for gi, (g0, g1) in enumerate(core_groups[r]):
            ncols = g1 - g0
            if ncols <= 0:
                continue
            blockd = desc_pad[g0:g1].T.reshape(2, P, ncols).transpose(1, 0, 2)
            desct[r, :, :, gi * P : gi * P + ncols] = blockd.astype(np.float16)

    pr.N, pr.B = N, B
    pr.NG, pr.NB, pr.GPAD = NG, NB, GPAD
    pr.SBLK, pr.SROWS, pr.NWIN2 = SBLK, SROWS, NWIN2
    pr.UPAD, pr.NWIN1 = UPAD, NWIN1
    pr.RMAX = RMAX
    pr.sc1, pr.sc2 = sc1, sc2
    pr.block_group = block_group
    pr.core_groups = core_groups
    pr.idx16_1, pr.idx16_2 = idx16_1, idx16_2
    pr.featsg, pr.poolw, pr.desct = featsg, poolw, desct
    return pr


# --------------------------------------------------------------------------
# Bass program builder (single SPMD program; per-core data via in_maps)
# --------------------------------------------------------------------------
def _build(pr):
    nc = bacc.Bacc("TRN2", target_bir_lowering=False, num_devices=NCORES,
                   num_swdge_queues=4)

    NB, NG, GPAD = pr.NB, pr.NG, pr.GPAD
    SROWS, NWIN2 = pr.SROWS, pr.NWIN2
    UPAD = pr.UPAD
    sc1, sc2 = pr.sc1, pr.sc2
    RMAX = pr.RMAX
    block_group = pr.block_group

    featsg_d = nc.dram_tensor("featsg", [UPAD, IN_DIM], F16, kind="ExternalInput")
    idx1_d = nc.dram_tensor("idx16_1", [P, sc1.T * 8], I16, kind="ExternalInput")
    dstc1_d = nc.dram_tensor("dstc1", [P, sc1.D, 1], F16, kind="ExternalInput")
    idx2_d = nc.dram_tensor("idx16_2", [P, sc2.T * 8], I16, kind="ExternalInput")
    dstc2_d = nc.dram_tensor("dstc2", [P, sc2.D, 1], F16, kind="ExternalInput")
    iotaw_d = nc.dram_tensor("iotaw", [P, RMAX, P], F16, kind="ExternalInput")
    ones1_d = nc.dram_tensor("ones1", [1, P], F16, kind="ExternalInput")
    ident_d = nc.dram_tensor("ident", [P, P], F16, kind="ExternalInput")
    poolw_d = nc.dram_tensor("poolw", [P, NB, P], F16, kind="ExternalInput")
    desct_d = nc.dram_tensor("desct", [P, 2, GPAD], F16, kind="ExternalInput")
    w1_d = nc.dram_tensor("w1", [P, HID], F16, kind="ExternalInput")
    w2t_d = nc.dram_tensor("w2t", [P, 4, HID], F16, kind="ExternalInput")
    b1_d = nc.dram_tensor("b1v", [1, HID], F16, kind="ExternalInput")
    b2_d = nc.dram_tensor("b2v", [1, HID], F16, kind="ExternalInput")
    lw1t_d = nc.dram_tensor("lw1t", [P, 6, U1], F16, kind="ExternalInput")
    lb1t_d = nc.dram_tensor("lb1t", [P, 4], F32, kind="ExternalInput")
    lw2t_d = nc.dram_tensor("lw2t", [P, 4, U2], F16, kind="ExternalInput")
    lb2t_d = nc.dram_tensor("lb2t", [P, 1], F32, kind="ExternalInput")
    cwt_d = nc.dram_tensor("cwt", [P, N_CLASSES], F16, kind="ExternalInput")
    cbt_d = nc.dram_tensor("cbt", [N_CLASSES, 1], F32, kind="ExternalInput")
    out_d = nc.dram_tensor("out", [N_CLASSES, GPAD], F32, kind="ExternalOutput")

    is_eq = mybir.AluOpType.is_equal
    add = mybir.AluOpType.add
    Copy = mybir.ActivationFunctionType.Copy
    Relu = mybir.ActivationFunctionType.Relu

    with tile.TileContext(nc) as tc:
        with (
            tc.tile_pool(name="const", bufs=1) as cp,
            tc.tile_pool(name="gath1", bufs=2) as gp1,
            tc.tile_pool(name="gath2", bufs=2) as gp2,
            tc.tile_pool(name="ind", bufs=3) as ip,
            tc.tile_pool(name="work", bufs=3) as wp,
            tc.tile_pool(name="psA", bufs=4, space="PSUM") as psA,
            tc.tile_pool(name="psB", bufs=3, space="PSUM") as psB,
            tc.tile_pool(name="psP", bufs=1, space="PSUM") as psP,
            tc.tile_pool(name="dram", bufs=1, space="DRAM") as dp,
        ):
            h1_d = dp.tile([SROWS, HID], F16)

            idx1_sb = cp.tile([P, sc1.T * 8], I16)
            nc.sync.dma_start(idx1_sb[:], idx1_d[:])
            dstc1_sb = cp.tile([P, sc1.D, 1], F16)
            nc.sync.dma_start(dstc1_sb[:], dstc1_d[:])
            idx2_sb = cp.tile([P, sc2.T * 8], I16)
            nc.sync.dma_start(idx2_sb[:], idx2_d[:])
            dstc2_sb = cp.tile([P, sc2.D, 1], F16)
            nc.sync.dma_start(dstc2_sb[:], dstc2_d[:])
            iotaw_sb = cp.tile([P, RMAX, P], F16)
            nc.sync.dma_start(iotaw_sb[:], iotaw_d[:])
            ones1_sb = cp.tile([1, P], F16)
            nc.sync.dma_start(ones1_sb[:], ones1_d[:])
            ident_sb = cp.tile([P, P], F16)
            nc.sync.dma_start(ident_sb[:], ident_d[:])
            poolw_sb = cp.tile([P, NB, P], F16)
            nc.sync.dma_start(poolw_sb[:], poolw_d[:])
            desct_sb = cp.tile([P, 2, GPAD], F16)
            nc.sync.dma_start(desct_sb[:], desct_d[:])
            w1_sb = cp.tile([P, HID], F16)
            nc.sync.dma_start(w1_sb[:], w1_d[:])
            w2t_sb = cp.tile([P, 4, HID], F16)
            nc.sync.dma_start(w2t_sb[:], w2t_d[:])
            b1_sb = cp.tile([1, HID], F16)
            nc.sync.dma_start(b1_sb[:], b1_d[:])
            b2_sb = cp.tile([1, HID], F16)
            nc.sync.dma_start(b2_sb[:], b2_d[:])
            lw1t_sb = cp.tile([P, 6, U1], F16)
            nc.sync.dma_start(lw1t_sb[:], lw1t_d[:])
            lb1t_sb = cp.tile([P, 4], F32)
            nc.sync.dma_start(lb1t_sb[:], lb1t_d[:])
            lw2t_sb = cp.tile([P, 4, U2], F16)
            nc.sync.dma_start(lw2t_sb[:], lw2t_d[:])
            lb2t_sb = cp.tile([P, 1], F32)
            nc.sync.dma_start(lb2t_sb[:], lb2t_d[:])
            cwt_sb = cp.tile([P, N_CLASSES], F16)
            nc.sync.dma_start(cwt_sb[:], cwt_d[:])
            cbt_sb = cp.tile([N_CLASSES, 1], F32)
            nc.sync.dma_start(cbt_sb[:], cbt_d[:])

            # round-robin gathers across the 4 SWDGE queues: each queue's
            # descriptor generation runs on a different Q7 core pair
            qrr = [0]

            def gather_group(gi, gs, gpool, cg, table, nrows, idx_sb, elem,
                             name):
                gt = gpool.tile([P, cg, elem], F16, tag=f"g{elem}",
                                name=f"{name}_{gs['base']}")
                for w, c0, n, rid in gs["runs"]:
                    lo, hi = w * WSZ, min((w + 1) * WSZ, nrows)
                    nc.gpsimd.dma_gather(
                        out_ap=gt[:, c0 - gs["base"] : c0 - gs["base"] + n, :],
                        in_ap=table[lo:hi, :],
                        idxs_ap=idx_sb[:, c0 * 8 : (c0 + n) * 8],
                        num_idxs=n * P,
                        num_idxs_reg=n * P,
                        elem_size=elem,
                        single_packet=False,
                        queue_num=qrr[0] % 4,
                    )
                    qrr[0] += 1
                return gt

            def indicator(gs, b, dstc_sb, name):
                slots = gs["bslots"][b]
                K = len(slots)
                c0 = gs["bcols"][b]
                ind = ip.tile([P, RMAX, P], F16, tag="ind", name=name)
                nc.vector.tensor_tensor(
                    out=ind[:, :K, :],
                    in0=iotaw_sb[:, :K, :],
                    in1=dstc_sb[:, c0 : c0 + K, :].to_broadcast((P, K, P)),
                    op=is_eq,
                )
                return ind, slots

            # ================= Layer 1 (needed sources) =================
            for gi, gs in enumerate(sc1.groups):
                g1 = gather_group(gi, gs, gp1, sc1.CG, featsg_d, UPAD, idx1_sb,
                                  IN_DIM, "g1")
                for b in range(gs["b0"], gs["b1"]):
                    ind1, slots = indicator(gs, b, dstc1_sb, f"i1_{b}")
                    aggT = psA.tile([P, P], F32, tag="psA", name=f"agg1_{b}")
                    for i, s in enumerate(slots):
                        nc.tensor.matmul(
                            out=aggT[:],
                            lhsT=g1[:, s, :],
                            rhs=ind1[:, i, :],
                            start=(i == 0),
                            stop=(i == len(slots) - 1),
                        )
                    aggT_sb = wp.tile([P, IN_DIM], F16, tag="agg1sb",
                                      name=f"agg1sb{b}")
                    nc.scalar.activation(aggT_sb[:], aggT[:], Copy)
                    h1ps = psB.tile([P, HID], F32, tag="psB", name=f"h1ps{b}")
                    nc.tensor.matmul(out=h1ps[:], lhsT=aggT_sb[:], rhs=w1_sb[:],
                                     start=True, stop=False)
                    nc.tensor.matmul(out=h1ps[:], lhsT=ones1_sb[:], rhs=b1_sb[:],
                                     start=False, stop=True)
                    h1 = wp.tile([P, HID], F16, tag="h1", name=f"h1_{b}")
                    nc.scalar.activation(h1[:], h1ps[:], Relu)
                    nc.sync.dma_start(h1_d[b * P : (b + 1) * P, :], h1[:])

            # ================= Layer 2 + pooling + head =================
            pool_ps = None
            for gi, gs in enumerate(sc2.groups):
                g2 = gather_group(gi, gs, gp2, sc2.CG, h1_d, SROWS, idx2_sb,
                                  HID, "g2")
                for b in range(gs["b0"], gs["b1"]):
                    grp = int(block_group[b])
                    first_in_grp = b == 0 or block_group[b - 1] != grp
                    last_in_grp = b == NB - 1 or block_group[b + 1] != grp

                    ind2, slots = indicator(gs, b, dstc2_sb, f"i2_{b}")
                    aggs = [
                        psA.tile([P, P], F32, tag="psA", name=f"agg2_{b}_{fc}")
                        for fc in range(4)
                    ]
                    for i, s in enumerate(slots):
                        for fc in range(4):
                            nc.tensor.matmul(
                                out=aggs[fc][:],
                                lhsT=g2[:, s, fc * P : (fc + 1) * P],
                                rhs=ind2[:, i, :],
                                start=(i == 0),
                                stop=(i == len(slots) - 1),
                            )
                    aggT_sb = wp.tile([P, 4, P], F16, tag="agg2sb",
                                      name=f"agg2sb{b}")
                    for fc in range(4):
                        if fc % 2 == 0:
                            nc.scalar.activation(aggT_sb[:, fc, :], aggs[fc][:],
                                                 Copy)
                        else:
                            nc.vector.tensor_copy(out=aggT_sb[:, fc, :],
                                                  in_=aggs[fc][:])
                    h2ps = psB.tile([P, HID], F32, tag="psB", name=f"h2ps{b}")
                    for fc in range(4):
                        nc.tensor.matmul(
                            out=h2ps[:],
                            lhsT=aggT_sb[:, fc, :],
                            rhs=w2t_sb[:, fc, :],
                            start=(fc == 0),
                            stop=False,
                        )
                    nc.tensor.matmul(out=h2ps[:], lhsT=ones1_sb[:], rhs=b2_sb[:],
                                     start=False, stop=True)
                    h2 = wp.tile([P, HID], F16, tag="h2", name=f"h2_{b}")
                    nc.scalar.activation(h2[:], h2ps[:], Relu)

                    if first_in_grp:
                        pool_ps = psP.tile([P, HID], F32, tag="psP",
                                           name=f"pool{grp}")
                    nc.tensor.matmul(
                        out=pool_ps[:],
                        lhsT=poolw_sb[:, b, :],
                        rhs=h2[:],
                        start=first_in_grp,
                        stop=last_in_grp,
                    )

                    if last_in_grp:
                        hg = wp.tile([P, HID], F16, tag="hg", name=f"hg{grp}")
                        nc.scalar.activation(hg[:], pool_ps[:], Copy)
                        hgT = wp.tile([P, 4, P], F16, tag="hgT", name=f"hgT{grp}")
                        for fc in range(4):
                            tps = psB.tile([P, P], F16, tag="psB",
                                           name=f"tps{grp}_{fc}")
                            nc.tensor.transpose(
                                out=tps[:],
                                in_=hg[:, fc * P : (fc + 1) * P],
                                identity=ident_sb[:],
                            )
                            nc.scalar.activation(hgT[:, fc, :], tps[:], Copy)
                        x1 = wp.tile([P, 4, P], F16, tag="x1", name=f"x1_{grp}")
                        for uc in range(4):
                            x1ps = psB.tile([P, P], F32, tag="psB",
                                            name=f"x1ps{grp}_{uc}")
                            for kc in range(6):
                                rhs = (
                                    hgT[:, kc, :]
                                    if kc < 4
                                    else desct_sb[:, kc - 4, grp * P : (grp + 1) * P]
                                )
                                nc.tensor.matmul(
                                    out=x1ps[:],
                                    lhsT=lw1t_sb[:, kc, uc * P : (uc + 1) * P],
                                    rhs=rhs,
                                    start=(kc == 0),
                                    stop=(kc == 5),
                                )
                            nc.scalar.activation(
                                x1[:, uc, :], x1ps[:], Relu,
                                bias=lb1t_sb[:, uc : uc + 1],
                            )
                        x2ps = psB.tile([P, P], F32, tag="psB", name=f"x2ps{grp}")
                        for kc in range(4):
                            nc.tensor.matmul(
                                out=x2ps[:],
                                lhsT=lw2t_sb[:, kc, :],
                                rhs=x1[:, kc, :],
                                start=(kc == 0),
                                stop=(kc == 3),
                            )
                        x2 = wp.tile([P, P], F16, tag="x2", name=f"x2_{grp}")
                        nc.scalar.activation(x2[:], x2ps[:], Relu,
                                             bias=lb2t_sb[:, :1])
                        lgps = psB.tile([P, P], F32, tag="psB", name=f"lg{grp}")
                        nc.tensor.matmul(
                            out=lgps[:N_CLASSES, :],
                            lhsT=cwt_sb[:],
                            rhs=x2[:],
                            start=True,
                            stop=True,
                        )
                        lg = wp.tile([N_CLASSES, P], F32, tag="lg",
                                     name=f"lgsb{grp}")
                        nc.vector.tensor_tensor(
                            out=lg[:],
                            in0=lgps[:N_CLASSES, :],
                            in1=cbt_sb[:, :1].to_broadcast((N_CLASSES, P)),
                            op=add,
                        )
                        nc.sync.dma_start(out_d[:, grp * P : (grp + 1) * P], lg[:])

    nc.compile()
    return nc


# --------------------------------------------------------------------------
# Entry point
# --------------------------------------------------------------------------
def prepare(features, descriptors, src, dst, node2graph,
            W1, b1, W2, b2, lw1, lb1, lw2, lb2, cw, cb):
    """Preprocess + build; returns (pr, nc, in_maps)."""
    pr = _preprocess(features, descriptors, src, dst, node2graph)
    nc = _build(pr)

    f16 = np.float16
    iotaw = np.broadcast_to(np.arange(P, dtype=f16), (P, pr.RMAX, P)).copy()

    w1 = np.asarray(W1, np.float32).astype(f16)
    w2t = np.asarray(W2, np.float32).reshape(4, P, HID).transpose(1, 0, 2).astype(f16)
    w2t = np.ascontiguousarray(w2t)
    b1v = np.asarray(b1, np.float32).reshape(1, HID).astype(f16)
    b2v = np.asarray(b2, np.float32).reshape(1, HID).astype(f16)

    KD = 768
    lw1_pad = np.zeros((KD, U1), np.float32)
    lw1_pad[: HID + N_DESC, :500] = np.asarray(lw1, np.float32)
    lw1t = np.ascontiguousarray(
        lw1_pad.reshape(6, P, U1).transpose(1, 0, 2)).astype(f16)
    lb1_pad = np.zeros((U1,), np.float32)
    lb1_pad[:500] = np.asarray(lb1, np.float32)
    lb1t = np.ascontiguousarray(lb1_pad.reshape(4, P).T)
    lw2_pad = np.zeros((U1, U2), np.float32)
    lw2_pad[:500, :100] = np.asarray(lw2, np.float32)
    lw2t = np.ascontiguousarray(
        lw2_pad.reshape(4, P, U2).transpose(1, 0, 2)).astype(f16)
    lb2_pad = np.zeros((U2, 1), np.float32)
    lb2_pad[:100, 0] = np.asarray(lb2, np.float32)
    cw_pad = np.zeros((P, N_CLASSES), np.float32)
    cw_pad[:100] = np.asarray(cw, np.float32)
    cbt = np.asarray(cb, np.float32).reshape(N_CLASSES, 1)

    in_maps = []
    for r in range(NCORES):
        in_maps.append({
            "featsg": pr.featsg[r],
            "idx16_1": pr.idx16_1[r],
            "dstc1": pr.sc1.dstcp[r][:, :, None],
            "idx16_2": pr.idx16_2[r],
            "dstc2": pr.sc2.dstcp[r][:, :, None],
            "iotaw": iotaw,
            "ones1": np.ones((1, P), dtype=f16),
            "ident": np.eye(P, dtype=f16),
            "poolw": pr.poolw[r],
            "desct": pr.desct[r],
            "w1": w1,
            "w2t": w2t,
            "b1v": b1v,
            "b2v": b2v,
            "lw1t": lw1t,
            "lb1t": lb1t,
            "lw2t": lw2t,
            "lb2t": lb2_pad,
            "cwt": cw_pad.astype(f16),
            "cbt": cbt,
        })

    return pr, nc, in_maps


def kernel(features, descriptors, src, dst, node2graph,
           W1, b1, W2, b2, lw1, lb1, lw2, lb2, cw, cb, _run_opts=None):
    opts0 = dict(_run_opts or {})
    opts0.pop("_last_result", None)
    pr, nc, in_maps = prepare(features, descriptors, src, dst, node2graph,
                              W1, b1, W2, b2, lw1, lb1, lw2, lb2, cw, cb)
    res = run_bass_kernel_spmd(nc, in_maps, core_ids=list(range(NCORES)), **opts0)

    out = np.zeros((pr.B, N_CLASSES), dtype=np.float32)
    for r in range(NCORES):
        o = np.asarray(res.results[r]["out"])
        for gi, (g0, g1) in enumerate(pr.core_groups[r]):
            ncols = g1 - g0
            if ncols > 0:
                out[g0:g1] = o[:, gi * P : gi * P + ncols].T
    if _run_opts is not None:
        _run_opts["_last_result"] = res
    return out


# revision 42
# speedup vs baseline: 2.4092x; 1.1818x over previous
"""Distributed 2-layer GCN + graph pooling + MLP head on 8 TRN2 NeuronCores.

Collective-free data-parallel strategy (per the sharding hint):
  - Graphs (and their nodes, contiguously -- node2graph is sorted) are
    partitioned into 8 shards with ~equal node counts. Weights replicated.
  - Each core owns the edges whose dst node it owns. Layer 2 needs
    h1[src] for those edges; instead of an AllGather, each core computes
    layer 1 *locally* for exactly the source nodes its edges reference
    (~40% of all nodes). No collectives -> no cross-core barrier.
  - Layer 1 aggregates raw *features* (segment_sum commutes with the
    linear map); its edge gather moves 128-dim rows from a per-core
    DEDUPED feature table (unique sources only -> 3 int16 windows).
  - h1 (fp16) for the needed sources is written to a core-local DRAM
    table; layer 2 gathers 512-dim rows from it (2 windows).
  - Edge gathers use GPSIMD dma_gather, whose descriptor-generation
    time (the kernel's serial bottleneck) is proportional to the index
    count. Each (group, window) run is packed contiguously per core
    (block boundaries fall mid-slot, so no per-(block,window) chunk
    quantization) and only the run tail is padded (with table row 0,
    dst column -1): total gathered slots track the real edge count to
    within ~5%, while the slot layout stays uniform across cores.
  - Segment-sum on chip: per aggregation block, a 0/1 indicator built by
    DVE (is_equal of a [0,128) iota row vs per-edge dst values) over the
    block's slot range turns edge chunks into PE matmuls:
    aggT[f, n] += G_slot[:, f].T @ I_col[:, n].  Slots shared between
    blocks are matmul'd into both blocks' PSUMs; the per-block dst
    columns carry -1 for foreign edges, zeroing their indicator.
  - Graph mean-pooling is another indicator matmul with 1/count weights
    (host-precomputed fp16), fused after layer 2 per node block.
  - The MLP head runs feature-major so biases are per-partition.

Device compute fp16 (PSUM fp32); biases fp32; output fp32.
"""

import sys

sys.path.insert(0, "/opt/trn_rl_repo")

import numpy as np

import concourse.bass as bass
import concourse.mybir as mybir
import concourse.tile as tile
from concourse import bacc
from concourse.bass_utils import run_bass_kernel_spmd

P = 128
NCORES = 8
IN_DIM = 128
HID = 512
N_DESC = 200
N_CLASSES = 2
U1 = 512  # padded head hidden 1 (500 -> 512)
U2 = 128  # padded head hidden 2 (100 -> 128)
WSZ = 32768  # int16 gather window (table rows per window)
SB1 = 64  # layer-1 gather-group slot budget (chunks of 128 edges)
SB2 = 20  # layer-2 gather-group slot budget

F16 = mybir.dt.float16
F32 = mybir.dt.float32
F8 = mybir.dt.float8e4
I16 = mybir.dt.int16
NP_F8 = mybir.dt.np(F8)
QUEUES = 4  # SWDGE queues to round-robin gathers over (1 for CoreSim runs)
FPAD = 256  # fp8 feature-table row (128 features + 128 zero pad -> 256B elem)


class Prep:
    pass


class Sched:
    pass


def _mk_sched(edata, nblocks, nwin, budget):
    """Contiguous-packing gather schedule, uniform across cores.

    edata[r] = (blk, win, dloc, gidx) int64 arrays per core: aggregation
    block, gather window, dst row-in-block [0,128), window-local gather
    row. Groups are consecutive block ranges sized so each group's total
    slot count stays <= budget. Returns a Sched with the group structure
    plus per-core packed int16 index streams (-1 tail padding) and
    per-block dst columns.
    """
    R = len(edata)

    cnt = np.zeros((R, nblocks, nwin), dtype=np.int64)
    for r, (blk, win, dloc, gidx) in enumerate(edata):
        np.add.at(cnt[r], (blk, win), 1)

    def group_slots(b0, b1):
        c = cnt[:, b0:b1, :].sum(axis=1)  # [R, nwin]
        return int(((c.max(axis=0) + P - 1) // P).sum())

    # greedy slot-budget grouping over consecutive blocks
    bounds = []
    b = 0
    while b < nblocks:
        b2 = b + 1
        while b2 < nblocks and group_slots(b, b2 + 1) <= budget:
            b2 += 1
        bounds.append((b, b2))
        b = b2
    ngroups = len(bounds)
    blk2grp = np.zeros(nblocks, dtype=np.int64)
    for g, (b0, b1) in enumerate(bounds):
        blk2grp[b0:b1] = g

    # pass 1: runs, per-(block,window) union slot ranges, indicator cols
    groups = []
    slot = 0
    col = 0
    rid = 0
    run_cnts = []  # per run: [R] real edge counts
    run_c0 = np.full((ngroups, nwin), -1, dtype=np.int64)
    s0_bw = np.full((nblocks, nwin), -1, dtype=np.int64)
    colbase_bw = np.full((nblocks, nwin), -1, dtype=np.int64)
    for g, (b0, b1) in enumerate(bounds):
        base = slot
        runs = []
        ranges = {b: [] for b in range(b0, b1)}  # (w, s0, s1) abs slots
        for w in range(nwin):
            c_r = cnt[:, b0:b1, w]  # [R, nb]
            tot = c_r.sum(axis=1)
            n = int((tot.max() + P - 1) // P)
            if n == 0:
                continue
            c0 = slot
            run_c0[g, w] = c0
            runs.append((w, c0, n, rid))
            run_cnts.append(tot.copy())
            rid += 1
            pfx = np.concatenate(
                [np.zeros((R, 1), dtype=np.int64), np.cumsum(c_r, axis=1)], axis=1
            )
            for bi in range(b1 - b0):
                m = c_r[:, bi] > 0
                if not m.any():
                    continue
                s0 = int((pfx[m, bi] // P).min()) + c0
                s1 = int(((pfx[m, bi + 1] - 1) // P).max()) + 1 + c0
                ranges[b0 + bi].append((w, s0, s1))
            slot += n
        if not runs:
            runs.append((0, slot, 1, rid))  # dummy run so the group tile exists
            run_cnts.append(np.zeros(R, dtype=np.int64))
            rid += 1
            run_c0[g, 0] = slot
            slot += 1
        # indicator columns per block (contiguous across its windows)
        bcols = {}
        bslots = {}
        for b in range(b0, b1):
            bcols[b] = col
            slots = []
            for w, s0, s1 in ranges[b]:
                s0_bw[b, w] = s0
                colbase_bw[b, w] = col + len(slots)
                slots.extend(range(s0 - base, s1 - base))
            if not slots:
                slots = [runs[0][1] - base]  # dummy col; dstc stays -1
            bslots[b] = slots
            col += len(slots)
        groups.append(
            dict(b0=b0, b1=b1, base=base, runs=runs, bcols=bcols, bslots=bslots,
                 cg=slot - base)
        )

    T, D = slot, col
    NRUNS = rid
    CG = max(gs["cg"] for gs in groups)
    RMAX = max(len(s) for gs in groups for s in gs["bslots"].values())

    # pass 2: per-core packed index streams and dst columns. Padding lanes
    # gather table row 0 (always valid); their dst columns stay -1 so the
    # indicator zeroes them. Every lane of every slot is written -> no
    # stale/NaN lanes, and the schedule stays a plain full-slot gather.
    idx_slot = np.zeros((R, T, P), dtype=np.int16)
    dstcp = np.full((R, P, D), -1.0, dtype=np.float16)
    nreal = np.zeros(R, dtype=np.int64)
    for r, (blk, win, dloc, gidx) in enumerate(edata):
        nreal[r] = len(blk)
        if len(blk) == 0:
            continue
        grp = blk2grp[blk]
        order = np.lexsort((blk, win, grp))
        blk_o, win_o = blk[order], win[order]
        dloc_o, gidx_o = dloc[order], gidx[order]
        grp_o = grp[order]
        key = grp_o * nwin + win_o
        starts = np.concatenate(
            [[0], np.cumsum(np.bincount(key, minlength=ngroups * nwin))]
        )
        pos = np.arange(len(key)) - starts[key]
        sabs = run_c0[grp_o, win_o] + pos // P
        lane = pos % P
        idx_slot[r][sabs, lane] = gidx_o.astype(np.int16)
        colv = colbase_bw[blk_o, win_o] + (sabs - s0_bw[blk_o, win_o])
        dstcp[r][lane, colv] = dloc_o.astype(np.float32)

    sc = Sched()
    sc.groups, sc.T, sc.D, sc.CG, sc.RMAX = groups, T, D, CG, RMAX
    sc.NRUNS = NRUNS
    sc.idx_slot, sc.dstcp, sc.nreal = idx_slot, dstcp, nreal
    return sc


def _pack_idx16(idx_slot):
    """[T, P] int16 slot-major stream -> dma_gather layout [128, T*8]."""
    wrapped = idx_slot.reshape(-1).reshape(-1, 16).T  # [16, T*8]
    return np.tile(wrapped, (8, 1))


# --------------------------------------------------------------------------
# Host-side preprocessing: partition, dedup tables, schedule
# --------------------------------------------------------------------------
def _preprocess(features, descriptors, src, dst, node2graph):
    pr = Prep()
    N = features.shape[0]
    B = descriptors.shape[0]

    n2g = np.asarray(node2graph).astype(np.int64)
    src = np.asarray(src).astype(np.int64)
    dst = np.asarray(dst).astype(np.int64)

    gstart = np.searchsorted(n2g, np.arange(B + 1))  # node range per graph

    # partition graphs into NCORES shards with ~equal node counts
    cuts = np.searchsorted(gstart, (np.arange(1, NCORES) * N) // NCORES)
    gcuts = np.concatenate([[0], cuts, [B]])

    # per-core pool groups of <=128 graphs; group nodes padded to 128-blocks
    core_groups = []
    for r in range(NCORES):
        g0, g1 = gcuts[r], gcuts[r + 1]
        groups = []
        g = g0
        while g < g1:
            ge = min(g + P, g1)
            groups.append((g, ge))
            g = ge
        if not groups:
            groups = [(g0, g0)]
        core_groups.append(groups)
    NG = max(len(gr) for gr in core_groups)

    blocks_per_group_core = np.zeros((NCORES, NG), dtype=np.int64)
    for r in range(NCORES):
        for gi, (g0, g1) in enumerate(core_groups[r]):
            nn = gstart[g1] - gstart[g0]
            blocks_per_group_core[r, gi] = max((nn + P - 1) // P, 1)
    bpg = blocks_per_group_core.max(axis=0)
    NB = int(bpg.sum())
    block_group = np.repeat(np.arange(NG), bpg)

    # padded-local index + owner of each node (layer-2 / pooling space)
    plocal = np.zeros(N, dtype=np.int64)
    owner = np.zeros(N, dtype=np.int64)
    group_base = np.concatenate([[0], np.cumsum(bpg) * P])
    for r in range(NCORES):
        for gi, (g0, g1) in enumerate(core_groups[r]):
            ns, ne = gstart[g0], gstart[g1]
            if ne > ns:
                plocal[ns:ne] = group_base[gi] + np.arange(ne - ns)
                owner[ns:ne] = r

    # per-core needed-source sets (sorted unique srcs of locally-owned edges)
    e_owner = owner[dst]
    uniq_r, l2_edges = [], []
    for r in range(NCORES):
        es = np.nonzero(e_owner == r)[0]
        uq = np.unique(src[es])
        uniq_r.append(uq)
        l2_edges.append(es)
    SBLK = max((len(u) + P - 1) // P for u in uniq_r)
    SROWS = SBLK * P
    NWIN2 = (SROWS + WSZ - 1) // WSZ
    assert SROWS <= 2 * WSZ, f"h1 table too large for int16 windows: {SROWS}"

    # layer-1 edges per core: all graph edges whose dst is a needed source.
    # Needed sources are ranked by in-degree (descending) so per-block edge
    # counts are similar across cores. The gather table is the per-core
    # deduped set of source features (unique srcs of layer-1 edges).
    l1_dat = []
    rank_maps = []
    usrcs = []
    for r in range(NCORES):
        uq = uniq_r[r]
        pos = np.searchsorted(uq, dst)
        pos_cl = np.minimum(pos, len(uq) - 1)
        m = uq[pos_cl] == dst  # edge's dst is in the needed set
        e1 = np.nonzero(m)[0]
        orank = pos[e1]
        indeg = np.bincount(orank, minlength=len(uq))
        order = np.argsort(-indeg, kind="stable")
        newrank = np.empty(len(uq), dtype=np.int64)
        newrank[order] = np.arange(len(uq))
        rank_maps.append(newrank)
        rank = newrank[orank]  # aggregation target (local row in h1 table)
        usrc = np.unique(src[e1])
        usrcs.append(usrc)
        gidx = np.searchsorted(usrc, src[e1])
        l1_dat.append((rank // P, gidx // WSZ, rank % P, gidx % WSZ))
    U = max(len(u) for u in usrcs)
    UPAD = ((U + P - 1) // P) * P
    NWIN1 = (UPAD + WSZ - 1) // WSZ

    sc1 = _mk_sched(l1_dat, SBLK, NWIN1, SB1)

    # layer-2 edges per core: local edges; src -> rank in needed set
    l2_dat = []
    for r in range(NCORES):
        es = l2_edges[r]
        dpl = plocal[dst[es]]
        rank = rank_maps[r][np.searchsorted(uniq_r[r], src[es])]
        l2_dat.append((dpl // P, rank // WSZ, dpl % P, rank % WSZ))

    sc2 = _mk_sched(l2_dat, NB, NWIN2, SB2)

    RMAX = max(sc1.RMAX, sc2.RMAX)

    idx16_1 = np.stack([_pack_idx16(sc1.idx_slot[r]) for r in range(NCORES)])
    idx16_2 = np.stack([_pack_idx16(sc2.idx_slot[r]) for r in range(NCORES)])

    # per-core deduped feature tables (fp8, rows padded to a 256B element)
    featsg = np.zeros((NCORES, UPAD, FPAD), dtype=NP_F8)
    f8feat = np.asarray(features, np.float32).astype(NP_F8)
    for r in range(NCORES):
        featsg[r, : len(usrcs[r]), :IN_DIM] = f8feat[usrcs[r]]

    # pooling weights [P(node-in-block), NB, P(graph-in-group)] = 1/count
    gcount = np.diff(gstart)
    inv_cnt = (1.0 / np.maximum(gcount, 1)).astype(np.float32)
    poolw = np.zeros((NCORES, P, NB, P), dtype=np.float16)
    for r in range(NCORES):
        for gi, (g0, g1) in enumerate(core_groups[r]):
            ns, ne = gstart[g0], gstart[g1]
            if ne <= ns:
                continue
            nodes = np.arange(ns, ne)
            pl = plocal[nodes]
            poolw[r, pl % P, pl // P, n2g[nodes] - g0] = inv_cnt[n2g[nodes]]

    # descriptors, feature-major, padded [P, 2, NG*P]
    GPAD = NG * P
    desc_pad = np.zeros((B, 256), dtype=np.float32)
    desc_pad[:, :N_DESC] = np.asarray(descriptors, dtype=np.float32)
    desct = np.zeros((NCORES, P, 2, GPAD), dtype=np.float16)
    for r in range(NCORES):
        for gi, (g0, g1) in enumerate(core_groups[r]):
            ncols = g1 - g0
            if ncols <= 0:
                continue
            blockd = desc_pad[g0:g1].T.reshape(2, P, ncols).transpose(1, 0, 2)
            desct[r, :, :, gi * P : gi * P + ncols] = blockd.astype(np.float16)

    pr.N, pr.B = N, B
    pr.NG, pr.NB, pr.GPAD = NG, NB, GPAD
    pr.SBLK, pr.SROWS, pr.NWIN2 = SBLK, SROWS, NWIN2
    pr.UPAD, pr.NWIN1 = UPAD, NWIN1
    pr.RMAX = RMAX
    pr.sc1, pr.sc2 = sc1, sc2
    pr.block_group = block_group
    pr.core_groups = core_groups
    pr.idx16_1, pr.idx16_2 = idx16_1, idx16_2
    pr.featsg, pr.poolw, pr.desct = featsg, poolw, desct
    return pr


# --------------------------------------------------------------------------
# Bass program builder (single SPMD program; per-core data via in_maps)
# --------------------------------------------------------------------------
def _build(pr):
    nc = bacc.Bacc("TRN2", target_bir_lowering=False, num_devices=NCORES,
                   num_swdge_queues=4)

    NB, NG, GPAD = pr.NB, pr.NG, pr.GPAD
    SROWS, NWIN2 = pr.SROWS, pr.NWIN2
    UPAD = pr.UPAD
    sc1, sc2 = pr.sc1, pr.sc2
    RMAX = pr.RMAX
    block_group = pr.block_group

    featsg_d = nc.dram_tensor("featsg", [UPAD, FPAD], F8, kind="ExternalInput")
    idx1_d = nc.dram_tensor("idx16_1", [P, sc1.T * 8], I16, kind="ExternalInput")
    dstc1_d = nc.dram_tensor("dstc1", [P, sc1.D, 1], F16, kind="ExternalInput")
    idx2_d = nc.dram_tensor("idx16_2", [P, sc2.T * 8], I16, kind="ExternalInput")
    dstc2_d = nc.dram_tensor("dstc2", [P, sc2.D, 1], F16, kind="ExternalInput")
    iotaw_d = nc.dram_tensor("iotaw", [P, RMAX, P], F16, kind="ExternalInput")
    ones1_d = nc.dram_tensor("ones1", [1, P], F16, kind="ExternalInput")
    ident_d = nc.dram_tensor("ident", [P, P], F16, kind="ExternalInput")
    poolw_d = nc.dram_tensor("poolw", [P, NB, P], F16, kind="ExternalInput")
    desct_d = nc.dram_tensor("desct", [P, 2, GPAD], F16, kind="ExternalInput")
    w1_d = nc.dram_tensor("w1", [P, HID], F16, kind="ExternalInput")
    w2t_d = nc.dram_tensor("w2t", [P, 4, HID], F16, kind="ExternalInput")
    b1_d = nc.dram_tensor("b1v", [1, HID], F16, kind="ExternalInput")
    b2_d = nc.dram_tensor("b2v", [1, HID], F16, kind="ExternalInput")
    lw1t_d = nc.dram_tensor("lw1t", [P, 6, U1], F16, kind="ExternalInput")
    lb1t_d = nc.dram_tensor("lb1t", [P, 4], F32, kind="ExternalInput")
    lw2t_d = nc.dram_tensor("lw2t", [P, 4, U2], F16, kind="ExternalInput")
    lb2t_d = nc.dram_tensor("lb2t", [P, 1], F32, kind="ExternalInput")
    cwt_d = nc.dram_tensor("cwt", [P, N_CLASSES], F16, kind="ExternalInput")
    cbt_d = nc.dram_tensor("cbt", [N_CLASSES, 1], F32, kind="ExternalInput")
    out_d = nc.dram_tensor("out", [N_CLASSES, GPAD], F32, kind="ExternalOutput")

    is_eq = mybir.AluOpType.is_equal
    add = mybir.AluOpType.add
    Copy = mybir.ActivationFunctionType.Copy
    Relu = mybir.ActivationFunctionType.Relu

    with tile.TileContext(nc) as tc:
        with (
            tc.tile_pool(name="const", bufs=1) as cp,
            tc.tile_pool(name="gath1", bufs=2) as gp1,
            tc.tile_pool(name="gath2", bufs=2) as gp2,
            tc.tile_pool(name="ind", bufs=3) as ip,
            tc.tile_pool(name="work", bufs=3) as wp,
            tc.tile_pool(name="psA", bufs=4, space="PSUM") as psA,
            tc.tile_pool(name="psB", bufs=3, space="PSUM") as psB,
            tc.tile_pool(name="psP", bufs=1, space="PSUM") as psP,
            tc.tile_pool(name="dram", bufs=1, space="DRAM") as dp,
        ):
            h1_d = dp.tile([SROWS, HID], F8)

            idx1_sb = cp.tile([P, sc1.T * 8], I16)
            nc.sync.dma_start(idx1_sb[:], idx1_d[:])
            dstc1_sb = cp.tile([P, sc1.D, 1], F16)
            nc.sync.dma_start(dstc1_sb[:], dstc1_d[:])
            idx2_sb = cp.tile([P, sc2.T * 8], I16)
            nc.sync.dma_start(idx2_sb[:], idx2_d[:])
            dstc2_sb = cp.tile([P, sc2.D, 1], F16)
            nc.sync.dma_start(dstc2_sb[:], dstc2_d[:])
            iotaw_sb = cp.tile([P, RMAX, P], F16)
            nc.sync.dma_start(iotaw_sb[:], iotaw_d[:])
            ones1_sb = cp.tile([1, P], F16)
            nc.sync.dma_start(ones1_sb[:], ones1_d[:])
            ident_sb = cp.tile([P, P], F16)
            nc.sync.dma_start(ident_sb[:], ident_d[:])
            poolw_sb = cp.tile([P, NB, P], F16)
            nc.sync.dma_start(poolw_sb[:], poolw_d[:])
            desct_sb = cp.tile([P, 2, GPAD], F16)
            nc.sync.dma_start(desct_sb[:], desct_d[:])
            w1_sb = cp.tile([P, HID], F16)
            nc.sync.dma_start(w1_sb[:], w1_d[:])
            w2t_sb = cp.tile([P, 4, HID], F16)
            nc.sync.dma_start(w2t_sb[:], w2t_d[:])
            b1_sb = cp.tile([1, HID], F16)
            nc.sync.dma_start(b1_sb[:], b1_d[:])
            b2_sb = cp.tile([1, HID], F16)
            nc.sync.dma_start(b2_sb[:], b2_d[:])
            lw1t_sb = cp.tile([P, 6, U1], F16)
            nc.sync.dma_start(lw1t_sb[:], lw1t_d[:])
            lb1t_sb = cp.tile([P, 4], F32)
            nc.sync.dma_start(lb1t_sb[:], lb1t_d[:])
            lw2t_sb = cp.tile([P, 4, U2], F16)
            nc.sync.dma_start(lw2t_sb[:], lw2t_d[:])
            lb2t_sb = cp.tile([P, 1], F32)
            nc.sync.dma_start(lb2t_sb[:], lb2t_d[:])
            cwt_sb = cp.tile([P, N_CLASSES], F16)
            nc.sync.dma_start(cwt_sb[:], cwt_d[:])
            cbt_sb = cp.tile([N_CLASSES, 1], F32)
            nc.sync.dma_start(cbt_sb[:], cbt_d[:])

            # round-robin gathers across the 4 SWDGE queues: each queue's
            # descriptor generation runs on a different Q7 core pair
            qrr = [0]

            def gather_group(gi, gs, gpool, cg, table, nrows, idx_sb, elem,
                             name):
                gt = gpool.tile([P, cg, elem], F8, tag=f"g{elem}",
                                name=f"{name}_{gs['base']}")
                for w, c0, n, rid in gs["runs"]:
                    lo, hi = w * WSZ, min((w + 1) * WSZ, nrows)
                    nc.gpsimd.dma_gather(
                        out_ap=gt[:, c0 - gs["base"] : c0 - gs["base"] + n, :],
                        in_ap=table[lo:hi, :],
                        idxs_ap=idx_sb[:, c0 * 8 : (c0 + n) * 8],
                        num_idxs=n * P,
                        num_idxs_reg=n * P,
                        elem_size=elem,
                        single_packet=False,
                        queue_num=qrr[0] % QUEUES,
                    )
                    qrr[0] += 1
                return gt

            def indicator(gs, b, dstc_sb, name):
                slots = gs["bslots"][b]
                K = len(slots)
                c0 = gs["bcols"][b]
                ind = ip.tile([P, RMAX, P], F8, tag="ind", name=name)
                nc.vector.tensor_tensor(
                    out=ind[:, :K, :],
                    in0=iotaw_sb[:, :K, :],
                    in1=dstc_sb[:, c0 : c0 + K, :].to_broadcast((P, K, P)),
                    op=is_eq,
                )
                return ind, slots

            # ================= Layer 1 (needed sources) =================
            for gi, gs in enumerate(sc1.groups):
                g1 = gather_group(gi, gs, gp1, sc1.CG, featsg_d, UPAD, idx1_sb,
                                  FPAD, "g1")
                for b in range(gs["b0"], gs["b1"]):
                    ind1, slots = indicator(gs, b, dstc1_sb, f"i1_{b}")
                    aggT = psA.tile([P, P], F32, tag="psA", name=f"agg1_{b}")
                    for i, s in enumerate(slots):
                        nc.tensor.matmul(
                            out=aggT[:],
                            lhsT=g1[:, s, :IN_DIM],
                            rhs=ind1[:, i, :],
                            start=(i == 0),
                            stop=(i == len(slots) - 1),
                        )
                    aggT_sb = wp.tile([P, IN_DIM], F16, tag="agg1sb",
                                      name=f"agg1sb{b}")
                    nc.scalar.activation(aggT_sb[:], aggT[:], Copy)
                    h1ps = psB.tile([P, HID], F32, tag="psB", name=f"h1ps{b}")
                    nc.tensor.matmul(out=h1ps[:], lhsT=aggT_sb[:], rhs=w1_sb[:],
                                     start=True, stop=False)
                    nc.tensor.matmul(out=h1ps[:], lhsT=ones1_sb[:], rhs=b1_sb[:],
                                     start=False, stop=True)
                    h1 = wp.tile([P, HID], F8, tag="h1", name=f"h1_{b}")
                    nc.scalar.activation(h1[:], h1ps[:], Relu)
                    nc.sync.dma_start(h1_d[b * P : (b + 1) * P, :], h1[:])

            # ================= Layer 2 + pooling + head =================
            pool_ps = None
            for gi, gs in enumerate(sc2.groups):
                g2 = gather_group(gi, gs, gp2, sc2.CG, h1_d, SROWS, idx2_sb,
                                  HID, "g2")  # fp8 rows: 512B elements
                for b in range(gs["b0"], gs["b1"]):
                    grp = int(block_group[b])
                    first_in_grp = b == 0 or block_group[b - 1] != grp
                    last_in_grp = b == NB - 1 or block_group[b + 1] != grp

                    ind2, slots = indicator(gs, b, dstc2_sb, f"i2_{b}")
                    aggs = [
                        psA.tile([P, P], F32, tag="psA", name=f"agg2_{b}_{fc}")
                        for fc in range(4)
                    ]
                    for i, s in enumerate(slots):
                        for fc in range(4):
                            nc.tensor.matmul(
                                out=aggs[fc][:],
                                lhsT=g2[:, s, fc * P : (fc + 1) * P],
                                rhs=ind2[:, i, :],
                                start=(i == 0),
                                stop=(i == len(slots) - 1),
                            )
                    aggT_sb = wp.tile([P, 4, P], F16, tag="agg2sb",
                                      name=f"agg2sb{b}")
                    for fc in range(4):
                        if fc % 2 == 0:
                            nc.scalar.activation(aggT_sb[:, fc, :], aggs[fc][:],
                                                 Copy)
                        else:
                            nc.vector.tensor_copy(out=aggT_sb[:, fc, :],
                                                  in_=aggs[fc][:])
                    h2ps = psB.tile([P, HID], F32, tag="psB", name=f"h2ps{b}")
                    for fc in range(4):
                        nc.tensor.matmul(
                            out=h2ps[:],
                            lhsT=aggT_sb[:, fc, :],
                            rhs=w2t_sb[:, fc, :],
                            start=(fc == 0),
                            stop=False,
                        )
                    nc.tensor.matmul(out=h2ps[:], lhsT=ones1_sb[:], rhs=b2_sb[:],
                                     start=False, stop=True)
                    h2 = wp.tile([P, HID], F16, tag="h2", name=f"h2_{b}")
                    nc.scalar.activation(h2[:], h2ps[:], Relu)

                    if first_in_grp:
                        pool_ps = psP.tile([P, HID], F32, tag="psP",
                                           name=f"pool{grp}")
                    nc.tensor.matmul(
                        out=pool_ps[:],
                        lhsT=poolw_sb[:, b, :],
                        rhs=h2[:],
                        start=first_in_grp,
                        stop=last_in_grp,
                    )

                    if last_in_grp:
                        hg = wp.tile([P, HID], F16, tag="hg", name=f"hg{grp}")
                        nc.scalar.activation(hg[:], pool_ps[:], Copy)
                        hgT = wp.tile([P, 4, P], F16, tag="hgT", name=f"hgT{grp}")
                        for fc in range(4):
                            tps = psB.tile([P, P], F16, tag="psB",
                                           name=f"tps{grp}_{fc}")
                            nc.tensor.transpose(
                                out=tps[:],
                                in_=hg[:, fc * P : (fc + 1) * P],
                                identity=ident_sb[:],
                            )
                            nc.scalar.activation(hgT[:, fc, :], tps[:], Copy)
                        x1 = wp.tile([P, 4, P], F16, tag="x1", name=f"x1_{grp}")
                        for uc in range(4):
                            x1ps = psB.tile([P, P], F32, tag="psB",
                                            name=f"x1ps{grp}_{uc}")
                            for kc in range(6):
                                rhs = (
                                    hgT[:, kc, :]
                                    if kc < 4
                                    else desct_sb[:, kc - 4, grp * P : (grp + 1) * P]
                                )
                                nc.tensor.matmul(
                                    out=x1ps[:],
                                    lhsT=lw1t_sb[:, kc, uc * P : (uc + 1) * P],
                                    rhs=rhs,
                                    start=(kc == 0),
                                    stop=(kc == 5),
                                )
                            nc.scalar.activation(
                                x1[:, uc, :], x1ps[:], Relu,
                                bias=lb1t_sb[:, uc : uc + 1],
                            )
                        x2ps = psB.tile([P, P], F32, tag="psB", name=f"x2ps{grp}")
                        for kc in range(4):
                            nc.tensor.matmul(
                                out=x2ps[:],
                                lhsT=lw2t_sb[:, kc, :],
                                rhs=x1[:, kc, :],
                                start=(kc == 0),
                                stop=(kc == 3),
                            )
                        x2 = wp.tile([P, P], F16, tag="x2", name=f"x2_{grp}")
                        nc.scalar.activation(x2[:], x2ps[:], Relu,
                                             bias=lb2t_sb[:, :1])
                        lgps = psB.tile([P, P], F32, tag="psB", name=f"lg{grp}")
                        nc.tensor.matmul(
                            out=lgps[:N_CLASSES, :],
                            lhsT=cwt_sb[:],
                            rhs=x2[:],
                            start=True,
                            stop=True,
                        )
                        lg = wp.tile([N_CLASSES, P], F32, tag="lg",
                                     name=f"lgsb{grp}")
                        nc.vector.tensor_tensor(
                            out=lg[:],
                            in0=lgps[:N_CLASSES, :],
                            in1=cbt_sb[:, :1].to_broadcast((N_CLASSES, P)),
                            op=add,
                        )
                        nc.sync.dma_start(out_d[:, grp * P : (grp + 1) * P], lg[:])

    nc.compile()
    return nc


# --------------------------------------------------------------------------
# Entry point
# --------------------------------------------------------------------------
def prepare(features, descriptors, src, dst, node2graph,
            W1, b1, W2, b2, lw1, lb1, lw2, lb2, cw, cb):
    """Preprocess + build; returns (pr, nc, in_maps)."""
    pr = _preprocess(features, descriptors, src, dst, node2graph)
    nc = _build(pr)

    f16 = np.float16
    iotaw = np.broadcast_to(np.arange(P, dtype=f16), (P, pr.RMAX, P)).copy()

    w1 = np.asarray(W1, np.float32).astype(f16)
    w2t = np.asarray(W2, np.float32).reshape(4, P, HID).transpose(1, 0, 2).astype(f16)
    w2t = np.ascontiguousarray(w2t)
    b1v = np.asarray(b1, np.float32).reshape(1, HID).astype(f16)
    b2v = np.asarray(b2, np.float32).reshape(1, HID).astype(f16)

    KD = 768
    lw1_pad = np.zeros((KD, U1), np.float32)
    lw1_pad[: HID + N_DESC, :500] = np.asarray(lw1, np.float32)
    lw1t = np.ascontiguousarray(
        lw1_pad.reshape(6, P, U1).transpose(1, 0, 2)).astype(f16)
    lb1_pad = np.zeros((U1,), np.float32)
    lb1_pad[:500] = np.asarray(lb1, np.float32)
    lb1t = np.ascontiguousarray(lb1_pad.reshape(4, P).T)
    lw2_pad = np.zeros((U1, U2), np.float32)
    lw2_pad[:500, :100] = np.asarray(lw2, np.float32)
    lw2t = np.ascontiguousarray(
        lw2_pad.reshape(4, P, U2).transpose(1, 0, 2)).astype(f16)
    lb2_pad = np.zeros((U2, 1), np.float32)
    lb2_pad[:100, 0] = np.asarray(lb2, np.float32)
    cw_pad = np.zeros((P, N_CLASSES), np.float32)
    cw_pad[:100] = np.asarray(cw, np.float32)
    cbt = np.asarray(cb, np.float32).reshape(N_CLASSES, 1)

    in_maps = []
    for r in range(NCORES):
        in_maps.append({
            "featsg": pr.featsg[r],
            "idx16_1": pr.idx16_1[r],
            "dstc1": pr.sc1.dstcp[r][:, :, None],
            "idx16_2": pr.idx16_2[r],
            "dstc2": pr.sc2.dstcp[r][:, :, None],
            "iotaw": iotaw,
            "ones1": np.ones((1, P), dtype=f16),
            "ident": np.eye(P, dtype=f16),
            "poolw": pr.poolw[r],
            "desct": pr.desct[r],
            "w1": w1,
            "w2t": w2t,
            "b1v": b1v,
            "b2v": b2v,
            "lw1t": lw1t,
            "lb1t": lb1t,
            "lw2t": lw2t,
            "lb2t": lb2_pad,
            "cwt": cw_pad.astype(f16),
            "cbt": cbt,
        })

    return pr, nc, in_maps


def kernel(features, descriptors, src, dst, node2graph,
           W1, b1, W2, b2, lw1, lb1, lw2, lb2, cw, cb, _run_opts=None):
    opts0 = dict(_run_opts or {})
    opts0.pop("_last_result", None)
    pr, nc, in_maps = prepare(features, descriptors, src, dst, node2graph,
                              W1, b1, W2, b2, lw1, lb1, lw2, lb2, cw, cb)
    res = run_bass_kernel_spmd(nc, in_maps, core_ids=list(range(NCORES)), **opts0)

    out = np.zeros((pr.B, N_CLASSES), dtype=np.float32)
    for r in range(NCORES):
        o = np.asarray(res.results[r]["out"])
        for gi, (g0, g1) in enumerate(pr.core_groups[r]):
            ncols = g1 - g0
            if ncols > 0:
                out[g0:g1] = o[:, gi * P : gi * P + ncols].T
    if _run_opts is not None:
        _run_opts["_last_result"] = res
    return out


# revision 56
# speedup vs baseline: 2.7208x; 1.1293x over previous
"""Distributed 2-layer GCN + graph pooling + MLP head on 8 TRN2 NeuronCores.

Collective-free data-parallel strategy (per the sharding hint):
  - Graphs (and their nodes, contiguously -- node2graph is sorted) are
    partitioned into 8 shards with ~equal node counts. Weights replicated.
  - Each core owns the edges whose dst node it owns. Layer 2 needs
    h1[src] for those edges; instead of an AllGather, each core computes
    layer 1 *locally* for exactly the source nodes its edges reference
    (~40% of all nodes). No collectives -> no cross-core barrier.
  - Layer 1 aggregates raw *features* (segment_sum commutes with the
    linear map); its edge gather moves 128-dim rows from a per-core
    DEDUPED feature table (unique sources only -> 3 int16 windows).
  - h1 (fp16) for the needed sources is written to a core-local DRAM
    table; layer 2 gathers 512-dim rows from it (2 windows).
  - Edge gathers use GPSIMD dma_gather, whose descriptor-generation
    time (the kernel's serial bottleneck) is proportional to the index
    count. Each (group, window) run is packed contiguously per core
    (block boundaries fall mid-slot, so no per-(block,window) chunk
    quantization) and only the run tail is padded (with table row 0,
    dst column -1): total gathered slots track the real edge count to
    within ~5%, while the slot layout stays uniform across cores.
  - Segment-sum on chip: per aggregation block, a 0/1 indicator built by
    DVE (is_equal of a [0,128) iota row vs per-edge dst values) over the
    block's slot range turns edge chunks into PE matmuls:
    aggT[f, n] += G_slot[:, f].T @ I_col[:, n].  Slots shared between
    blocks are matmul'd into both blocks' PSUMs; the per-block dst
    columns carry -1 for foreign edges, zeroing their indicator.
  - Graph mean-pooling is another indicator matmul with 1/count weights
    (host-precomputed fp16), fused after layer 2 per node block.
  - The MLP head runs feature-major so biases are per-partition.

Device compute fp16 (PSUM fp32); biases fp32; output fp32.
"""

import sys

sys.path.insert(0, "/opt/trn_rl_repo")

import numpy as np

import concourse.bass as bass
import concourse.mybir as mybir
import concourse.tile as tile
from concourse import bacc
from concourse.bass_utils import run_bass_kernel_spmd

P = 128
NCORES = 8
IN_DIM = 128
HID = 512
N_DESC = 200
N_CLASSES = 2
U1 = 512  # padded head hidden 1 (500 -> 512)
U2 = 128  # padded head hidden 2 (100 -> 128)
WSZ = 32768  # int16 gather window, layer-1 feature table
WSZ2 = 16384  # layer-2 h1-table window; small so early L2 gathers depend
#               only on early h1 blocks (kills the L1->L2 pipeline valley)
SB1 = 64  # layer-1 gather-group slot budget (chunks of 128 edges)
SB2 = 20  # layer-2 gather-group slot budget

F16 = mybir.dt.float16
F32 = mybir.dt.float32
F8 = mybir.dt.float8e4
I16 = mybir.dt.int16
I8 = mybir.dt.int8
NP_F8 = mybir.dt.np(F8)
QUEUES = 4  # SWDGE queues to round-robin gathers over (1 for CoreSim runs)
FPAD = 256  # fp8 feature-table row (128 features + 128 zero pad -> 256B elem)


class Prep:
    pass


class Sched:
    pass


def _mk_sched(edata, nblocks, nwin, budget):
    """Contiguous-packing gather schedule, uniform across cores.

    edata[r] = (blk, win, dloc, gidx) int64 arrays per core: aggregation
    block, gather window, dst row-in-block [0,128), window-local gather
    row. Groups are consecutive block ranges sized so each group's total
    slot count stays <= budget. Returns a Sched with the group structure
    plus per-core packed int16 index streams (-1 tail padding) and
    per-block dst columns.
    """
    R = len(edata)

    cnt = np.zeros((R, nblocks, nwin), dtype=np.int64)
    for r, (blk, win, dloc, gidx) in enumerate(edata):
        np.add.at(cnt[r], (blk, win), 1)

    def group_slots(b0, b1):
        c = cnt[:, b0:b1, :].sum(axis=1)  # [R, nwin]
        return int(((c.max(axis=0) + P - 1) // P).sum())

    # greedy slot-budget grouping over consecutive blocks
    bounds = []
    b = 0
    while b < nblocks:
        b2 = b + 1
        while b2 < nblocks and group_slots(b, b2 + 1) <= budget:
            b2 += 1
        bounds.append((b, b2))
        b = b2
    ngroups = len(bounds)
    blk2grp = np.zeros(nblocks, dtype=np.int64)
    for g, (b0, b1) in enumerate(bounds):
        blk2grp[b0:b1] = g

    # pass 1: runs, per-(block,window) union slot ranges, indicator cols
    groups = []
    slot = 0
    col = 0
    rid = 0
    run_cnts = []  # per run: [R] real edge counts
    run_c0 = np.full((ngroups, nwin), -1, dtype=np.int64)
    s0_bw = np.full((nblocks, nwin), -1, dtype=np.int64)
    colbase_bw = np.full((nblocks, nwin), -1, dtype=np.int64)
    for g, (b0, b1) in enumerate(bounds):
        base = slot
        runs = []
        ranges = {b: [] for b in range(b0, b1)}  # (w, s0, s1) abs slots
        for w in range(nwin):
            c_r = cnt[:, b0:b1, w]  # [R, nb]
            tot = c_r.sum(axis=1)
            n = int((tot.max() + P - 1) // P)
            if n == 0:
                continue
            c0 = slot
            run_c0[g, w] = c0
            runs.append((w, c0, n, rid))
            run_cnts.append(tot.copy())
            rid += 1
            pfx = np.concatenate(
                [np.zeros((R, 1), dtype=np.int64), np.cumsum(c_r, axis=1)], axis=1
            )
            for bi in range(b1 - b0):
                m = c_r[:, bi] > 0
                if not m.any():
                    continue
                s0 = int((pfx[m, bi] // P).min()) + c0
                s1 = int(((pfx[m, bi + 1] - 1) // P).max()) + 1 + c0
                ranges[b0 + bi].append((w, s0, s1))
            slot += n
        if not runs:
            runs.append((0, slot, 1, rid))  # dummy run so the group tile exists
            run_cnts.append(np.zeros(R, dtype=np.int64))
            rid += 1
            run_c0[g, 0] = slot
            slot += 1
        # indicator columns per block (contiguous across its windows)
        bcols = {}
        bslots = {}
        for b in range(b0, b1):
            bcols[b] = col
            slots = []
            for w, s0, s1 in ranges[b]:
                s0_bw[b, w] = s0
                colbase_bw[b, w] = col + len(slots)
                slots.extend(range(s0 - base, s1 - base))
            if not slots:
                slots = [runs[0][1] - base]  # dummy col; dstc stays -1
            bslots[b] = slots
            col += len(slots)
        groups.append(
            dict(b0=b0, b1=b1, base=base, runs=runs, bcols=bcols, bslots=bslots,
                 cg=slot - base)
        )

    T, D = slot, col
    NRUNS = rid
    CG = max(gs["cg"] for gs in groups)
    RMAX = max(len(s) for gs in groups for s in gs["bslots"].values())

    # pass 2: per-core packed index streams and dst columns. Padding lanes
    # gather table row 0 (always valid); their dst columns stay -1 so the
    # indicator zeroes them. Every lane of every slot is written -> no
    # stale/NaN lanes, and the schedule stays a plain full-slot gather.
    idx_slot = np.zeros((R, T, P), dtype=np.int16)
    dstcp = np.full((R, P, D), -1, dtype=np.int8)
    nreal = np.zeros(R, dtype=np.int64)
    for r, (blk, win, dloc, gidx) in enumerate(edata):
        nreal[r] = len(blk)
        if len(blk) == 0:
            continue
        grp = blk2grp[blk]
        order = np.lexsort((blk, win, grp))
        blk_o, win_o = blk[order], win[order]
        dloc_o, gidx_o = dloc[order], gidx[order]
        grp_o = grp[order]
        key = grp_o * nwin + win_o
        starts = np.concatenate(
            [[0], np.cumsum(np.bincount(key, minlength=ngroups * nwin))]
        )
        pos = np.arange(len(key)) - starts[key]
        sabs = run_c0[grp_o, win_o] + pos // P
        lane = pos % P
        idx_slot[r][sabs, lane] = gidx_o.astype(np.int16)
        colv = colbase_bw[blk_o, win_o] + (sabs - s0_bw[blk_o, win_o])
        dstcp[r][lane, colv] = dloc_o.astype(np.int8)

    sc = Sched()
    sc.groups, sc.T, sc.D, sc.CG, sc.RMAX = groups, T, D, CG, RMAX
    sc.NRUNS = NRUNS
    sc.idx_slot, sc.dstcp, sc.nreal = idx_slot, dstcp, nreal
    return sc


def _pack_idx16(idx_slot):
    """[T, P] int16 slot-major stream -> dma_gather layout [128, T*8]."""
    wrapped = idx_slot.reshape(-1).reshape(-1, 16).T  # [16, T*8]
    return np.tile(wrapped, (8, 1))


# --------------------------------------------------------------------------
# Host-side preprocessing: partition, dedup tables, schedule
# --------------------------------------------------------------------------
def _preprocess(features, descriptors, src, dst, node2graph):
    pr = Prep()
    N = features.shape[0]
    B = descriptors.shape[0]

    n2g = np.asarray(node2graph).astype(np.int64)
    src = np.asarray(src).astype(np.int64)
    dst = np.asarray(dst).astype(np.int64)

    gstart = np.searchsorted(n2g, np.arange(B + 1))  # node range per graph

    # partition graphs into NCORES shards with ~equal node counts
    cuts = np.searchsorted(gstart, (np.arange(1, NCORES) * N) // NCORES)
    gcuts = np.concatenate([[0], cuts, [B]])

    # per-core pool groups of <=128 graphs; group nodes padded to 128-blocks
    core_groups = []
    for r in range(NCORES):
        g0, g1 = gcuts[r], gcuts[r + 1]
        groups = []
        g = g0
        while g < g1:
            ge = min(g + P, g1)
            groups.append((g, ge))
            g = ge
        if not groups:
            groups = [(g0, g0)]
        core_groups.append(groups)
    NG = max(len(gr) for gr in core_groups)

    blocks_per_group_core = np.zeros((NCORES, NG), dtype=np.int64)
    for r in range(NCORES):
        for gi, (g0, g1) in enumerate(core_groups[r]):
            nn = gstart[g1] - gstart[g0]
            blocks_per_group_core[r, gi] = max((nn + P - 1) // P, 1)
    bpg = blocks_per_group_core.max(axis=0)
    NB = int(bpg.sum())
    block_group = np.repeat(np.arange(NG), bpg)

    # padded-local index + owner of each node (layer-2 / pooling space)
    plocal = np.zeros(N, dtype=np.int64)
    owner = np.zeros(N, dtype=np.int64)
    group_base = np.concatenate([[0], np.cumsum(bpg) * P])
    for r in range(NCORES):
        for gi, (g0, g1) in enumerate(core_groups[r]):
            ns, ne = gstart[g0], gstart[g1]
            if ne > ns:
                plocal[ns:ne] = group_base[gi] + np.arange(ne - ns)
                owner[ns:ne] = r

    # per-core needed-source sets (sorted unique srcs of locally-owned edges)
    e_owner = owner[dst]
    uniq_r, l2_edges = [], []
    for r in range(NCORES):
        es = np.nonzero(e_owner == r)[0]
        uq = np.unique(src[es])
        uniq_r.append(uq)
        l2_edges.append(es)
    SBLK = max((len(u) + P - 1) // P for u in uniq_r)
    SROWS = SBLK * P
    NWIN2 = (SROWS + WSZ2 - 1) // WSZ2

    # layer-1 edges per core: all graph edges whose dst is a needed source.
    # Needed sources are ranked by in-degree (descending) so per-block edge
    # counts are similar across cores. The gather table is the per-core
    # deduped set of source features (unique srcs of layer-1 edges).
    l1_dat = []
    rank_maps = []
    usrcs = []
    for r in range(NCORES):
        uq = uniq_r[r]
        pos = np.searchsorted(uq, dst)
        pos_cl = np.minimum(pos, len(uq) - 1)
        m = uq[pos_cl] == dst  # edge's dst is in the needed set
        e1 = np.nonzero(m)[0]
        orank = pos[e1]
        indeg = np.bincount(orank, minlength=len(uq))
        order = np.argsort(-indeg, kind="stable")
        newrank = np.empty(len(uq), dtype=np.int64)
        newrank[order] = np.arange(len(uq))
        rank_maps.append(newrank)
        rank = newrank[orank]  # aggregation target (local row in h1 table)
        usrc = np.unique(src[e1])
        usrcs.append(usrc)
        gidx = np.searchsorted(usrc, src[e1])
        l1_dat.append((rank // P, gidx // WSZ, rank % P, gidx % WSZ))
    U = max(len(u) for u in usrcs)
    UPAD = ((U + P - 1) // P) * P
    NWIN1 = (UPAD + WSZ - 1) // WSZ

    sc1 = _mk_sched(l1_dat, SBLK, NWIN1, SB1)

    # layer-2 edges per core: local edges; src -> rank in needed set
    l2_dat = []
    for r in range(NCORES):
        es = l2_edges[r]
        dpl = plocal[dst[es]]
        rank = rank_maps[r][np.searchsorted(uniq_r[r], src[es])]
        l2_dat.append((dpl // P, rank // WSZ2, dpl % P, rank % WSZ2))

    sc2 = _mk_sched(l2_dat, NB, NWIN2, SB2)

    RMAX = max(sc1.RMAX, sc2.RMAX)

    idx16_1 = np.stack([_pack_idx16(sc1.idx_slot[r]) for r in range(NCORES)])
    idx16_2 = np.stack([_pack_idx16(sc2.idx_slot[r]) for r in range(NCORES)])

    # per-core deduped feature tables (fp8, rows padded to a 256B element)
    featsg = np.zeros((NCORES, UPAD, FPAD), dtype=NP_F8)
    f8feat = np.asarray(features, np.float32).astype(NP_F8)
    for r in range(NCORES):
        featsg[r, : len(usrcs[r]), :IN_DIM] = f8feat[usrcs[r]]

    # pooling weights [P(node-in-block), NB, P(graph-in-group)] = 1/count
    gcount = np.diff(gstart)
    inv_cnt = (1.0 / np.maximum(gcount, 1)).astype(np.float32)
    poolw = np.zeros((NCORES, P, NB, P), dtype=np.float16)
    for r in range(NCORES):
        for gi, (g0, g1) in enumerate(core_groups[r]):
            ns, ne = gstart[g0], gstart[g1]
            if ne <= ns:
                continue
            nodes = np.arange(ns, ne)
            pl = plocal[nodes]
            poolw[r, pl % P, pl // P, n2g[nodes] - g0] = inv_cnt[n2g[nodes]]

    # descriptors, feature-major, padded [P, 2, NG*P]
    GPAD = NG * P
    desc_pad = np.zeros((B, 256), dtype=np.float32)
    desc_pad[:, :N_DESC] = np.asarray(descriptors, dtype=np.float32)
    desct = np.zeros((NCORES, P, 2, GPAD), dtype=np.float16)
    for r in range(NCORES):
        for gi, (g0, g1) in enumerate(core_groups[r]):
            ncols = g1 - g0
            if ncols <= 0:
                continue
            blockd = desc_pad[g0:g1].T.reshape(2, P, ncols).transpose(1, 0, 2)
            desct[r, :, :, gi * P : gi * P + ncols] = blockd.astype(np.float16)

    pr.N, pr.B = N, B
    pr.NG, pr.NB, pr.GPAD = NG, NB, GPAD
    pr.SBLK, pr.SROWS, pr.NWIN2 = SBLK, SROWS, NWIN2
    pr.UPAD, pr.NWIN1 = UPAD, NWIN1
    pr.RMAX = RMAX
    pr.sc1, pr.sc2 = sc1, sc2
    pr.block_group = block_group
    pr.core_groups = core_groups
    pr.idx16_1, pr.idx16_2 = idx16_1, idx16_2
    pr.featsg, pr.poolw, pr.desct = featsg, poolw, desct
    return pr


# --------------------------------------------------------------------------
# Bass program builder (single SPMD program; per-core data via in_maps)
# --------------------------------------------------------------------------
def _build(pr):
    nc = bacc.Bacc("TRN2", target_bir_lowering=False, num_devices=NCORES,
                   num_swdge_queues=4)

    NB, NG, GPAD = pr.NB, pr.NG, pr.GPAD
    SROWS, NWIN2 = pr.SROWS, pr.NWIN2
    UPAD = pr.UPAD
    sc1, sc2 = pr.sc1, pr.sc2
    RMAX = pr.RMAX
    block_group = pr.block_group

    featsg_d = nc.dram_tensor("featsg", [UPAD, FPAD], F8, kind="ExternalInput")
    idx1_d = nc.dram_tensor("idx16_1", [P, sc1.T * 8], I16, kind="ExternalInput")
    dstc1_d = nc.dram_tensor("dstc1", [P, sc1.D, 1], I8, kind="ExternalInput")
    idx2_d = nc.dram_tensor("idx16_2", [P, sc2.T * 8], I16, kind="ExternalInput")
    dstc2_d = nc.dram_tensor("dstc2", [P, sc2.D, 1], I8, kind="ExternalInput")
    iotaw_d = nc.dram_tensor("iotaw", [P, RMAX, P], I8, kind="ExternalInput")
    ones1_d = nc.dram_tensor("ones1", [1, P], F16, kind="ExternalInput")
    ident_d = nc.dram_tensor("ident", [P, P], F16, kind="ExternalInput")
    poolw_d = nc.dram_tensor("poolw", [P, NB, P], F16, kind="ExternalInput")
    desct_d = nc.dram_tensor("desct", [P, 2, GPAD], F16, kind="ExternalInput")
    w1_d = nc.dram_tensor("w1", [P, HID], F16, kind="ExternalInput")
    w2t_d = nc.dram_tensor("w2t", [P, 4, HID], F16, kind="ExternalInput")
    b1_d = nc.dram_tensor("b1v", [1, HID], F16, kind="ExternalInput")
    b2_d = nc.dram_tensor("b2v", [1, HID], F16, kind="ExternalInput")
    lw1t_d = nc.dram_tensor("lw1t", [P, 6, U1], F16, kind="ExternalInput")
    lb1t_d = nc.dram_tensor("lb1t", [P, 4], F32, kind="ExternalInput")
    lw2t_d = nc.dram_tensor("lw2t", [P, 4, U2], F16, kind="ExternalInput")
    lb2t_d = nc.dram_tensor("lb2t", [P, 1], F32, kind="ExternalInput")
    cwt_d = nc.dram_tensor("cwt", [P, N_CLASSES], F16, kind="ExternalInput")
    cbt_d = nc.dram_tensor("cbt", [N_CLASSES, 1], F32, kind="ExternalInput")
    out_d = nc.dram_tensor("out", [N_CLASSES, GPAD], F32, kind="ExternalOutput")

    is_eq = mybir.AluOpType.is_equal
    add = mybir.AluOpType.add
    Copy = mybir.ActivationFunctionType.Copy
    Relu = mybir.ActivationFunctionType.Relu

    with tile.TileContext(nc) as tc:
        with (
            tc.tile_pool(name="const", bufs=1) as cp,
            tc.tile_pool(name="gath1", bufs=3) as gp1,
            tc.tile_pool(name="gath2", bufs=3) as gp2,
            tc.tile_pool(name="ind", bufs=3) as ip,
            tc.tile_pool(name="work", bufs=3) as wp,
            tc.tile_pool(name="psA", bufs=4, space="PSUM") as psA,
            tc.tile_pool(name="psB", bufs=3, space="PSUM") as psB,
            tc.tile_pool(name="psP", bufs=1, space="PSUM") as psP,
            tc.tile_pool(name="dram", bufs=1, space="DRAM") as dp,
        ):
            h1_d = dp.tile([SROWS, HID], F8)

            idx1_sb = cp.tile([P, sc1.T * 8], I16)
            nc.sync.dma_start(idx1_sb[:], idx1_d[:])
            dstc1_sb = cp.tile([P, sc1.D, 1], I8)
            nc.sync.dma_start(dstc1_sb[:], dstc1_d[:])
            idx2_sb = cp.tile([P, sc2.T * 8], I16)
            nc.sync.dma_start(idx2_sb[:], idx2_d[:])
            dstc2_sb = cp.tile([P, sc2.D, 1], I8)
            nc.sync.dma_start(dstc2_sb[:], dstc2_d[:])
            iotaw_sb = cp.tile([P, RMAX, P], I8)
            nc.sync.dma_start(iotaw_sb[:], iotaw_d[:])
            ones1_sb = cp.tile([1, P], F16)
            nc.sync.dma_start(ones1_sb[:], ones1_d[:])
            ident_sb = cp.tile([P, P], F16)
            nc.sync.dma_start(ident_sb[:], ident_d[:])
            poolw_sb = cp.tile([P, NB, P], F16)
            nc.sync.dma_start(poolw_sb[:], poolw_d[:])
            desct_sb = cp.tile([P, 2, GPAD], F16)
            nc.sync.dma_start(desct_sb[:], desct_d[:])
            w1_sb = cp.tile([P, HID], F16)
            nc.sync.dma_start(w1_sb[:], w1_d[:])
            w2t_sb = cp.tile([P, 4, HID], F16)
            nc.sync.dma_start(w2t_sb[:], w2t_d[:])
            b1_sb = cp.tile([1, HID], F16)
            nc.sync.dma_start(b1_sb[:], b1_d[:])
            b2_sb = cp.tile([1, HID], F16)
            nc.sync.dma_start(b2_sb[:], b2_d[:])
            lw1t_sb = cp.tile([P, 6, U1], F16)
            nc.sync.dma_start(lw1t_sb[:], lw1t_d[:])
            lb1t_sb = cp.tile([P, 4], F32)
            nc.sync.dma_start(lb1t_sb[:], lb1t_d[:])
            lw2t_sb = cp.tile([P, 4, U2], F16)
            nc.sync.dma_start(lw2t_sb[:], lw2t_d[:])
            lb2t_sb = cp.tile([P, 1], F32)
            nc.sync.dma_start(lb2t_sb[:], lb2t_d[:])
            cwt_sb = cp.tile([P, N_CLASSES], F16)
            nc.sync.dma_start(cwt_sb[:], cwt_d[:])
            cbt_sb = cp.tile([N_CLASSES, 1], F32)
            nc.sync.dma_start(cbt_sb[:], cbt_d[:])

            # round-robin gathers across the 4 SWDGE queues: each queue's
            # descriptor generation runs on a different Q7 core pair
            qrr = [0]

            def gather_group(gi, gs, gpool, cg, table, nrows, idx_sb, elem,
                             wsz, name):
                gt = gpool.tile([P, cg, elem], F8, tag=f"g{elem}",
                                name=f"{name}_{gs['base']}")
                for w, c0, n, rid in gs["runs"]:
                    lo, hi = w * wsz, min((w + 1) * wsz, nrows)
                    nc.gpsimd.dma_gather(
                        out_ap=gt[:, c0 - gs["base"] : c0 - gs["base"] + n, :],
                        in_ap=table[lo:hi, :],
                        idxs_ap=idx_sb[:, c0 * 8 : (c0 + n) * 8],
                        num_idxs=n * P,
                        num_idxs_reg=n * P,
                        elem_size=elem,
                        single_packet=False,
                        queue_num=qrr[0] % QUEUES,
                    )
                    qrr[0] += 1
                return gt

            def indicator(gs, b, dstc_sb, name):
                slots = gs["bslots"][b]
                K = len(slots)
                c0 = gs["bcols"][b]
                ind = ip.tile([P, RMAX, P], F8, tag="ind", name=name)
                nc.vector.tensor_tensor(
                    out=ind[:, :K, :],
                    in0=iotaw_sb[:, :K, :],
                    in1=dstc_sb[:, c0 : c0 + K, :].to_broadcast((P, K, P)),
                    op=is_eq,
                )
                return ind, slots

            # ================= Layer 1 (needed sources) =================
            for gi, gs in enumerate(sc1.groups):
                g1 = gather_group(gi, gs, gp1, sc1.CG, featsg_d, UPAD, idx1_sb,
                                  FPAD, WSZ, "g1")
                for b in range(gs["b0"], gs["b1"]):
                    ind1, slots = indicator(gs, b, dstc1_sb, f"i1_{b}")
                    aggT = psA.tile([P, P], F32, tag="psA", name=f"agg1_{b}")
                    for i, s in enumerate(slots):
                        nc.tensor.matmul(
                            out=aggT[:],
                            lhsT=g1[:, s, :IN_DIM],
                            rhs=ind1[:, i, :],
                            start=(i == 0),
                            stop=(i == len(slots) - 1),
                        )
                    aggT_sb = wp.tile([P, IN_DIM], F16, tag="agg1sb",
                                      name=f"agg1sb{b}")
                    nc.scalar.activation(aggT_sb[:], aggT[:], Copy)
                    h1ps = psB.tile([P, HID], F32, tag="psB", name=f"h1ps{b}")
                    nc.tensor.matmul(out=h1ps[:], lhsT=aggT_sb[:], rhs=w1_sb[:],
                                     start=True, stop=False)
                    nc.tensor.matmul(out=h1ps[:], lhsT=ones1_sb[:], rhs=b1_sb[:],
                                     start=False, stop=True)
                    h1 = wp.tile([P, HID], F8, tag="h1", name=f"h1_{b}")
                    nc.scalar.activation(h1[:], h1ps[:], Relu)
                    nc.sync.dma_start(h1_d[b * P : (b + 1) * P, :], h1[:])

            # ================= Layer 2 + pooling + head =================
            pool_ps = None
            for gi, gs in enumerate(sc2.groups):
                g2 = gather_group(gi, gs, gp2, sc2.CG, h1_d, SROWS, idx2_sb,
                                  HID, WSZ2, "g2")  # fp8 rows: 512B elements
                for b in range(gs["b0"], gs["b1"]):
                    grp = int(block_group[b])
                    first_in_grp = b == 0 or block_group[b - 1] != grp
                    last_in_grp = b == NB - 1 or block_group[b + 1] != grp

                    ind2, slots = indicator(gs, b, dstc2_sb, f"i2_{b}")
                    aggs = [
                        psA.tile([P, P], F32, tag="psA", name=f"agg2_{b}_{fc}")
                        for fc in range(4)
                    ]
                    for i, s in enumerate(slots):
                        for fc in range(4):
                            nc.tensor.matmul(
                                out=aggs[fc][:],
                                lhsT=g2[:, s, fc * P : (fc + 1) * P],
                                rhs=ind2[:, i, :],
                                start=(i == 0),
                                stop=(i == len(slots) - 1),
                            )
                    aggT_sb = wp.tile([P, 4, P], F16, tag="agg2sb",
                                      name=f"agg2sb{b}")
                    for fc in range(4):
                        if fc % 2 == 0:
                            nc.scalar.activation(aggT_sb[:, fc, :], aggs[fc][:],
                                                 Copy)
                        else:
                            nc.vector.tensor_copy(out=aggT_sb[:, fc, :],
                                                  in_=aggs[fc][:])
                    h2ps = psB.tile([P, HID], F32, tag="psB", name=f"h2ps{b}")
                    for fc in range(4):
                        nc.tensor.matmul(
                            out=h2ps[:],
                            lhsT=aggT_sb[:, fc, :],
                            rhs=w2t_sb[:, fc, :],
                            start=(fc == 0),
                            stop=False,
                        )
                    nc.tensor.matmul(out=h2ps[:], lhsT=ones1_sb[:], rhs=b2_sb[:],
                                     start=False, stop=True)
                    h2 = wp.tile([P, HID], F16, tag="h2", name=f"h2_{b}")
                    nc.scalar.activation(h2[:], h2ps[:], Relu)

                    if first_in_grp:
                        pool_ps = psP.tile([P, HID], F32, tag="psP",
                                           name=f"pool{grp}")
                    nc.tensor.matmul(
                        out=pool_ps[:],
                        lhsT=poolw_sb[:, b, :],
                        rhs=h2[:],
                        start=first_in_grp,
                        stop=last_in_grp,
                    )

                    if last_in_grp:
                        hg = wp.tile([P, HID], F16, tag="hg", name=f"hg{grp}")
                        nc.scalar.activation(hg[:], pool_ps[:], Copy)
                        hgT = wp.tile([P, 4, P], F16, tag="hgT", name=f"hgT{grp}")
                        for fc in range(4):
                            tps = psB.tile([P, P], F16, tag="psB",
                                           name=f"tps{grp}_{fc}")
                            nc.tensor.transpose(
                                out=tps[:],
                                in_=hg[:, fc * P : (fc + 1) * P],
                                identity=ident_sb[:],
                            )
                            nc.scalar.activation(hgT[:, fc, :], tps[:], Copy)
                        x1 = wp.tile([P, 4, P], F16, tag="x1", name=f"x1_{grp}")
                        for uc in range(4):
                            x1ps = psB.tile([P, P], F32, tag="psB",
                                            name=f"x1ps{grp}_{uc}")
                            for kc in range(6):
                                rhs = (
                                    hgT[:, kc, :]
                                    if kc < 4
                                    else desct_sb[:, kc - 4, grp * P : (grp + 1) * P]
                                )
                                nc.tensor.matmul(
                                    out=x1ps[:],
                                    lhsT=lw1t_sb[:, kc, uc * P : (uc + 1) * P],
                                    rhs=rhs,
                                    start=(kc == 0),
                                    stop=(kc == 5),
                                )
                            nc.scalar.activation(
                                x1[:, uc, :], x1ps[:], Relu,
                                bias=lb1t_sb[:, uc : uc + 1],
                            )
                        x2ps = psB.tile([P, P], F32, tag="psB", name=f"x2ps{grp}")
                        for kc in range(4):
                            nc.tensor.matmul(
                                out=x2ps[:],
                                lhsT=lw2t_sb[:, kc, :],
                                rhs=x1[:, kc, :],
                                start=(kc == 0),
                                stop=(kc == 3),
                            )
                        x2 = wp.tile([P, P], F16, tag="x2", name=f"x2_{grp}")
                        nc.scalar.activation(x2[:], x2ps[:], Relu,
                                             bias=lb2t_sb[:, :1])
                        lgps = psB.tile([P, P], F32, tag="psB", name=f"lg{grp}")
                        nc.tensor.matmul(
                            out=lgps[:N_CLASSES, :],
                            lhsT=cwt_sb[:],
                            rhs=x2[:],
                            start=True,
                            stop=True,
                        )
                        lg = wp.tile([N_CLASSES, P], F32, tag="lg",
                                     name=f"lgsb{grp}")
                        nc.vector.tensor_tensor(
                            out=lg[:],
                            in0=lgps[:N_CLASSES, :],
                            in1=cbt_sb[:, :1].to_broadcast((N_CLASSES, P)),
                            op=add,
                        )
                        nc.sync.dma_start(out_d[:, grp * P : (grp + 1) * P], lg[:])

    nc.compile()
    return nc


# --------------------------------------------------------------------------
# Entry point
# --------------------------------------------------------------------------
def prepare(features, descriptors, src, dst, node2graph,
            W1, b1, W2, b2, lw1, lb1, lw2, lb2, cw, cb):
    """Preprocess + build; returns (pr, nc, in_maps)."""
    pr = _preprocess(features, descriptors, src, dst, node2graph)
    nc = _build(pr)

    f16 = np.float16
    iotaw = np.broadcast_to(np.arange(P, dtype=np.int8), (P, pr.RMAX, P)).copy()

    w1 = np.asarray(W1, np.float32).astype(f16)
    w2t = np.asarray(W2, np.float32).reshape(4, P, HID).transpose(1, 0, 2).astype(f16)
    w2t = np.ascontiguousarray(w2t)
    b1v = np.asarray(b1, np.float32).reshape(1, HID).astype(f16)
    b2v = np.asarray(b2, np.float32).reshape(1, HID).astype(f16)

    KD = 768
    lw1_pad = np.zeros((KD, U1), np.float32)
    lw1_pad[: HID + N_DESC, :500] = np.asarray(lw1, np.float32)
    lw1t = np.ascontiguousarray(
        lw1_pad.reshape(6, P, U1).transpose(1, 0, 2)).astype(f16)
    lb1_pad = np.zeros((U1,), np.float32)
    lb1_pad[:500] = np.asarray(lb1, np.float32)
    lb1t = np.ascontiguousarray(lb1_pad.reshape(4, P).T)
    lw2_pad = np.zeros((U1, U2), np.float32)
    lw2_pad[:500, :100] = np.asarray(lw2, np.float32)
    lw2t = np.ascontiguousarray(
        lw2_pad.reshape(4, P, U2).transpose(1, 0, 2)).astype(f16)
    lb2_pad = np.zeros((U2, 1), np.float32)
    lb2_pad[:100, 0] = np.asarray(lb2, np.float32)
    cw_pad = np.zeros((P, N_CLASSES), np.float32)
    cw_pad[:100] = np.asarray(cw, np.float32)
    cbt = np.asarray(cb, np.float32).reshape(N_CLASSES, 1)

    in_maps = []
    for r in range(NCORES):
        in_maps.append({
            "featsg": pr.featsg[r],
            "idx16_1": pr.idx16_1[r],
            "dstc1": pr.sc1.dstcp[r][:, :, None],
            "idx16_2": pr.idx16_2[r],
            "dstc2": pr.sc2.dstcp[r][:, :, None],
            "iotaw": iotaw,
            "ones1": np.ones((1, P), dtype=f16),
            "ident": np.eye(P, dtype=f16),
            "poolw": pr.poolw[r],
            "desct": pr.desct[r],
            "w1": w1,
            "w2t": w2t,
            "b1v": b1v,
            "b2v": b2v,
            "lw1t": lw1t,
            "lb1t": lb1t,
            "lw2t": lw2t,
            "lb2t": lb2_pad,
            "cwt": cw_pad.astype(f16),
            "cbt": cbt,
        })

    return pr, nc, in_maps


def kernel(features, descriptors, src, dst, node2graph,
           W1, b1, W2, b2, lw1, lb1, lw2, lb2, cw, cb, _run_opts=None):
    opts0 = dict(_run_opts or {})
    opts0.pop("_last_result", None)
    pr, nc, in_maps = prepare(features, descriptors, src, dst, node2graph,
                              W1, b1, W2, b2, lw1, lb1, lw2, lb2, cw, cb)
    res = run_bass_kernel_spmd(nc, in_maps, core_ids=list(range(NCORES)), **opts0)

    out = np.zeros((pr.B, N_CLASSES), dtype=np.float32)
    for r in range(NCORES):
        o = np.asarray(res.results[r]["out"])
        for gi, (g0, g1) in enumerate(pr.core_groups[r]):
            ncols = g1 - g0
            if ncols > 0:
                out[g0:g1] = o[:, gi * P : gi * P + ncols].T
    if _run_opts is not None:
        _run_opts["_last_result"] = res
    return out


# revision 59
# speedup vs baseline: 2.8048x; 1.0309x over previous
"""Distributed 2-layer GCN + graph pooling + MLP head on 8 TRN2 NeuronCores.

Collective-free data-parallel strategy (per the sharding hint):
  - Graphs (and their nodes, contiguously -- node2graph is sorted) are
    partitioned into 8 shards with ~equal node counts. Weights replicated.
  - Each core owns the edges whose dst node it owns. Layer 2 needs
    h1[src] for those edges; instead of an AllGather, each core computes
    layer 1 *locally* for exactly the source nodes its edges reference
    (~40% of all nodes). No collectives -> no cross-core barrier.
  - Layer 1 aggregates raw *features* (segment_sum commutes with the
    linear map); its edge gather moves 128-dim rows from a per-core
    DEDUPED feature table (unique sources only -> 3 int16 windows).
  - h1 (fp16) for the needed sources is written to a core-local DRAM
    table; layer 2 gathers 512-dim rows from it (2 windows).
  - Edge gathers use GPSIMD dma_gather, whose descriptor-generation
    time (the kernel's serial bottleneck) is proportional to the index
    count. Each (group, window) run is packed contiguously per core
    (block boundaries fall mid-slot, so no per-(block,window) chunk
    quantization) and only the run tail is padded (with table row 0,
    dst column -1): total gathered slots track the real edge count to
    within ~5%, while the slot layout stays uniform across cores.
  - Segment-sum on chip: per aggregation block, a 0/1 indicator built by
    DVE (is_equal of a [0,128) iota row vs per-edge dst values) over the
    block's slot range turns edge chunks into PE matmuls:
    aggT[f, n] += G_slot[:, f].T @ I_col[:, n].  Slots shared between
    blocks are matmul'd into both blocks' PSUMs; the per-block dst
    columns carry -1 for foreign edges, zeroing their indicator.
  - Graph mean-pooling is another indicator matmul with 1/count weights
    (host-precomputed fp16), fused after layer 2 per node block.
  - The MLP head runs feature-major so biases are per-partition.

Device compute fp16 (PSUM fp32); biases fp32; output fp32.
"""

import sys

sys.path.insert(0, "/opt/trn_rl_repo")

import numpy as np

import concourse.bass as bass
import concourse.mybir as mybir
import concourse.tile as tile
from concourse import bacc
from concourse.bass_utils import run_bass_kernel_spmd

P = 128
NCORES = 8
IN_DIM = 128
HID = 512
N_DESC = 200
N_CLASSES = 2
U1 = 512  # padded head hidden 1 (500 -> 512)
U2 = 128  # padded head hidden 2 (100 -> 128)
WSZ = 32768  # int16 gather window, layer-1 feature table
WSZ2 = 32768  # layer-2 h1-table window
SB1 = 64  # layer-1 gather-group slot budget (chunks of 128 edges)
SB2 = 20  # layer-2 gather-group slot budget

F16 = mybir.dt.float16
F32 = mybir.dt.float32
F8 = mybir.dt.float8e4
I16 = mybir.dt.int16
I8 = mybir.dt.int8
NP_F8 = mybir.dt.np(F8)
QUEUES = 4  # SWDGE queues to round-robin gathers over (1 for CoreSim runs)
FPAD = 256  # fp8 feature-table row (128 features + 128 zero pad -> 256B elem)


class Prep:
    pass


class Sched:
    pass


def _mk_sched(edata, nblocks, nwin, budget):
    """Contiguous-packing gather schedule, uniform across cores.

    edata[r] = (blk, win, dloc, gidx) int64 arrays per core: aggregation
    block, gather window, dst row-in-block [0,128), window-local gather
    row. Groups are consecutive block ranges sized so each group's total
    slot count stays <= budget. Returns a Sched with the group structure
    plus per-core packed int16 index streams (-1 tail padding) and
    per-block dst columns.
    """
    R = len(edata)

    cnt = np.zeros((R, nblocks, nwin), dtype=np.int64)
    for r, (blk, win, dloc, gidx) in enumerate(edata):
        np.add.at(cnt[r], (blk, win), 1)

    def group_slots(b0, b1):
        c = cnt[:, b0:b1, :].sum(axis=1)  # [R, nwin]
        return int(((c.max(axis=0) + P - 1) // P).sum())

    # greedy slot-budget grouping over consecutive blocks
    bounds = []
    b = 0
    while b < nblocks:
        b2 = b + 1
        while b2 < nblocks and group_slots(b, b2 + 1) <= budget:
            b2 += 1
        bounds.append((b, b2))
        b = b2
    ngroups = len(bounds)
    blk2grp = np.zeros(nblocks, dtype=np.int64)
    for g, (b0, b1) in enumerate(bounds):
        blk2grp[b0:b1] = g

    # pass 1: runs, per-(block,window) union slot ranges, indicator cols
    groups = []
    slot = 0
    col = 0
    rid = 0
    run_cnts = []  # per run: [R] real edge counts
    run_c0 = np.full((ngroups, nwin), -1, dtype=np.int64)
    s0_bw = np.full((nblocks, nwin), -1, dtype=np.int64)
    colbase_bw = np.full((nblocks, nwin), -1, dtype=np.int64)
    for g, (b0, b1) in enumerate(bounds):
        base = slot
        runs = []
        ranges = {b: [] for b in range(b0, b1)}  # (w, s0, s1) abs slots
        for w in range(nwin):
            c_r = cnt[:, b0:b1, w]  # [R, nb]
            tot = c_r.sum(axis=1)
            n = int((tot.max() + P - 1) // P)
            if n == 0:
                continue
            c0 = slot
            run_c0[g, w] = c0
            runs.append((w, c0, n, rid))
            run_cnts.append(tot.copy())
            rid += 1
            pfx = np.concatenate(
                [np.zeros((R, 1), dtype=np.int64), np.cumsum(c_r, axis=1)], axis=1
            )
            for bi in range(b1 - b0):
                m = c_r[:, bi] > 0
                if not m.any():
                    continue
                s0 = int((pfx[m, bi] // P).min()) + c0
                s1 = int(((pfx[m, bi + 1] - 1) // P).max()) + 1 + c0
                ranges[b0 + bi].append((w, s0, s1))
            slot += n
        if not runs:
            runs.append((0, slot, 1, rid))  # dummy run so the group tile exists
            run_cnts.append(np.zeros(R, dtype=np.int64))
            rid += 1
            run_c0[g, 0] = slot
            slot += 1
        # indicator columns per block (contiguous across its windows)
        bcols = {}
        bslots = {}
        for b in range(b0, b1):
            bcols[b] = col
            slots = []
            for w, s0, s1 in ranges[b]:
                s0_bw[b, w] = s0
                colbase_bw[b, w] = col + len(slots)
                slots.extend(range(s0 - base, s1 - base))
            if not slots:
                slots = [runs[0][1] - base]  # dummy col; dstc stays -1
            bslots[b] = slots
            col += len(slots)
        groups.append(
            dict(b0=b0, b1=b1, base=base, runs=runs, bcols=bcols, bslots=bslots,
                 cg=slot - base)
        )

    T, D = slot, col
    NRUNS = rid
    CG = max(gs["cg"] for gs in groups)
    RMAX = max(len(s) for gs in groups for s in gs["bslots"].values())

    # pass 2: per-core packed index streams and dst columns. Padding lanes
    # gather table row 0 (always valid); their dst columns stay -1 so the
    # indicator zeroes them. Every lane of every slot is written -> no
    # stale/NaN lanes, and the schedule stays a plain full-slot gather.
    idx_slot = np.zeros((R, T, P), dtype=np.int16)
    dstcp = np.full((R, P, D), -1, dtype=np.int8)
    nreal = np.zeros(R, dtype=np.int64)
    for r, (blk, win, dloc, gidx) in enumerate(edata):
        nreal[r] = len(blk)
        if len(blk) == 0:
            continue
        grp = blk2grp[blk]
        order = np.lexsort((blk, win, grp))
        blk_o, win_o = blk[order], win[order]
        dloc_o, gidx_o = dloc[order], gidx[order]
        grp_o = grp[order]
        key = grp_o * nwin + win_o
        starts = np.concatenate(
            [[0], np.cumsum(np.bincount(key, minlength=ngroups * nwin))]
        )
        pos = np.arange(len(key)) - starts[key]
        sabs = run_c0[grp_o, win_o] + pos // P
        lane = pos % P
        idx_slot[r][sabs, lane] = gidx_o.astype(np.int16)
        colv = colbase_bw[blk_o, win_o] + (sabs - s0_bw[blk_o, win_o])
        dstcp[r][lane, colv] = dloc_o.astype(np.int8)

    sc = Sched()
    sc.groups, sc.T, sc.D, sc.CG, sc.RMAX = groups, T, D, CG, RMAX
    sc.NRUNS = NRUNS
    sc.idx_slot, sc.dstcp, sc.nreal = idx_slot, dstcp, nreal
    return sc


def _pack_idx16(idx_slot):
    """[T, P] int16 slot-major stream -> dma_gather layout [128, T*8]."""
    wrapped = idx_slot.reshape(-1).reshape(-1, 16).T  # [16, T*8]
    return np.tile(wrapped, (8, 1))


# --------------------------------------------------------------------------
# Host-side preprocessing: partition, dedup tables, schedule
# --------------------------------------------------------------------------
def _preprocess(features, descriptors, src, dst, node2graph):
    pr = Prep()
    N = features.shape[0]
    B = descriptors.shape[0]

    n2g = np.asarray(node2graph).astype(np.int64)
    src = np.asarray(src).astype(np.int64)
    dst = np.asarray(dst).astype(np.int64)

    gstart = np.searchsorted(n2g, np.arange(B + 1))  # node range per graph

    # partition graphs into NCORES shards with ~equal node counts
    cuts = np.searchsorted(gstart, (np.arange(1, NCORES) * N) // NCORES)
    gcuts = np.concatenate([[0], cuts, [B]])

    # per-core pool groups of <=128 graphs; group nodes padded to 128-blocks
    core_groups = []
    for r in range(NCORES):
        g0, g1 = gcuts[r], gcuts[r + 1]
        groups = []
        g = g0
        while g < g1:
            ge = min(g + P, g1)
            groups.append((g, ge))
            g = ge
        if not groups:
            groups = [(g0, g0)]
        core_groups.append(groups)
    NG = max(len(gr) for gr in core_groups)

    blocks_per_group_core = np.zeros((NCORES, NG), dtype=np.int64)
    for r in range(NCORES):
        for gi, (g0, g1) in enumerate(core_groups[r]):
            nn = gstart[g1] - gstart[g0]
            blocks_per_group_core[r, gi] = max((nn + P - 1) // P, 1)
    bpg = blocks_per_group_core.max(axis=0)
    NB = int(bpg.sum())
    block_group = np.repeat(np.arange(NG), bpg)

    # padded-local index + owner of each node (layer-2 / pooling space)
    plocal = np.zeros(N, dtype=np.int64)
    owner = np.zeros(N, dtype=np.int64)
    group_base = np.concatenate([[0], np.cumsum(bpg) * P])
    for r in range(NCORES):
        for gi, (g0, g1) in enumerate(core_groups[r]):
            ns, ne = gstart[g0], gstart[g1]
            if ne > ns:
                plocal[ns:ne] = group_base[gi] + np.arange(ne - ns)
                owner[ns:ne] = r

    # per-core needed-source sets (sorted unique srcs of locally-owned edges)
    e_owner = owner[dst]
    uniq_r, l2_edges = [], []
    for r in range(NCORES):
        es = np.nonzero(e_owner == r)[0]
        uq = np.unique(src[es])
        uniq_r.append(uq)
        l2_edges.append(es)
    SBLK = max((len(u) + P - 1) // P for u in uniq_r)
    SROWS = SBLK * P
    NWIN2 = (SROWS + WSZ2 - 1) // WSZ2

    # layer-1 edges per core: all graph edges whose dst is a needed source.
    # Needed sources are ranked by in-degree (descending) so per-block edge
    # counts are similar across cores. The gather table is the per-core
    # deduped set of source features (unique srcs of layer-1 edges).
    l1_dat = []
    rank_maps = []
    usrcs = []
    for r in range(NCORES):
        uq = uniq_r[r]
        pos = np.searchsorted(uq, dst)
        pos_cl = np.minimum(pos, len(uq) - 1)
        m = uq[pos_cl] == dst  # edge's dst is in the needed set
        e1 = np.nonzero(m)[0]
        orank = pos[e1]
        indeg = np.bincount(orank, minlength=len(uq))
        order = np.argsort(-indeg, kind="stable")
        newrank = np.empty(len(uq), dtype=np.int64)
        newrank[order] = np.arange(len(uq))
        rank_maps.append(newrank)
        rank = newrank[orank]  # aggregation target (local row in h1 table)
        usrc = np.unique(src[e1])
        usrcs.append(usrc)
        gidx = np.searchsorted(usrc, src[e1])
        l1_dat.append((rank // P, gidx // WSZ, rank % P, gidx % WSZ))
    U = max(len(u) for u in usrcs)
    UPAD = ((U + P - 1) // P) * P
    NWIN1 = (UPAD + WSZ - 1) // WSZ

    sc1 = _mk_sched(l1_dat, SBLK, NWIN1, SB1)

    # layer-2 edges per core: local edges; src -> rank in needed set
    l2_dat = []
    for r in range(NCORES):
        es = l2_edges[r]
        dpl = plocal[dst[es]]
        rank = rank_maps[r][np.searchsorted(uniq_r[r], src[es])]
        l2_dat.append((dpl // P, rank // WSZ2, dpl % P, rank % WSZ2))

    sc2 = _mk_sched(l2_dat, NB, NWIN2, SB2)

    RMAX = max(sc1.RMAX, sc2.RMAX)

    idx16_1 = np.stack([_pack_idx16(sc1.idx_slot[r]) for r in range(NCORES)])
    idx16_2 = np.stack([_pack_idx16(sc2.idx_slot[r]) for r in range(NCORES)])

    # per-core deduped feature tables (fp8, rows padded to a 256B element)
    featsg = np.zeros((NCORES, UPAD, FPAD), dtype=NP_F8)
    f8feat = np.asarray(features, np.float32).astype(NP_F8)
    for r in range(NCORES):
        featsg[r, : len(usrcs[r]), :IN_DIM] = f8feat[usrcs[r]]

    # pooling weights [P(node-in-block), NB, P(graph-in-group)] = 1/count
    gcount = np.diff(gstart)
    inv_cnt = (1.0 / np.maximum(gcount, 1)).astype(np.float32)
    poolw = np.zeros((NCORES, P, NB, P), dtype=np.float16)
    for r in range(NCORES):
        for gi, (g0, g1) in enumerate(core_groups[r]):
            ns, ne = gstart[g0], gstart[g1]
            if ne <= ns:
                continue
            nodes = np.arange(ns, ne)
            pl = plocal[nodes]
            poolw[r, pl % P, pl // P, n2g[nodes] - g0] = inv_cnt[n2g[nodes]]

    # descriptors, feature-major, padded [P, 2, NG*P]
    GPAD = NG * P
    desc_pad = np.zeros((B, 256), dtype=np.float32)
    desc_pad[:, :N_DESC] = np.asarray(descriptors, dtype=np.float32)
    desct = np.zeros((NCORES, P, 2, GPAD), dtype=np.float16)
    for r in range(NCORES):
        for gi, (g0, g1) in enumerate(core_groups[r]):
            ncols = g1 - g0
            if ncols <= 0:
                continue
            blockd = desc_pad[g0:g1].T.reshape(2, P, ncols).transpose(1, 0, 2)
            desct[r, :, :, gi * P : gi * P + ncols] = blockd.astype(np.float16)

    pr.N, pr.B = N, B
    pr.NG, pr.NB, pr.GPAD = NG, NB, GPAD
    pr.SBLK, pr.SROWS, pr.NWIN2 = SBLK, SROWS, NWIN2
    pr.UPAD, pr.NWIN1 = UPAD, NWIN1
    pr.RMAX = RMAX
    pr.sc1, pr.sc2 = sc1, sc2
    pr.block_group = block_group
    pr.core_groups = core_groups
    pr.idx16_1, pr.idx16_2 = idx16_1, idx16_2
    pr.featsg, pr.poolw, pr.desct = featsg, poolw, desct
    return pr


# --------------------------------------------------------------------------
# Bass program builder (single SPMD program; per-core data via in_maps)
# --------------------------------------------------------------------------
def _build(pr):
    nc = bacc.Bacc("TRN2", target_bir_lowering=False, num_devices=NCORES,
                   num_swdge_queues=4)

    NB, NG, GPAD = pr.NB, pr.NG, pr.GPAD
    SROWS, NWIN2 = pr.SROWS, pr.NWIN2
    UPAD = pr.UPAD
    sc1, sc2 = pr.sc1, pr.sc2
    RMAX = pr.RMAX
    block_group = pr.block_group

    featsg_d = nc.dram_tensor("featsg", [UPAD, FPAD], F8, kind="ExternalInput")
    idx1_d = nc.dram_tensor("idx16_1", [P, sc1.T * 8], I16, kind="ExternalInput")
    dstc1_d = nc.dram_tensor("dstc1", [P, sc1.D, 1], I8, kind="ExternalInput")
    idx2_d = nc.dram_tensor("idx16_2", [P, sc2.T * 8], I16, kind="ExternalInput")
    dstc2_d = nc.dram_tensor("dstc2", [P, sc2.D, 1], I8, kind="ExternalInput")
    iotaw_d = nc.dram_tensor("iotaw", [P, RMAX, P], I8, kind="ExternalInput")
    ones1_d = nc.dram_tensor("ones1", [1, P], F16, kind="ExternalInput")
    ident_d = nc.dram_tensor("ident", [P, P], F16, kind="ExternalInput")
    poolw_d = nc.dram_tensor("poolw", [P, NB, P], F16, kind="ExternalInput")
    desct_d = nc.dram_tensor("desct", [P, 2, GPAD], F16, kind="ExternalInput")
    w1_d = nc.dram_tensor("w1", [P, HID], F16, kind="ExternalInput")
    w2t_d = nc.dram_tensor("w2t", [P, 4, HID], F16, kind="ExternalInput")
    b1_d = nc.dram_tensor("b1v", [1, HID], F16, kind="ExternalInput")
    b2_d = nc.dram_tensor("b2v", [1, HID], F16, kind="ExternalInput")
    lw1t_d = nc.dram_tensor("lw1t", [P, 6, U1], F16, kind="ExternalInput")
    lb1t_d = nc.dram_tensor("lb1t", [P, 4], F32, kind="ExternalInput")
    lw2t_d = nc.dram_tensor("lw2t", [P, 4, U2], F16, kind="ExternalInput")
    lb2t_d = nc.dram_tensor("lb2t", [P, 1], F32, kind="ExternalInput")
    cwt_d = nc.dram_tensor("cwt", [P, N_CLASSES], F16, kind="ExternalInput")
    cbt_d = nc.dram_tensor("cbt", [N_CLASSES, 1], F32, kind="ExternalInput")
    out_d = nc.dram_tensor("out", [N_CLASSES, GPAD], F32, kind="ExternalOutput")

    is_eq = mybir.AluOpType.is_equal
    add = mybir.AluOpType.add
    Copy = mybir.ActivationFunctionType.Copy
    Relu = mybir.ActivationFunctionType.Relu

    with tile.TileContext(nc) as tc:
        with (
            tc.tile_pool(name="const", bufs=1) as cp,
            tc.tile_pool(name="gath1", bufs=3) as gp1,
            tc.tile_pool(name="gath2", bufs=3) as gp2,
            tc.tile_pool(name="ind", bufs=3) as ip,
            tc.tile_pool(name="work", bufs=3) as wp,
            tc.tile_pool(name="psA", bufs=2, space="PSUM") as psA,
            tc.tile_pool(name="psA2", bufs=3, space="PSUM") as psA2,
            tc.tile_pool(name="psB", bufs=2, space="PSUM") as psB,
            tc.tile_pool(name="psP", bufs=1, space="PSUM") as psP,
            tc.tile_pool(name="dram", bufs=1, space="DRAM") as dp,
        ):
            h1_d = dp.tile([SROWS, HID], F8)

            idx1_sb = cp.tile([P, sc1.T * 8], I16)
            nc.sync.dma_start(idx1_sb[:], idx1_d[:])
            dstc1_sb = cp.tile([P, sc1.D, 1], I8)
            nc.sync.dma_start(dstc1_sb[:], dstc1_d[:])
            idx2_sb = cp.tile([P, sc2.T * 8], I16)
            nc.sync.dma_start(idx2_sb[:], idx2_d[:])
            dstc2_sb = cp.tile([P, sc2.D, 1], I8)
            nc.sync.dma_start(dstc2_sb[:], dstc2_d[:])
            iotaw_sb = cp.tile([P, RMAX, P], I8)
            nc.sync.dma_start(iotaw_sb[:], iotaw_d[:])
            ones1_sb = cp.tile([1, P], F16)
            nc.sync.dma_start(ones1_sb[:], ones1_d[:])
            ident_sb = cp.tile([P, P], F16)
            nc.sync.dma_start(ident_sb[:], ident_d[:])
            poolw_sb = cp.tile([P, NB, P], F16)
            nc.sync.dma_start(poolw_sb[:], poolw_d[:])
            desct_sb = cp.tile([P, 2, GPAD], F16)
            nc.sync.dma_start(desct_sb[:], desct_d[:])
            w1_sb = cp.tile([P, HID], F16)
            nc.sync.dma_start(w1_sb[:], w1_d[:])
            w2t_sb = cp.tile([P, 4, HID], F16)
            nc.sync.dma_start(w2t_sb[:], w2t_d[:])
            b1_sb = cp.tile([1, HID], F16)
            nc.sync.dma_start(b1_sb[:], b1_d[:])
            b2_sb = cp.tile([1, HID], F16)
            nc.sync.dma_start(b2_sb[:], b2_d[:])
            lw1t_sb = cp.tile([P, 6, U1], F16)
            nc.sync.dma_start(lw1t_sb[:], lw1t_d[:])
            lb1t_sb = cp.tile([P, 4], F32)
            nc.sync.dma_start(lb1t_sb[:], lb1t_d[:])
            lw2t_sb = cp.tile([P, 4, U2], F16)
            nc.sync.dma_start(lw2t_sb[:], lw2t_d[:])
            lb2t_sb = cp.tile([P, 1], F32)
            nc.sync.dma_start(lb2t_sb[:], lb2t_d[:])
            cwt_sb = cp.tile([P, N_CLASSES], F16)
            nc.sync.dma_start(cwt_sb[:], cwt_d[:])
            cbt_sb = cp.tile([N_CLASSES, 1], F32)
            nc.sync.dma_start(cbt_sb[:], cbt_d[:])

            # round-robin gathers across the 4 SWDGE queues: each queue's
            # descriptor generation runs on a different Q7 core pair
            qrr = [0]

            def gather_group(gi, gs, gpool, cg, table, nrows, idx_sb, elem,
                             wsz, name):
                gt = gpool.tile([P, cg, elem], F8, tag=f"g{elem}",
                                name=f"{name}_{gs['base']}")
                for w, c0, n, rid in gs["runs"]:
                    lo, hi = w * wsz, min((w + 1) * wsz, nrows)
                    nc.gpsimd.dma_gather(
                        out_ap=gt[:, c0 - gs["base"] : c0 - gs["base"] + n, :],
                        in_ap=table[lo:hi, :],
                        idxs_ap=idx_sb[:, c0 * 8 : (c0 + n) * 8],
                        num_idxs=n * P,
                        num_idxs_reg=n * P,
                        elem_size=elem,
                        single_packet=False,
                        queue_num=qrr[0] % QUEUES,
                    )
                    qrr[0] += 1
                return gt

            def indicator(gs, b, dstc_sb, name):
                slots = gs["bslots"][b]
                K = len(slots)
                c0 = gs["bcols"][b]
                ind = ip.tile([P, RMAX, P], F8, tag="ind", name=name)
                nc.vector.tensor_tensor(
                    out=ind[:, :K, :],
                    in0=iotaw_sb[:, :K, :],
                    in1=dstc_sb[:, c0 : c0 + K, :].to_broadcast((P, K, P)),
                    op=is_eq,
                )
                return ind, slots

            # ================= Layer 1 (needed sources) =================
            for gi, gs in enumerate(sc1.groups):
                g1 = gather_group(gi, gs, gp1, sc1.CG, featsg_d, UPAD, idx1_sb,
                                  FPAD, WSZ, "g1")
                for b in range(gs["b0"], gs["b1"]):
                    ind1, slots = indicator(gs, b, dstc1_sb, f"i1_{b}")
                    aggT = psA.tile([P, P], F32, tag="psA", name=f"agg1_{b}")
                    for i, s in enumerate(slots):
                        nc.tensor.matmul(
                            out=aggT[:],
                            lhsT=g1[:, s, :IN_DIM],
                            rhs=ind1[:, i, :],
                            start=(i == 0),
                            stop=(i == len(slots) - 1),
                        )
                    aggT_sb = wp.tile([P, IN_DIM], F16, tag="agg1sb",
                                      name=f"agg1sb{b}")
                    nc.scalar.activation(aggT_sb[:], aggT[:], Copy)
                    h1ps = psB.tile([P, HID], F32, tag="psB", name=f"h1ps{b}")
                    nc.tensor.matmul(out=h1ps[:], lhsT=aggT_sb[:], rhs=w1_sb[:],
                                     start=True, stop=False)
                    nc.tensor.matmul(out=h1ps[:], lhsT=ones1_sb[:], rhs=b1_sb[:],
                                     start=False, stop=True)
                    h1 = wp.tile([P, HID], F8, tag="h1", name=f"h1_{b}")
                    nc.scalar.activation(h1[:], h1ps[:], Relu)
                    nc.sync.dma_start(h1_d[b * P : (b + 1) * P, :], h1[:])

            # ================= Layer 2 + pooling + head =================
            pool_ps = None
            for gi, gs in enumerate(sc2.groups):
                g2 = gather_group(gi, gs, gp2, sc2.CG, h1_d, SROWS, idx2_sb,
                                  HID, WSZ2, "g2")  # fp8 rows: 512B elements
                for b in range(gs["b0"], gs["b1"]):
                    grp = int(block_group[b])
                    first_in_grp = b == 0 or block_group[b - 1] != grp
                    last_in_grp = b == NB - 1 or block_group[b + 1] != grp

                    ind2, slots = indicator(gs, b, dstc2_sb, f"i2_{b}")
                    # wide-rhs aggregation: one matmul per slot, agg in
                    # node-major [n, 512] form (transposed back below)
                    agg = psA2.tile([P, HID], F32, tag="psA2", name=f"agg2_{b}")
                    for i, s in enumerate(slots):
                        nc.tensor.matmul(
                            out=agg[:],
                            lhsT=ind2[:, i, :],
                            rhs=g2[:, s, :],
                            start=(i == 0),
                            stop=(i == len(slots) - 1),
                        )
                    agg_sb = wp.tile([P, HID], F16, tag="agg2sb",
                                     name=f"agg2sb{b}")
                    nc.scalar.activation(agg_sb[:, : HID // 2],
                                         agg[:, : HID // 2], Copy)
                    nc.vector.tensor_copy(out=agg_sb[:, HID // 2 :],
                                          in_=agg[:, HID // 2 :])
                    aggT_sb = wp.tile([P, 4, P], F16, tag="aggT2",
                                      name=f"aggT2_{b}")
                    for fc in range(4):
                        tps = psB.tile([P, P], F16, tag="psB",
                                       name=f"t2_{b}_{fc}")
                        nc.tensor.transpose(
                            out=tps[:],
                            in_=agg_sb[:, fc * P : (fc + 1) * P],
                            identity=ident_sb[:],
                        )
                        if fc % 2 == 0:
                            nc.scalar.activation(aggT_sb[:, fc, :], tps[:],
                                                 Copy)
                        else:
                            nc.vector.tensor_copy(out=aggT_sb[:, fc, :],
                                                  in_=tps[:])
                    h2ps = psB.tile([P, HID], F32, tag="psB", name=f"h2ps{b}")
                    for fc in range(4):
                        nc.tensor.matmul(
                            out=h2ps[:],
                            lhsT=aggT_sb[:, fc, :],
                            rhs=w2t_sb[:, fc, :],
                            start=(fc == 0),
                            stop=False,
                        )
                    nc.tensor.matmul(out=h2ps[:], lhsT=ones1_sb[:], rhs=b2_sb[:],
                                     start=False, stop=True)
                    h2 = wp.tile([P, HID], F16, tag="h2", name=f"h2_{b}")
                    nc.scalar.activation(h2[:], h2ps[:], Relu)

                    if first_in_grp:
                        pool_ps = psP.tile([P, HID], F32, tag="psP",
                                           name=f"pool{grp}")
                    nc.tensor.matmul(
                        out=pool_ps[:],
                        lhsT=poolw_sb[:, b, :],
                        rhs=h2[:],
                        start=first_in_grp,
                        stop=last_in_grp,
                    )

                    if last_in_grp:
                        hg = wp.tile([P, HID], F16, tag="hg", name=f"hg{grp}")
                        nc.scalar.activation(hg[:], pool_ps[:], Copy)
                        hgT = wp.tile([P, 4, P], F16, tag="hgT", name=f"hgT{grp}")
                        for fc in range(4):
                            tps = psB.tile([P, P], F16, tag="psB",
                                           name=f"tps{grp}_{fc}")
                            nc.tensor.transpose(
                                out=tps[:],
                                in_=hg[:, fc * P : (fc + 1) * P],
                                identity=ident_sb[:],
                            )
                            nc.scalar.activation(hgT[:, fc, :], tps[:], Copy)
                        x1 = wp.tile([P, 4, P], F16, tag="x1", name=f"x1_{grp}")
                        for uc in range(4):
                            x1ps = psB.tile([P, P], F32, tag="psB",
                                            name=f"x1ps{grp}_{uc}")
                            for kc in range(6):
                                rhs = (
                                    hgT[:, kc, :]
                                    if kc < 4
                                    else desct_sb[:, kc - 4, grp * P : (grp + 1) * P]
                                )
                                nc.tensor.matmul(
                                    out=x1ps[:],
                                    lhsT=lw1t_sb[:, kc, uc * P : (uc + 1) * P],
                                    rhs=rhs,
                                    start=(kc == 0),
                                    stop=(kc == 5),
                                )
                            nc.scalar.activation(
                                x1[:, uc, :], x1ps[:], Relu,
                                bias=lb1t_sb[:, uc : uc + 1],
                            )
                        x2ps = psB.tile([P, P], F32, tag="psB", name=f"x2ps{grp}")
                        for kc in range(4):
                            nc.tensor.matmul(
                                out=x2ps[:],
                                lhsT=lw2t_sb[:, kc, :],
                                rhs=x1[:, kc, :],
                                start=(kc == 0),
                                stop=(kc == 3),
                            )
                        x2 = wp.tile([P, P], F16, tag="x2", name=f"x2_{grp}")
                        nc.scalar.activation(x2[:], x2ps[:], Relu,
                                             bias=lb2t_sb[:, :1])
                        lgps = psB.tile([P, P], F32, tag="psB", name=f"lg{grp}")
                        nc.tensor.matmul(
                            out=lgps[:N_CLASSES, :],
                            lhsT=cwt_sb[:],
                            rhs=x2[:],
                            start=True,
                            stop=True,
                        )
                        lg = wp.tile([N_CLASSES, P], F32, tag="lg",
                                     name=f"lgsb{grp}")
                        nc.vector.tensor_tensor(
                            out=lg[:],
                            in0=lgps[:N_CLASSES, :],
                            in1=cbt_sb[:, :1].to_broadcast((N_CLASSES, P)),
                            op=add,
                        )
                        nc.sync.dma_start(out_d[:, grp * P : (grp + 1) * P], lg[:])

    nc.compile()
    return nc


# --------------------------------------------------------------------------
# Entry point
# --------------------------------------------------------------------------
def prepare(features, descriptors, src, dst, node2graph,
            W1, b1, W2, b2, lw1, lb1, lw2, lb2, cw, cb):
    """Preprocess + build; returns (pr, nc, in_maps)."""
    pr = _preprocess(features, descriptors, src, dst, node2graph)
    nc = _build(pr)

    f16 = np.float16
    iotaw = np.broadcast_to(np.arange(P, dtype=np.int8), (P, pr.RMAX, P)).copy()

    w1 = np.asarray(W1, np.float32).astype(f16)
    w2t = np.asarray(W2, np.float32).reshape(4, P, HID).transpose(1, 0, 2).astype(f16)
    w2t = np.ascontiguousarray(w2t)
    b1v = np.asarray(b1, np.float32).reshape(1, HID).astype(f16)
    b2v = np.asarray(b2, np.float32).reshape(1, HID).astype(f16)

    KD = 768
    lw1_pad = np.zeros((KD, U1), np.float32)
    lw1_pad[: HID + N_DESC, :500] = np.asarray(lw1, np.float32)
    lw1t = np.ascontiguousarray(
        lw1_pad.reshape(6, P, U1).transpose(1, 0, 2)).astype(f16)
    lb1_pad = np.zeros((U1,), np.float32)
    lb1_pad[:500] = np.asarray(lb1, np.float32)
    lb1t = np.ascontiguousarray(lb1_pad.reshape(4, P).T)
    lw2_pad = np.zeros((U1, U2), np.float32)
    lw2_pad[:500, :100] = np.asarray(lw2, np.float32)
    lw2t = np.ascontiguousarray(
        lw2_pad.reshape(4, P, U2).transpose(1, 0, 2)).astype(f16)
    lb2_pad = np.zeros((U2, 1), np.float32)
    lb2_pad[:100, 0] = np.asarray(lb2, np.float32)
    cw_pad = np.zeros((P, N_CLASSES), np.float32)
    cw_pad[:100] = np.asarray(cw, np.float32)
    cbt = np.asarray(cb, np.float32).reshape(N_CLASSES, 1)

    in_maps = []
    for r in range(NCORES):
        in_maps.append({
            "featsg": pr.featsg[r],
            "idx16_1": pr.idx16_1[r],
            "dstc1": pr.sc1.dstcp[r][:, :, None],
            "idx16_2": pr.idx16_2[r],
            "dstc2": pr.sc2.dstcp[r][:, :, None],
            "iotaw": iotaw,
            "ones1": np.ones((1, P), dtype=f16),
            "ident": np.eye(P, dtype=f16),
            "poolw": pr.poolw[r],
            "desct": pr.desct[r],
            "w1": w1,
            "w2t": w2t,
            "b1v": b1v,
            "b2v": b2v,
            "lw1t": lw1t,
            "lb1t": lb1t,
            "lw2t": lw2t,
            "lb2t": lb2_pad,
            "cwt": cw_pad.astype(f16),
            "cbt": cbt,
        })

    return pr, nc, in_maps


def kernel(features, descriptors, src, dst, node2graph,
           W1, b1, W2, b2, lw1, lb1, lw2, lb2, cw, cb, _run_opts=None):
    opts0 = dict(_run_opts or {})
    opts0.pop("_last_result", None)
    pr, nc, in_maps = prepare(features, descriptors, src, dst, node2graph,
                              W1, b1, W2, b2, lw1, lb1, lw2, lb2, cw, cb)
    res = run_bass_kernel_spmd(nc, in_maps, core_ids=list(range(NCORES)), **opts0)

    out = np.zeros((pr.B, N_CLASSES), dtype=np.float32)
    for r in range(NCORES):
        o = np.asarray(res.results[r]["out"])
        for gi, (g0, g1) in enumerate(pr.core_groups[r]):
            ncols = g1 - g0
            if ncols > 0:
                out[g0:g1] = o[:, gi * P : gi * P + ncols].T
    if _run_opts is not None:
        _run_opts["_last_result"] = res
    return out
